# revision 8
# baseline (speedup 1.0000x reference)
"""AtomicConvolution Trainium2 kernel (8 NeuronCores, data-parallel over B).

Pipeline per core (2 complexes, 4096 atoms, layout [par=(a_lo*32+m), free=a_hi]):
  host gathers neighbor coords -> R on DVE -> radial fn split into table-set
  batched ACT phases (all Exp, then all Sin; DVE/GpSimd do the per-p affine
  preps) writing contiguous-per-p rsf (bf16) -> per-atom-group masked type
  reduction on TensorE (block-diagonal 0/1 weights from is_equal on GpSimd)
  -> PSUM -> sym kept in SBUF (bf16, PSUM-native layout; host unscrambles)
  -> per-atom BN stats via selector matmuls + AllReduce -> normalize in
  place -> one output DMA.  Stored rsf = -2*f; fixed in the BN epilogue
  (eps*4, negated inv).
"""
import sys
import types
import numpy as np

ATOM_TYPES = (1, 6, 7, 8, 16)
BN_EPS = 1e-5
B, N, M, P = 16, 2048, 32, 48
T = len(ATOM_TYPES)
NC_CORES = 8
B_LOC = B // NC_CORES            # 2 complexes per core
A = B_LOC * N                    # 4096 atoms per core
AH = A // 4                      # 1024 free columns
C_OUT = P * T                    # 240 channels
NTILE = 26                       # ceil(1024 / 40) psum tiles
OUTW = NTILE * 480               # 12480 staged output columns

_TRACE = [False]

# ---------------------------------------------------------------- env patches
import concourse.bass as bass
import concourse.mybir as mybir
import concourse.tile as tile
import concourse.bass_utils as bu
from concourse.bass_utils import run_bass_kernel_spmd
from concourse.tile import TileContext, add_dep_helper


def _patch_tile_tail_drain():
    tile_mod = tile
    ScopedClock = None
    for _n in dir(tile_mod):
        if "ScopedClock" in _n:
            ScopedClock = getattr(tile_mod, _n)

    def _drain(self, tick_clock, wait_clock):
        nc = self.nc
        nops = [nc.sync.nop(nofuse=True) for _ in range(30)]
        drain_inst = nc.sync.drain()
        wait_clock.add_sem_waits(
            drain_inst.ins, ScopedClock({None: tick_clock.global_clock})
        )
        si = drain_inst.ins.sync_info
        if si is not None and si.on_wait and len(si.on_wait) > 1:
            waits = list(si.on_wait)
            si.on_wait = waits[:1]
            rest = waits[1:]
            assert len(rest) <= len(nops)
            for i, nop in enumerate(nops):
                chunk = rest[i:i + 1]
                if not chunk:
                    break
                nsi = nop.ins.sync_info
                if nsi is None:
                    nop.ins.sync_info = mybir.SyncInfo(on_wait=chunk, on_update=[])
                else:
                    nsi.on_wait = chunk
        nc.all_engine_barrier()
        popped = nc._tile_sem_poison_stack.pop()
        assert popped is self._sem_poison
        nc.clear_and_free_semaphores(list(self.sems.allocated().values()))
        nc.all_engine_barrier()

    TileContext._drain_and_barrier = _drain


WAIT_CAP = 1


def _make_spare_nops(nc, counts):
    # SP-engine carrier nops: the only engine whose sequencer NoOp reliably
    # encodes with sem waits in this walrus build.
    return {"carriers": [nc.sync.nop(nofuse=True) for _ in range(4000)]}


def _fix_sync_waits(nc, spares, relay):
    clr = nc.sync.sem_clear(relay)
    relay_count = [0]
    carriers = spares["carriers"]
    spare_names = {c.ins.name for c in carriers}
    # move the freshly-appended clear to the very beginning of the first block
    fn0 = nc.m.functions[0]
    for bb in fn0.blocks:
        if clr.ins in bb.instructions:
            bb.instructions.remove(clr.ins)
    fn0.blocks[0].instructions.insert(0, clr.ins)
    for fn in nc.m.functions:
        for bb in fn.blocks:
            bb.instructions[:] = [
                i for i in bb.instructions if i.name not in spare_names
            ]
    for fn in nc.m.functions:
        for bb in fn.blocks:
            new = []
            for inst in bb.instructions:
                si = inst.sync_info
                waits = list(si.on_wait) if si is not None and si.on_wait else []
                if len(waits) > WAIT_CAP:
                    for w in waits:
                        assert carriers, "out of relay carriers"
                        car = carriers.pop()
                        car.then_inc(relay, 1)
                        car.ins.sync_info.on_wait = [w]
                        relay_count[0] += 1
                        new.append(car.ins)
                    si.on_wait = [mybir.SyncWait(
                        sync_type="semaphore", id=relay.num,
                        ant_name=relay.name, wait_mode="sem-ge-imm",
                        wait_value=relay_count[0], wait_reg=None)]
                new.append(inst)
            bb.instructions[:] = new


def _patch_walrus_dyndma(size=16384):
    if getattr(bu.run_command, "_walrus_patched", False):
        return
    _orig = bu.run_command

    def run2(cmd, cwd=None, **kw):
        try:
            if cmd and "walrus_driver" in str(cmd[0]) and any(
                "codegen" in str(c) for c in cmd
            ):
                cmd = list(cmd) + [
                    f"--dynamic-dma-scratch-size-per-partition={size}"
                ]
        except Exception:
            pass
        return _orig(cmd, cwd=cwd, **kw)

    run2._walrus_patched = True
    bu.run_command = run2


def _install_ntff_hook():
    if "antenv.axon_hooks" in sys.modules:
        return
    try:
        from trn_agent_boot.trn_boot import _ntff_profile_via_ctypes
        hook = _ntff_profile_via_ctypes("/opt/axon/libaxon_pjrt.so")
    except Exception:
        hook = None
    m = types.ModuleType("antenv.axon_hooks")
    m._hook = hook
    m.get_axon_ntff_profile_hook = lambda: m._hook
    m.set_axon_ntff_profile_hook = lambda h: setattr(m, "_hook", h)
    sys.modules["antenv.axon_hooks"] = m
    try:
        import antenv
        antenv.axon_hooks = m
    except Exception:
        pass


_patch_tile_tail_drain()
_patch_walrus_dyndma()
_install_ntff_hook()

DT = mybir.dt


def _mk_ap(base_ap, off_elems, free_dims):
    return bass.AP(base_ap.tensor, base_ap.offset + off_elems,
                   [base_ap.ap[0]] + free_dims)


# ---------------------------------------------------------------- bass build
def build_nc(rcv, rsv, rev):
    nc = bass.Bass(dynamic_dma_scratch_size=8192)
    f32, bf16, i32 = DT.float32, DT.bfloat16, DT.int32

    PIH = float(np.pi / 2.0)
    AL = mybir.AluOpType
    AF = mybir.ActivationFunctionType

    def register_const(value, dtype=f32):
        value = float(value)
        if (dtype, value) in nc.const_aps.aps:
            return
        t = nc.alloc_sbuf_tensor(
            f"uconst-{dtype.name}-{value}", [128, 1], dtype)
        nc.gpsimd.memset(t.ap(), value)
        nc.const_aps.aps[(dtype, value)] = t.ap()

    # Exp bias consts: -re*rs^2 per p; Sin bias: -pi/2
    for p in range(P):
        register_const(-float(rev[p]) * float(rsv[p]) * float(rsv[p]))
    register_const(-PIH)
    nc.all_engine_barrier()

    rr_ext = nc.declare_dram_parameter("rr", [128, AH], f32, isOutput=False)
    zt_ext = nc.declare_dram_parameter("zt", [128, AH], bf16, isOutput=False)
    tc_ext = nc.declare_dram_parameter("tcode", [128, 32], bf16, isOutput=False)
    s8_ext = nc.declare_dram_parameter("sel8", [128, 8], f32, isOutput=False)
    sb_ext = nc.declare_dram_parameter("selbc", [8, 128], f32, isOutput=False)
    out_ext = nc.declare_dram_parameter("out", [128, OUTW], bf16, isOutput=True)

    st_in = nc.dram_tensor("st_in", [8, 520], f32)
    st_out = nc.dram_tensor("st_out", [8, 520], f32, addr_space="Shared")

    relay_sem = nc.semaphore("wait_relay").__enter__()
    with TileContext(nc) as tc:
        spares = _make_spare_nops(nc, {})
        with tc.tile_pool(name="main", bufs=1) as pool, \
             tc.tile_pool(name="work", bufs=2) as wpool, \
             tc.tile_pool(name="psum", bufs=2, space="PSUM") as ppool, \
             tc.tile_pool(name="pstat", bufs=1, space="PSUM") as spool:

            # ---- loads
            zt = pool.tile([128, AH], bf16)
            nc.sync.dma_start(out=zt[:], in_=zt_ext[:])
            tcode = pool.tile([128, 32], bf16)
            nc.sync.dma_start(out=tcode[:], in_=tc_ext[:])
            sel8 = pool.tile([128, 8], f32)
            nc.sync.dma_start(out=sel8[:], in_=s8_ext[:])
            selbc = pool.tile([128, 128], f32)
            nc.sync.dma_start(out=selbc[0:8, :], in_=sb_ext[:])
            rr = pool.tile([128, AH], f32)
            nc.sync.dma_start(out=rr[:], in_=rr_ext[:])
            r2 = pool.tile([128, AH], f32)
            nc.vector.tensor_tensor(out=r2[:], in0=rr[:], in1=rr[:],
                                    op=AL.mult)

            # rsf layout: contiguous per p -> col = p*AH + a_hi
            rsf = pool.tile([128, P * AH], bf16)

            # ---- phase A: all Exp.  kk_p = exp(-re*(r2 - 2 rs rr) - re rs^2)
            for p in range(P):
                rs_p, re_p = float(rsv[p]), float(rev[p])
                yt = wpool.tile([128, AH], f32, tag="yt")
                nc.vector.scalar_tensor_tensor(
                    out=yt[:], in0=rr[:], scalar=-2.0 * rs_p, in1=r2[:],
                    op0=AL.mult, op1=AL.add)
                nc.scalar.activation(
                    out=rsf[:, p * AH:(p + 1) * AH], in_=yt[:], func=AF.Exp,
                    scale=-re_p, bias=-re_p * rs_p * rs_p)

            # ---- phase B: all Sin, merge (csn - 1) * kk = -2*f_p in place
            for p in range(P):
                rc_p = float(rcv[p])
                rt = wpool.tile([128, AH], f32, tag="rt")
                nc.gpsimd.tensor_scalar(
                    out=rt[:], in0=rr[:], scalar1=rc_p,
                    scalar2=float(np.pi / rc_p), op0=AL.min, op1=AL.mult)
                cs = wpool.tile([128, AH], bf16, tag="cs")
                nc.scalar.activation(out=cs[:], in_=rt[:], func=AF.Sin,
                                     bias=-PIH)
                psl = rsf[:, p * AH:(p + 1) * AH]
                nc.vector.scalar_tensor_tensor(
                    out=psl, in0=cs[:], scalar=1.0, in1=psl,
                    op0=AL.subtract, op1=AL.mult)

            # ---- TensorE masked reduction; sym parked in SBUF (bf16)
            # group g = one a_hi; psum tile: rows 32*gp + (al*5+t5),
            # cols gf*48 + p, for g = ti*40 + gf*4 + gp
            sym = pool.tile([128, OUTW], bf16)
            rs1 = pool.tile([128, 260], f32)
            rs2 = pool.tile([128, 260], f32)
            GRP_T = 40
            for ti in range(NTILE):
                ngrp = GRP_T if ti < 25 else 24
                nfgf = 10 if ti < 25 else 6
                ncol = nfgf * 48
                wmask = wpool.tile([128, GRP_T * 32], bf16, tag="wmask")
                in0 = _mk_ap(zt[:], ti * GRP_T, [[1, ngrp], [0, 32]])
                t0 = _mk_ap(tcode[:], 0, [[0, ngrp], [1, 32]])
                nc.vector.tensor_tensor(
                    out=wmask[:, :ngrp * 32], in0=in0, in1=t0, op=AL.is_equal)
                stp = ppool.tile([128, 480], f32, tag="stp")
                for gi in range(ngrp):
                    g = ti * GRP_T + gi
                    gp, gf = gi % 4, gi // 4
                    rhs = _mk_ap(rsf[:], g, [[AH, P]])
                    nc.tensor.matmul(
                        out=stp[32 * gp:32 * gp + 32, gf * 48:(gf + 1) * 48],
                        lhsT=wmask[:, gi * 32:(gi + 1) * 32],
                        rhs=rhs, start=True, stop=True,
                        tile_position=(0, 32 * gp))
                # park to SBUF (ACT), stats rowsums (DVE)
                nc.scalar.activation(
                    out=sym[:, ti * 480:ti * 480 + ncol], in_=stp[:, :ncol],
                    func=AF.Copy)
                sqt = wpool.tile([128, 480], f32, tag="sqt")
                ssl = sym[:, ti * 480:ti * 480 + ncol]
                nc.vector.tensor_tensor(
                    out=sqt[:, :ncol], in0=ssl, in1=ssl, op=AL.mult)
                nc.vector.tensor_reduce(
                    out=rs1[:, ti * 10:ti * 10 + nfgf],
                    in_=_mk_ap(stp[:], 0, [[48, nfgf], [1, 48]]),
                    axis=mybir.AxisListType.X, op=AL.add)
                nc.vector.tensor_reduce(
                    out=rs2[:, ti * 10:ti * 10 + nfgf],
                    in_=_mk_ap(sqt[:], 0, [[48, nfgf], [1, 48]]),
                    axis=mybir.AxisListType.X, op=AL.add)

            # ---- per-atom (really per-n: al-parity folded by sel8) stats
            s1ps = spool.tile([128, 260], f32)
            s2ps = spool.tile([128, 260], f32)
            nc.tensor.matmul(out=s1ps[0:8, :], lhsT=sel8[:, 0:8], rhs=rs1[:],
                             start=True, stop=True)
            nc.tensor.matmul(out=s2ps[0:8, :], lhsT=sel8[:, 0:8], rhs=rs2[:],
                             start=True, stop=True)
            stt = pool.tile([128, 520], f32)
            nc.vector.tensor_copy(out=stt[0:8, 0:260], in_=s1ps[0:8, :])
            nc.vector.tensor_copy(out=stt[0:8, 260:520], in_=s2ps[0:8, :])
            nc.sync.dma_start(out=st_in[:], in_=stt[0:8, :])
            nc.gpsimd.collective_compute(
                "AllReduce", AL.add,
                ins=[st_in[:]], outs=[st_out[:]],
                replica_groups=[list(range(NC_CORES))])
            sall = pool.tile([128, 520], f32)
            nc.sync.dma_start(out=sall[0:8, :], in_=st_out[:])

            inv_n = 1.0 / (B * C_OUT)
            mean = pool.tile([128, 260], f32)
            nc.vector.tensor_scalar(out=mean[0:8, :], in0=sall[0:8, 0:260],
                                    scalar1=inv_n, scalar2=None, op0=AL.mult)
            vpe = pool.tile([128, 260], f32)
            nc.vector.tensor_scalar(out=vpe[0:8, :], in0=sall[0:8, 260:520],
                                    scalar1=inv_n, scalar2=None, op0=AL.mult)
            msq = wpool.tile([128, 260], f32, tag="msq")
            nc.vector.tensor_tensor(out=msq[0:8, :], in0=mean[0:8, :],
                                    in1=mean[0:8, :], op=AL.mult)
            nc.vector.tensor_tensor(out=vpe[0:8, :], in0=vpe[0:8, :],
                                    in1=msq[0:8, :], op=AL.subtract)
            nc.vector.tensor_scalar(out=vpe[0:8, :], in0=vpe[0:8, :],
                                    scalar1=float(4.0 * BN_EPS), scalar2=None,
                                    op0=AL.add)
            sdev = pool.tile([128, 260], f32)
            nc.scalar.activation(out=sdev[0:8, :], in_=vpe[0:8, :],
                                 func=AF.Sqrt)
            inv = pool.tile([128, 260], f32)
            nc.vector.reciprocal(out=inv[0:8, :], in_=sdev[0:8, :])
            nc.vector.tensor_scalar(out=inv[0:8, :], in0=inv[0:8, :],
                                    scalar1=-1.0, scalar2=None, op0=AL.mult)

            # broadcast stats to all 128 rows via selector matmul, then SBUF
            mbps = spool.tile([128, 260], f32)
            ibps = spool.tile([128, 260], f32)
            nc.tensor.matmul(out=mbps[:], lhsT=selbc[0:8, :], rhs=mean[0:8, :],
                             start=True, stop=True)
            nc.tensor.matmul(out=ibps[:], lhsT=selbc[0:8, :], rhs=inv[0:8, :],
                             start=True, stop=True)
            mbc = pool.tile([128, 260], f32)
            nc.vector.tensor_copy(out=mbc[:], in_=mbps[:])
            ibc = pool.tile([128, 260], f32)
            nc.vector.tensor_copy(out=ibc[:], in_=ibps[:])

            # ---- normalize in place (alternate DVE / GpSimd), one out DMA
            for ti in range(NTILE):
                nfgf = 10 if ti < 25 else 6
                ncol = nfgf * 48
                eng = nc.vector
                ssl = sym[:, ti * 480:ti * 480 + ncol]
                t1 = wpool.tile([128, 480], f32, tag="t1")
                eng.tensor_tensor(
                    out=t1[:, :ncol], in0=ssl,
                    in1=_mk_ap(mbc[:], ti * 10, [[1, nfgf], [0, 48]]),
                    op=AL.subtract)
                eng.tensor_tensor(
                    out=ssl, in0=t1[:, :ncol],
                    in1=_mk_ap(ibc[:], ti * 10, [[1, nfgf], [0, 48]]),
                    op=AL.mult)
            nc.sync.dma_start(out=out_ext[:], in_=sym[:])

    _fix_sync_waits(nc, spares, relay_sem)
    return nc


# ---------------------------------------------------------------- host driver
def _host_tables():
    from ml_dtypes import bfloat16
    tcode = np.full((128, 32), -1.0, np.float32)
    for al in range(4):
        for t5 in range(T):
            tcode[al * 32:(al + 1) * 32, al * 5 + t5] = float(ATOM_TYPES[t5])
    sel8 = np.zeros((128, 8), np.float32)
    selbc = np.zeros((8, 128), np.float32)
    for gp in range(4):
        for al in range(4):
            for t5 in range(T):
                row = 32 * gp + 5 * al + t5
                col = 4 * (al % 2) + gp
                sel8[row, col] = 1.0
                selbc[col, row] = 1.0
    # output unscramble: [128, OUTW] -> [4096, 240]
    a = np.arange(A)
    al = a // AH
    ah = a % AH
    ti = ah // 40
    rem = ah % 40
    gf = rem // 4
    gp = rem % 4
    c = np.arange(C_OUT)
    t5 = c // P
    p = c % P
    rows = (32 * gp[:, None] + 5 * al[:, None] + t5[None, :]).astype(np.int64)
    cols = ((ti * 480 + gf * 48)[:, None] + p[None, :]).astype(np.int64)
    return tcode.astype(bfloat16), sel8, selbc, rows, cols


_TBL = [None]


def kernel(X, rc, rs, re, Nbrs, Nbrs_Z):
    from ml_dtypes import bfloat16
    X = np.asarray(X, np.float32)
    rc = np.asarray(rc, np.float32).ravel()
    rs = np.asarray(rs, np.float32).ravel()
    re = np.asarray(re, np.float32).ravel()
    Nbrs = np.asarray(Nbrs, np.int32)
    Nbrs_Z = np.asarray(Nbrs_Z, np.int32)

    nc = build_nc(rc, rs, re)
    if _TBL[0] is None:
        _TBL[0] = _host_tables()
    tcode, sel8, selbc, orows, ocols = _TBL[0]

    # per-(a,m)-tile layouts: partition p = (a//1024)*32 + m, free = a % 1024
    in_maps = []
    for core in range(NC_CORES):
        bsl = slice(core * B_LOC, (core + 1) * B_LOC)
        Xc = X[bsl].reshape(A, 3)                       # a = b_loc*2048 + n
        Nb = Nbrs[bsl].reshape(A, M)
        Zb = Nbrs_Z[bsl].reshape(A, M)
        gidx = Nb + (np.arange(A)[:, None] // N) * N    # [A, M] global rows
        a_lo = np.arange(A) // AH
        a_hi = np.arange(A) % AH
        part = (a_lo[:, None] * 32 + np.arange(M)[None]).astype(np.int32)
        zt = np.zeros((128, AH), np.float32)
        zt[part.ravel(), np.repeat(a_hi, M)] = Zb.ravel().astype(np.float32)
        D = Xc[gidx] - Xc[:, None, :]                   # [A, M, 3]
        Rv = np.sqrt(np.sum(D * D, axis=2, dtype=np.float32))
        rr = np.zeros((128, AH), np.float32)
        rr[part.ravel(), np.repeat(a_hi, M)] = Rv.ravel()
        in_maps.append({
            "rr": rr, "zt": zt.astype(bfloat16), "tcode": tcode,
            "sel8": sel8, "selbc": selbc,
        })

    res = run_bass_kernel_spmd(nc, in_maps, core_ids=list(range(NC_CORES)),
                               trace=_TRACE[0])
    if _TRACE[0]:
        kernel.last_exec_ns = res.exec_time_ns
        kernel.last_profile = res

    out = np.zeros((B, N, C_OUT), np.float32)
    for core in range(NC_CORES):
        o = np.asarray(res.results[core]["out"]).astype(np.float32)
        out[core * B_LOC:(core + 1) * B_LOC] = o[orows, ocols].reshape(
            B_LOC, N, C_OUT)
    return out


# revision 10
# speedup vs baseline: 2.6793x; 2.6793x over previous
"""AtomicConvolution Trainium2 kernel (8 NeuronCores, data-parallel over B).

Pipeline per core (2 complexes, 4096 atoms, layout [par=(a_lo*32+m), free=a_hi]):
  host computes R (gather + norm) -> radial fn on device in two table-set
  batched ACT phases (Square+Exp phase, then Sin phase; half-angle form
  rsf = (sin(pi/2 - theta/2))^2 * 2*exp(..) avoids the slow 3-operand DVE
  op) writing contiguous-per-p rsf (bf16) -> per-atom-group masked type
  reduction on TensorE (block-diagonal 0/1 weights from is_equal) -> PSUM
  -> sym parked in SBUF (bf16, PSUM-native layout; host unscrambles) ->
  per-atom BN stats via selector matmuls + split AllReduce (first half
  overlaps the remaining tile loop) -> normalize in place -> one out DMA.
  Stored rsf = +2*f; BN epilogue uses eps*4 to compensate.
"""
import sys
import types
import numpy as np

ATOM_TYPES = (1, 6, 7, 8, 16)
BN_EPS = 1e-5
B, N, M, P = 16, 2048, 32, 48
T = len(ATOM_TYPES)
NC_CORES = 8
B_LOC = B // NC_CORES            # 2 complexes per core
A = B_LOC * N                    # 4096 atoms per core
AH = A // 4                      # 1024 free columns
C_OUT = P * T                    # 240 channels
NTILE = 26                       # ceil(1024 / 40) psum tiles
OUTW = NTILE * 480               # 12480 staged output columns
DVE_PREP = 32                    # how many p's compute the exp arg on DVE
HALF_T = 13                      # collective split point (tiles 0..12 | 13..25)

_TRACE = [False]

# ---------------------------------------------------------------- env patches
import concourse.bass as bass
import concourse.mybir as mybir
import concourse.tile as tile
import concourse.bass_utils as bu
from concourse.bass_utils import run_bass_kernel_spmd
from concourse.tile import TileContext, add_dep_helper


def _patch_tile_tail_drain():
    tile_mod = tile
    ScopedClock = None
    for _n in dir(tile_mod):
        if "ScopedClock" in _n:
            ScopedClock = getattr(tile_mod, _n)

    def _drain(self, tick_clock, wait_clock):
        nc = self.nc
        nops = [nc.sync.nop(nofuse=True) for _ in range(30)]
        drain_inst = nc.sync.drain()
        wait_clock.add_sem_waits(
            drain_inst.ins, ScopedClock({None: tick_clock.global_clock})
        )
        si = drain_inst.ins.sync_info
        if si is not None and si.on_wait and len(si.on_wait) > 1:
            waits = list(si.on_wait)
            si.on_wait = waits[:1]
            rest = waits[1:]
            assert len(rest) <= len(nops)
            for i, nop in enumerate(nops):
                chunk = rest[i:i + 1]
                if not chunk:
                    break
                nsi = nop.ins.sync_info
                if nsi is None:
                    nop.ins.sync_info = mybir.SyncInfo(on_wait=chunk, on_update=[])
                else:
                    nsi.on_wait = chunk
        nc.all_engine_barrier()
        popped = nc._tile_sem_poison_stack.pop()
        assert popped is self._sem_poison
        nc.clear_and_free_semaphores(list(self.sems.allocated().values()))
        nc.all_engine_barrier()

    TileContext._drain_and_barrier = _drain


WAIT_CAP = 1


def _make_spare_nops(nc, counts):
    # SP-engine carrier nops: the only engine whose sequencer NoOp reliably
    # encodes with sem waits in this walrus build.
    return {"carriers": [nc.sync.nop(nofuse=True) for _ in range(4000)]}


def _fix_sync_waits(nc, spares, relay):
    clr = nc.sync.sem_clear(relay)
    relay_count = [0]
    carriers = spares["carriers"]
    spare_names = {c.ins.name for c in carriers}
    # move the freshly-appended clear to the very beginning of the first block
    fn0 = nc.m.functions[0]
    for bb in fn0.blocks:
        if clr.ins in bb.instructions:
            bb.instructions.remove(clr.ins)
    fn0.blocks[0].instructions.insert(0, clr.ins)
    for fn in nc.m.functions:
        for bb in fn.blocks:
            bb.instructions[:] = [
                i for i in bb.instructions if i.name not in spare_names
            ]
    for fn in nc.m.functions:
        for bb in fn.blocks:
            new = []
            for inst in bb.instructions:
                si = inst.sync_info
                waits = list(si.on_wait) if si is not None and si.on_wait else []
                if len(waits) > WAIT_CAP:
                    for w in waits:
                        assert carriers, "out of relay carriers"
                        car = carriers.pop()
                        car.then_inc(relay, 1)
                        car.ins.sync_info.on_wait = [w]
                        relay_count[0] += 1
                        new.append(car.ins)
                    si.on_wait = [mybir.SyncWait(
                        sync_type="semaphore", id=relay.num,
                        ant_name=relay.name, wait_mode="sem-ge-imm",
                        wait_value=relay_count[0], wait_reg=None)]
                new.append(inst)
            bb.instructions[:] = new


def _patch_walrus_dyndma(size=16384):
    if getattr(bu.run_command, "_walrus_patched", False):
        return
    _orig = bu.run_command

    def run2(cmd, cwd=None, **kw):
        try:
            if cmd and "walrus_driver" in str(cmd[0]) and any(
                "codegen" in str(c) for c in cmd
            ):
                cmd = list(cmd) + [
                    f"--dynamic-dma-scratch-size-per-partition={size}"
                ]
        except Exception:
            pass
        return _orig(cmd, cwd=cwd, **kw)

    run2._walrus_patched = True
    bu.run_command = run2


def _install_ntff_hook():
    if "antenv.axon_hooks" in sys.modules:
        return
    try:
        from trn_agent_boot.trn_boot import _ntff_profile_via_ctypes
        hook = _ntff_profile_via_ctypes("/opt/axon/libaxon_pjrt.so")
    except Exception:
        hook = None
    m = types.ModuleType("antenv.axon_hooks")
    m._hook = hook
    m.get_axon_ntff_profile_hook = lambda: m._hook
    m.set_axon_ntff_profile_hook = lambda h: setattr(m, "_hook", h)
    sys.modules["antenv.axon_hooks"] = m
    try:
        import antenv
        antenv.axon_hooks = m
    except Exception:
        pass


_patch_tile_tail_drain()
_patch_walrus_dyndma()
_install_ntff_hook()

DT = mybir.dt


def _mk_ap(base_ap, off_elems, free_dims):
    return bass.AP(base_ap.tensor, base_ap.offset + off_elems,
                   [base_ap.ap[0]] + free_dims)


# ---------------------------------------------------------------- bass build
def build_nc(rcv, rsv, rev):
    nc = bass.Bass(dynamic_dma_scratch_size=8192)
    f32, bf16 = DT.float32, DT.bfloat16

    PIH = float(np.pi / 2.0)
    AL = mybir.AluOpType
    AF = mybir.ActivationFunctionType

    rr_ext = nc.declare_dram_parameter("rr", [128, AH], f32, isOutput=False)
    zt_ext = nc.declare_dram_parameter("zt", [128, AH], bf16, isOutput=False)
    tc_ext = nc.declare_dram_parameter("tcode", [128, 32], bf16, isOutput=False)
    s8_ext = nc.declare_dram_parameter("sel8", [128, 8], f32, isOutput=False)
    sb_ext = nc.declare_dram_parameter("selbc", [8, 128], f32, isOutput=False)
    # bias table: col p -> -rs_p (Square bias), col 48+p -> exp-path bias,
    # col 96 -> +pi/2 (Sin), col 97 -> ln2
    bi_ext = nc.declare_dram_parameter("biases", [128, 98], f32, isOutput=False)
    out_ext = nc.declare_dram_parameter("out", [128, OUTW], bf16, isOutput=True)

    st_in = [nc.dram_tensor(f"st_in{h}", [8, 260], f32) for h in range(2)]
    st_out = [nc.dram_tensor(f"st_out{h}", [8, 260], f32, addr_space="Shared")
              for h in range(2)]

    relay_sem = nc.semaphore("wait_relay").__enter__()
    with TileContext(nc) as tc:
        spares = _make_spare_nops(nc, {})
        with tc.tile_pool(name="main", bufs=1) as pool, \
             tc.tile_pool(name="work", bufs=2) as wpool, \
             tc.tile_pool(name="psum", bufs=2, space="PSUM") as ppool, \
             tc.tile_pool(name="pstat", bufs=1, space="PSUM") as spool:

            # ---- loads
            zt = pool.tile([128, AH], bf16)
            nc.sync.dma_start(out=zt[:], in_=zt_ext[:])
            tcode = pool.tile([128, 32], bf16)
            nc.sync.dma_start(out=tcode[:], in_=tc_ext[:])
            sel8 = pool.tile([128, 8], f32)
            nc.sync.dma_start(out=sel8[:], in_=s8_ext[:])
            selbc = pool.tile([128, 128], f32)
            nc.sync.dma_start(out=selbc[0:8, :], in_=sb_ext[:])
            bia = pool.tile([128, 98], f32)
            nc.sync.dma_start(out=bia[:], in_=bi_ext[:])
            rr = pool.tile([128, AH], f32)
            nc.sync.dma_start(out=rr[:], in_=rr_ext[:])
            r2 = pool.tile([128, AH], f32)
            nc.vector.tensor_tensor(out=r2[:], in0=rr[:], in1=rr[:],
                                    op=AL.mult)

            # rsf layout: contiguous per p -> col = p*AH + a_hi
            rsf = pool.tile([128, P * AH], bf16)

            # ---- phase A: kk'_p = 2*exp(-re*(R-rs)^2), bf16.
            # DVE_PREP of the p's build the exp argument on DVE (2 fast ops),
            # the rest use ACT Square (same table set as Exp -> no reload).
            last_exp = None
            for p in range(P):
                re_p, rs_p = float(rev[p]), float(rsv[p])
                if p < DVE_PREP:
                    t1 = wpool.tile([128, AH], f32, tag="t1")
                    nc.vector.tensor_scalar(
                        out=t1[:], in0=rr[:], scalar1=-2.0 * rs_p,
                        scalar2=None, op0=AL.mult)
                    u = wpool.tile([128, AH], f32, tag="u")
                    nc.vector.tensor_tensor(out=u[:], in0=t1[:], in1=r2[:],
                                            op=AL.add)
                    # exp(-re*u + (ln2 - re*rs^2)) = 2*exp(-re*(R-rs)^2)
                    ei = nc.scalar.activation(
                        out=rsf[:, p * AH:(p + 1) * AH], in_=u[:],
                        func=AF.Exp, scale=-re_p,
                        bias=bia[:, 48 + p:49 + p])
                else:
                    u = wpool.tile([128, AH], f32, tag="u")
                    nc.scalar.activation(out=u[:], in_=rr[:], func=AF.Square,
                                         bias=bia[:, p:p + 1])
                    ei = nc.scalar.activation(
                        out=rsf[:, p * AH:(p + 1) * AH], in_=u[:],
                        func=AF.Exp, scale=-re_p,
                        bias=bia[:, 97:98])
                last_exp = ei

            # ---- phase B: s = sin(pi/2 - pi*min(R,rc)/(2rc)) (>=0, LUT-safe)
            # rsf *= s*s  ->  rsf = 2*f_p.  Sins forced after all Exps so the
            # ACT table set switches exactly once.
            for p in range(P):
                rc_p = float(rcv[p])
                rt = wpool.tile([128, AH], f32, tag="rt")
                nc.vector.tensor_scalar(
                    out=rt[:], in0=rr[:], scalar1=rc_p,
                    scalar2=float(np.pi / (2.0 * rc_p)),
                    op0=AL.min, op1=AL.mult)
                cs = wpool.tile([128, AH], bf16, tag="cs")
                si = nc.scalar.activation(out=cs[:], in_=rt[:], func=AF.Sin,
                                          scale=-1.0, bias=bia[:, 96:97])
                add_dep_helper(si.ins, last_exp.ins,
                               reason="keep Sin phase after Exp phase")
                s2t = wpool.tile([128, AH], bf16, tag="s2t")
                nc.vector.tensor_tensor(out=s2t[:], in0=cs[:], in1=cs[:],
                                        op=AL.mult)
                psl = rsf[:, p * AH:(p + 1) * AH]
                nc.vector.tensor_tensor(out=psl, in0=s2t[:], in1=psl,
                                        op=AL.mult)

            # ---- TensorE masked reduction; sym parked in SBUF (bf16)
            # group g = one a_hi; psum tile: rows 32*gp + (al*5+t5),
            # cols gf*48 + p, for g = ti*40 + gf*4 + gp
            sym = pool.tile([128, OUTW], bf16)
            rs1 = pool.tile([128, 260], f32)
            rs2 = pool.tile([128, 260], f32)
            mbc = pool.tile([128, 260], f32)
            ibc = pool.tile([128, 260], f32)
            sall = [None, None]
            GRP_T = 40

            def stats_half(h):
                # selector matmuls -> [8, 130] per-n stats -> AllReduce
                c0, c1 = (0, 130) if h == 0 else (130, 260)
                s1ps = spool.tile([128, 130], f32, tag="s1ps")
                s2ps = spool.tile([128, 130], f32, tag="s2ps")
                nc.tensor.matmul(out=s1ps[0:8, :], lhsT=sel8[:, 0:8],
                                 rhs=rs1[:, c0:c1], start=True, stop=True)
                nc.tensor.matmul(out=s2ps[0:8, :], lhsT=sel8[:, 0:8],
                                 rhs=rs2[:, c0:c1], start=True, stop=True)
                stt = wpool.tile([128, 260], f32, tag="stt")
                nc.vector.tensor_copy(out=stt[0:8, 0:130], in_=s1ps[0:8, :])
                nc.vector.tensor_copy(out=stt[0:8, 130:260], in_=s2ps[0:8, :])
                nc.sync.dma_start(out=st_in[h][:], in_=stt[0:8, :])
                nc.gpsimd.collective_compute(
                    "AllReduce", AL.add,
                    ins=[st_in[h][:]], outs=[st_out[h][:]],
                    replica_groups=[list(range(NC_CORES))])
                sa = pool.tile([128, 260], f32)
                nc.sync.dma_start(out=sa[0:8, :], in_=st_out[h][:])
                sall[h] = sa

            def epilogue_half(h):
                # mean/inv on [8,130], broadcast to 128 rows via matmul
                c0, c1 = (0, 130) if h == 0 else (130, 260)
                sa = sall[h]
                inv_n = 1.0 / (B * C_OUT)
                mean = wpool.tile([128, 130], f32, tag="mean")
                nc.vector.tensor_scalar(out=mean[0:8, :], in0=sa[0:8, 0:130],
                                        scalar1=inv_n, scalar2=None,
                                        op0=AL.mult)
                vpe = wpool.tile([128, 130], f32, tag="vpe")
                nc.vector.tensor_scalar(out=vpe[0:8, :], in0=sa[0:8, 130:260],
                                        scalar1=inv_n, scalar2=None,
                                        op0=AL.mult)
                msq = wpool.tile([128, 130], f32, tag="msq")
                nc.vector.tensor_tensor(out=msq[0:8, :], in0=mean[0:8, :],
                                        in1=mean[0:8, :], op=AL.mult)
                nc.vector.tensor_tensor(out=vpe[0:8, :], in0=vpe[0:8, :],
                                        in1=msq[0:8, :], op=AL.subtract)
                nc.vector.tensor_scalar(out=vpe[0:8, :], in0=vpe[0:8, :],
                                        scalar1=float(4.0 * BN_EPS),
                                        scalar2=None, op0=AL.add)
                sdev = wpool.tile([128, 130], f32, tag="sdev")
                nc.scalar.activation(out=sdev[0:8, :], in_=vpe[0:8, :],
                                     func=AF.Sqrt)
                inv = wpool.tile([128, 130], f32, tag="inv")
                nc.vector.reciprocal(out=inv[0:8, :], in_=sdev[0:8, :])
                mbp = spool.tile([128, 130], f32, tag="mbp")
                ibp = spool.tile([128, 130], f32, tag="ibp")
                nc.tensor.matmul(out=mbp[:], lhsT=selbc[0:8, :],
                                 rhs=mean[0:8, :], start=True, stop=True)
                nc.tensor.matmul(out=ibp[:], lhsT=selbc[0:8, :],
                                 rhs=inv[0:8, :], start=True, stop=True)
                nc.vector.tensor_copy(out=mbc[:, c0:c1], in_=mbp[:])
                nc.vector.tensor_copy(out=ibc[:, c0:c1], in_=ibp[:])

            def normalize_tile(ti):
                nfgf = 10 if ti < 25 else 6
                ncol = nfgf * 48
                ssl = sym[:, ti * 480:ti * 480 + ncol]
                t1 = wpool.tile([128, 480], f32, tag="nt")
                nc.vector.tensor_tensor(
                    out=t1[:, :ncol], in0=ssl,
                    in1=_mk_ap(mbc[:], ti * 10, [[1, nfgf], [0, 48]]),
                    op=AL.subtract)
                nc.vector.tensor_tensor(
                    out=ssl, in0=t1[:, :ncol],
                    in1=_mk_ap(ibc[:], ti * 10, [[1, nfgf], [0, 48]]),
                    op=AL.mult)

            for ti in range(NTILE):
                ngrp = GRP_T if ti < 25 else 24
                nfgf = 10 if ti < 25 else 6
                ncol = nfgf * 48
                wmask = wpool.tile([128, GRP_T * 32], bf16, tag="wmask")
                in0 = _mk_ap(zt[:], ti * GRP_T, [[1, ngrp], [0, 32]])
                t0 = _mk_ap(tcode[:], 0, [[0, ngrp], [1, 32]])
                nc.vector.tensor_tensor(
                    out=wmask[:, :ngrp * 32], in0=in0, in1=t0, op=AL.is_equal)
                stp = ppool.tile([128, 480], f32, tag="stp")
                for gi in range(ngrp):
                    g = ti * GRP_T + gi
                    gp, gf = gi % 4, gi // 4
                    rhs = _mk_ap(rsf[:], g, [[AH, P]])
                    nc.tensor.matmul(
                        out=stp[32 * gp:32 * gp + 32, gf * 48:(gf + 1) * 48],
                        lhsT=wmask[:, gi * 32:(gi + 1) * 32],
                        rhs=rhs, start=True, stop=True,
                        tile_position=(0, 32 * gp))
                # park to SBUF, stats rowsums (all DVE)
                ssl = sym[:, ti * 480:ti * 480 + ncol]
                nc.vector.tensor_copy(out=ssl, in_=stp[:, :ncol])
                sqt = wpool.tile([128, 480], f32, tag="sqt")
                nc.vector.tensor_tensor(
                    out=sqt[:, :ncol], in0=ssl, in1=ssl, op=AL.mult)
                nc.vector.tensor_reduce(
                    out=rs1[:, ti * 10:ti * 10 + nfgf],
                    in_=_mk_ap(stp[:], 0, [[48, nfgf], [1, 48]]),
                    axis=mybir.AxisListType.X, op=AL.add)
                nc.vector.tensor_reduce(
                    out=rs2[:, ti * 10:ti * 10 + nfgf],
                    in_=_mk_ap(sqt[:], 0, [[48, nfgf], [1, 48]]),
                    axis=mybir.AxisListType.X, op=AL.add)
                if ti == HALF_T - 1:
                    stats_half(0)
                    epilogue_half(0)
            stats_half(1)
            for ti in range(HALF_T):
                normalize_tile(ti)
            epilogue_half(1)
            for ti in range(HALF_T, NTILE):
                normalize_tile(ti)
            nc.sync.dma_start(out=out_ext[:], in_=sym[:])

    _fix_sync_waits(nc, spares, relay_sem)
    return nc


# ---------------------------------------------------------------- host driver
def _host_tables(rsv, rev):
    from ml_dtypes import bfloat16
    LN2 = float(np.log(2.0))
    tcode = np.full((128, 32), -1.0, np.float32)
    for al in range(4):
        for t5 in range(T):
            tcode[al * 32:(al + 1) * 32, al * 5 + t5] = float(ATOM_TYPES[t5])
    sel8 = np.zeros((128, 8), np.float32)
    selbc = np.zeros((8, 128), np.float32)
    for gp in range(4):
        for al in range(4):
            for t5 in range(T):
                row = 32 * gp + 5 * al + t5
                col = 4 * (al % 2) + gp
                sel8[row, col] = 1.0
                selbc[col, row] = 1.0
    biases = np.zeros((128, 98), np.float32)
    for p in range(P):
        biases[:, p] = -float(rsv[p])
        biases[:, 48 + p] = LN2 - float(rev[p]) * float(rsv[p]) ** 2
    biases[:, 96] = float(np.pi / 2.0)
    biases[:, 97] = LN2
    # output unscramble: [128, OUTW] -> [4096, 240]
    a = np.arange(A)
    al = a // AH
    ah = a % AH
    ti = ah // 40
    rem = ah % 40
    gf = rem // 4
    gp = rem % 4
    c = np.arange(C_OUT)
    t5 = c // P
    p = c % P
    rows = (32 * gp[:, None] + 5 * al[:, None] + t5[None, :]).astype(np.int64)
    cols = ((ti * 480 + gf * 48)[:, None] + p[None, :]).astype(np.int64)
    return tcode.astype(bfloat16), sel8, selbc, biases, rows, cols


def kernel(X, rc, rs, re, Nbrs, Nbrs_Z):
    from ml_dtypes import bfloat16
    X = np.asarray(X, np.float32)
    rc = np.asarray(rc, np.float32).ravel()
    rs = np.asarray(rs, np.float32).ravel()
    re = np.asarray(re, np.float32).ravel()
    Nbrs = np.asarray(Nbrs, np.int32)
    Nbrs_Z = np.asarray(Nbrs_Z, np.int32)

    nc = build_nc(rc, rs, re)
    tcode, sel8, selbc, biases, orows, ocols = _host_tables(rs, re)

    # per-(a,m)-tile layouts: partition p = (a//1024)*32 + m, free = a % 1024
    in_maps = []
    for core in range(NC_CORES):
        bsl = slice(core * B_LOC, (core + 1) * B_LOC)
        Xc = X[bsl].reshape(A, 3)                       # a = b_loc*2048 + n
        Nb = Nbrs[bsl].reshape(A, M)
        Zb = Nbrs_Z[bsl].reshape(A, M)
        gidx = Nb + (np.arange(A)[:, None] // N) * N    # [A, M] global rows
        a_hi = np.arange(A) % AH
        part = ((np.arange(A) // AH)[:, None] * 32
                + np.arange(M)[None]).astype(np.int32)
        zt = np.zeros((128, AH), np.float32)
        zt[part.ravel(), np.repeat(a_hi, M)] = Zb.ravel().astype(np.float32)
        D = Xc[gidx] - Xc[:, None, :]                   # [A, M, 3]
        Rv = np.sqrt(np.sum(D * D, axis=2, dtype=np.float32))
        rr = np.zeros((128, AH), np.float32)
        rr[part.ravel(), np.repeat(a_hi, M)] = Rv.ravel()
        in_maps.append({
            "rr": rr, "zt": zt.astype(bfloat16), "tcode": tcode,
            "sel8": sel8, "selbc": selbc, "biases": biases,
        })

    res = run_bass_kernel_spmd(nc, in_maps, core_ids=list(range(NC_CORES)),
                               trace=_TRACE[0])
    if _TRACE[0]:
        kernel.last_exec_ns = res.exec_time_ns
        kernel.last_profile = res

    out = np.zeros((B, N, C_OUT), np.float32)
    for core in range(NC_CORES):
        o = np.asarray(res.results[core]["out"]).astype(np.float32)
        out[core * B_LOC:(core + 1) * B_LOC] = o[orows, ocols].reshape(
            B_LOC, N, C_OUT)
    return out


# revision 16
# speedup vs baseline: 2.8329x; 1.0573x over previous
"""AtomicConvolution Trainium2 kernel (8 NeuronCores, data-parallel over B).

Pipeline per core (2 complexes, 4096 atoms, layout [par=(a_lo*32+m), free=a_hi]):
  host computes R (gather + norm) -> radial fn on device in two table-set
  batched ACT phases (Square+Exp phase, then Sin phase; half-angle form
  rsf = (sin(pi/2 - theta/2))^2 * 2*exp(..) avoids the slow 3-operand DVE
  op) writing contiguous-per-p rsf (bf16) -> per-atom-group masked type
  reduction on TensorE (block-diagonal 0/1 weights from is_equal) -> PSUM
  -> sym parked in SBUF (bf16, PSUM-native layout; host unscrambles) ->
  per-atom BN stats via selector matmuls + split AllReduce (first half
  overlaps the remaining tile loop) -> normalize in place -> one out DMA.
  Stored rsf = +2*f; BN epilogue uses eps*4 to compensate.
"""
import sys
import types
import numpy as np

ATOM_TYPES = (1, 6, 7, 8, 16)
BN_EPS = 1e-5
B, N, M, P = 16, 2048, 32, 48
T = len(ATOM_TYPES)
NC_CORES = 8
B_LOC = B // NC_CORES            # 2 complexes per core
A = B_LOC * N                    # 4096 atoms per core
AH = A // 4                      # 1024 free columns
C_OUT = P * T                    # 240 channels
NTILE = 26                       # ceil(1024 / 40) psum tiles
OUTW = NTILE * 480               # 12480 staged output columns
DVE_PREP = 32                    # how many p's compute the exp arg on DVE
HALF_T = 13                      # collective split point (tiles 0..12 | 13..25)

_TRACE = [False]

# ---------------------------------------------------------------- env patches
import concourse.bass as bass
import concourse.mybir as mybir
import concourse.tile as tile
import concourse.bass_utils as bu
from concourse.bass_utils import run_bass_kernel_spmd
from concourse.tile import TileContext, add_dep_helper


def _patch_tile_tail_drain():
    tile_mod = tile
    ScopedClock = None
    for _n in dir(tile_mod):
        if "ScopedClock" in _n:
            ScopedClock = getattr(tile_mod, _n)

    def _drain(self, tick_clock, wait_clock):
        nc = self.nc
        nops = [nc.sync.nop(nofuse=True) for _ in range(30)]
        drain_inst = nc.sync.drain()
        wait_clock.add_sem_waits(
            drain_inst.ins, ScopedClock({None: tick_clock.global_clock})
        )
        si = drain_inst.ins.sync_info
        if si is not None and si.on_wait and len(si.on_wait) > 1:
            waits = list(si.on_wait)
            si.on_wait = waits[:1]
            rest = waits[1:]
            assert len(rest) <= len(nops)
            for i, nop in enumerate(nops):
                chunk = rest[i:i + 1]
                if not chunk:
                    break
                nsi = nop.ins.sync_info
                if nsi is None:
                    nop.ins.sync_info = mybir.SyncInfo(on_wait=chunk, on_update=[])
                else:
                    nsi.on_wait = chunk
        nc.all_engine_barrier()
        popped = nc._tile_sem_poison_stack.pop()
        assert popped is self._sem_poison
        nc.clear_and_free_semaphores(list(self.sems.allocated().values()))
        nc.all_engine_barrier()

    TileContext._drain_and_barrier = _drain


WAIT_CAP = 1


def _make_spare_nops(nc, counts):
    # SP-engine carrier nops: the only engine whose sequencer NoOp reliably
    # encodes with sem waits in this walrus build.
    return {"carriers": [nc.sync.nop(nofuse=True) for _ in range(4000)]}


def _fix_sync_waits(nc, spares, relay):
    clr = nc.sync.sem_clear(relay)
    relay_count = [0]
    carriers = spares["carriers"]
    spare_names = {c.ins.name for c in carriers}
    # move the freshly-appended clear to the very beginning of the first block
    fn0 = nc.m.functions[0]
    for bb in fn0.blocks:
        if clr.ins in bb.instructions:
            bb.instructions.remove(clr.ins)
    fn0.blocks[0].instructions.insert(0, clr.ins)
    for fn in nc.m.functions:
        for bb in fn.blocks:
            bb.instructions[:] = [
                i for i in bb.instructions if i.name not in spare_names
            ]
    for fn in nc.m.functions:
        for bb in fn.blocks:
            new = []
            for inst in bb.instructions:
                si = inst.sync_info
                waits = list(si.on_wait) if si is not None and si.on_wait else []
                if len(waits) > WAIT_CAP:
                    for w in waits:
                        assert carriers, "out of relay carriers"
                        car = carriers.pop()
                        car.then_inc(relay, 1)
                        car.ins.sync_info.on_wait = [w]
                        relay_count[0] += 1
                        new.append(car.ins)
                    si.on_wait = [mybir.SyncWait(
                        sync_type="semaphore", id=relay.num,
                        ant_name=relay.name, wait_mode="sem-ge-imm",
                        wait_value=relay_count[0], wait_reg=None)]
                new.append(inst)
            bb.instructions[:] = new


def _patch_walrus_dyndma(size=16384):
    if getattr(bu.run_command, "_walrus_patched", False):
        return
    _orig = bu.run_command

    def run2(cmd, cwd=None, **kw):
        try:
            if cmd and "walrus_driver" in str(cmd[0]) and any(
                "codegen" in str(c) for c in cmd
            ):
                cmd = list(cmd) + [
                    f"--dynamic-dma-scratch-size-per-partition={size}"
                ]
        except Exception:
            pass
        return _orig(cmd, cwd=cwd, **kw)

    run2._walrus_patched = True
    bu.run_command = run2


def _install_ntff_hook():
    if "antenv.axon_hooks" in sys.modules:
        return
    try:
        from trn_agent_boot.trn_boot import _ntff_profile_via_ctypes
        hook = _ntff_profile_via_ctypes("/opt/axon/libaxon_pjrt.so")
    except Exception:
        hook = None
    m = types.ModuleType("antenv.axon_hooks")
    m._hook = hook
    m.get_axon_ntff_profile_hook = lambda: m._hook
    m.set_axon_ntff_profile_hook = lambda h: setattr(m, "_hook", h)
    sys.modules["antenv.axon_hooks"] = m
    try:
        import antenv
        antenv.axon_hooks = m
    except Exception:
        pass


_patch_tile_tail_drain()
_patch_walrus_dyndma()
_install_ntff_hook()

DT = mybir.dt


def _mk_ap(base_ap, off_elems, free_dims):
    return bass.AP(base_ap.tensor, base_ap.offset + off_elems,
                   [base_ap.ap[0]] + free_dims)


# ---------------------------------------------------------------- bass build
def build_nc(rcv, rsv, rev):
    nc = bass.Bass(dynamic_dma_scratch_size=8192)
    f32, bf16 = DT.float32, DT.bfloat16

    PIH = float(np.pi / 2.0)
    AL = mybir.AluOpType
    AF = mybir.ActivationFunctionType

    rr_ext = nc.declare_dram_parameter("rr", [128, AH], f32, isOutput=False)
    zt_ext = nc.declare_dram_parameter("zt", [128, AH], bf16, isOutput=False)
    tc_ext = nc.declare_dram_parameter("tcode", [128, 32], bf16, isOutput=False)
    s8_ext = nc.declare_dram_parameter("sel8", [128, 8], bf16, isOutput=False)
    sb_ext = nc.declare_dram_parameter("selbc", [8, 128], f32, isOutput=False)
    # bias table: col p -> -rs_p (Square bias), col 48+p -> exp-path bias,
    # col 96 -> +pi/2 (Sin), col 97 -> ln2
    bi_ext = nc.declare_dram_parameter("biases", [128, 98], f32, isOutput=False)
    out_ext = nc.declare_dram_parameter("out", [128, OUTW], bf16, isOutput=True)

    st_in = [nc.dram_tensor(f"st_in{h}", [8, 260], f32) for h in range(2)]
    st_out = [nc.dram_tensor(f"st_out{h}", [8, 260], f32, addr_space="Shared")
              for h in range(2)]

    relay_sem = nc.semaphore("wait_relay").__enter__()
    with TileContext(nc) as tc:
        spares = _make_spare_nops(nc, {})
        with tc.tile_pool(name="main", bufs=1) as pool, \
             tc.tile_pool(name="work", bufs=2) as wpool, \
             tc.tile_pool(name="uprep", bufs=4) as upool, \
             tc.tile_pool(name="rcap", bufs=6) as rpool, \
             tc.tile_pool(name="psum", bufs=2, space="PSUM") as ppool, \
             tc.tile_pool(name="py", bufs=2, space="PSUM") as ypool, \
             tc.tile_pool(name="pstat", bufs=1, space="PSUM") as spool:

            # ---- loads
            zt = pool.tile([128, AH], bf16)
            nc.sync.dma_start(out=zt[:], in_=zt_ext[:])
            tcode = pool.tile([128, 32], bf16)
            nc.sync.dma_start(out=tcode[:], in_=tc_ext[:])
            sel8 = pool.tile([128, 8], bf16)
            nc.sync.dma_start(out=sel8[:], in_=s8_ext[:])
            selbc = pool.tile([128, 128], f32)
            nc.sync.dma_start(out=selbc[0:8, :], in_=sb_ext[:])
            bia = pool.tile([128, 98], f32)
            nc.sync.dma_start(out=bia[:], in_=bi_ext[:])
            rr = pool.tile([128, AH], f32)
            nc.sync.dma_start(out=rr[:], in_=rr_ext[:])
            r2 = pool.tile([128, AH], f32)
            nc.vector.tensor_tensor(out=r2[:], in0=rr[:], in1=rr[:],
                                    op=AL.mult)

            # rsf layout: contiguous per p -> col = p*AH + a_hi
            rsf = pool.tile([128, P * AH], bf16)

            # ---- phase A: kk'_p = 2*exp(-re*(R-rs)^2), bf16.
            # ACT-path p's (Square+Exp, one table set) run first so ACT
            # starts immediately; DVE_PREP p's build the exp arg on DVE
            # meanwhile (deeper uprep pool so DVE runs ahead).
            last_exp = None
            p_order = list(range(DVE_PREP, P)) + list(range(DVE_PREP))
            for p in p_order:
                re_p, rs_p = float(rev[p]), float(rsv[p])
                if p < DVE_PREP:
                    t1 = wpool.tile([128, AH], f32, tag="t1")
                    nc.vector.tensor_scalar(
                        out=t1[:], in0=rr[:], scalar1=-2.0 * rs_p,
                        scalar2=None, op0=AL.mult)
                    u = upool.tile([128, AH], f32, tag="u")
                    nc.vector.tensor_tensor(out=u[:], in0=t1[:], in1=r2[:],
                                            op=AL.add)
                    # exp(-re*u + (ln2 - re*rs^2)) = 2*exp(-re*(R-rs)^2)
                    ei = nc.scalar.activation(
                        out=rsf[:, p * AH:(p + 1) * AH], in_=u[:],
                        func=AF.Exp, scale=-re_p,
                        bias=bia[:, 48 + p:49 + p])
                else:
                    ua = wpool.tile([128, AH], f32, tag="ua")
                    nc.scalar.activation(out=ua[:], in_=rr[:], func=AF.Square,
                                         bias=bia[:, p:p + 1])
                    ei = nc.scalar.activation(
                        out=rsf[:, p * AH:(p + 1) * AH], in_=ua[:],
                        func=AF.Exp, scale=-re_p,
                        bias=bia[:, 97:98])
                last_exp = ei

            # ---- phase B: s = sin(pi/2 - pi*min(R,rc)/(2rc)) (>=0, LUT-safe)
            # rsf *= s*s  ->  rsf = 2*f_p.  Sins forced after all Exps so the
            # ACT table set switches exactly once; rcap pool is deep so DVE
            # computes sin args well ahead.
            for p in range(P):
                rc_p = float(rcv[p])
                rt = rpool.tile([128, AH], bf16, tag="rt")
                nc.vector.tensor_scalar(
                    out=rt[:], in0=rr[:], scalar1=rc_p,
                    scalar2=float(np.pi / (2.0 * rc_p)),
                    op0=AL.min, op1=AL.mult)
                cs = wpool.tile([128, AH], bf16, tag="cs")
                si = nc.scalar.activation(out=cs[:], in_=rt[:], func=AF.Sin,
                                          scale=-1.0, bias=bia[:, 96:97])
                add_dep_helper(si.ins, last_exp.ins,
                               reason="keep Sin phase after Exp phase")
                s2t = wpool.tile([128, AH], bf16, tag="s2t")
                nc.vector.tensor_tensor(out=s2t[:], in0=cs[:], in1=cs[:],
                                        op=AL.mult)
                psl = rsf[:, p * AH:(p + 1) * AH]
                nc.vector.tensor_tensor(out=psl, in0=s2t[:], in1=psl,
                                        op=AL.mult)

            # ---- TensorE masked reduction; sym parked in SBUF (bf16)
            # group g = one a_hi; psum tile: rows 32*gp + (al*5+t5),
            # cols gf*48 + p, for g = ti*40 + gf*4 + gp
            sym = pool.tile([128, OUTW], bf16)
            rs1 = pool.tile([128, 260], f32)
            rs2 = pool.tile([128, 260], f32)
            mbc = pool.tile([128, 260], f32)
            ibc = pool.tile([128, 260], f32)
            sall = [None, None]
            GRP_T = 40

            def stats_half(h):
                # per-n stats already folded per tile; just ship + AllReduce
                c0, c1 = (0, 130) if h == 0 else (130, 260)
                nc.sync.dma_start(out=st_in[h][:, 0:130], in_=rs1[0:8, c0:c1])
                nc.sync.dma_start(out=st_in[h][:, 130:260],
                                  in_=rs2[0:8, c0:c1])
                nc.gpsimd.collective_compute(
                    "AllReduce", AL.add,
                    ins=[st_in[h][:]], outs=[st_out[h][:]],
                    replica_groups=[list(range(NC_CORES))])
                sa = pool.tile([128, 260], f32)
                nc.sync.dma_start(out=sa[0:8, :], in_=st_out[h][:])
                sall[h] = sa

            def epilogue_half(h):
                # mean/inv on [8,130], broadcast to 128 rows via matmul
                c0, c1 = (0, 130) if h == 0 else (130, 260)
                sa = sall[h]
                inv_n = 1.0 / (B * C_OUT)
                mean = wpool.tile([128, 130], f32, tag="mean")
                nc.vector.tensor_scalar(out=mean[0:8, :], in0=sa[0:8, 0:130],
                                        scalar1=inv_n, scalar2=None,
                                        op0=AL.mult)
                vpe = wpool.tile([128, 130], f32, tag="vpe")
                nc.vector.tensor_scalar(out=vpe[0:8, :], in0=sa[0:8, 130:260],
                                        scalar1=inv_n, scalar2=None,
                                        op0=AL.mult)
                msq = wpool.tile([128, 130], f32, tag="msq")
                nc.vector.tensor_tensor(out=msq[0:8, :], in0=mean[0:8, :],
                                        in1=mean[0:8, :], op=AL.mult)
                nc.vector.tensor_tensor(out=vpe[0:8, :], in0=vpe[0:8, :],
                                        in1=msq[0:8, :], op=AL.subtract)
                nc.vector.tensor_scalar(out=vpe[0:8, :], in0=vpe[0:8, :],
                                        scalar1=float(4.0 * BN_EPS),
                                        scalar2=None, op0=AL.add)
                sdev = wpool.tile([128, 130], f32, tag="sdev")
                nc.scalar.activation(out=sdev[0:8, :], in_=vpe[0:8, :],
                                     func=AF.Sqrt)
                inv = wpool.tile([128, 130], f32, tag="inv")
                nc.vector.reciprocal(out=inv[0:8, :], in_=sdev[0:8, :])
                mbp = spool.tile([128, 130], f32, tag="mbp")
                ibp = spool.tile([128, 130], f32, tag="ibp")
                nc.tensor.matmul(out=mbp[:], lhsT=selbc[0:8, :],
                                 rhs=mean[0:8, :], start=True, stop=True)
                nc.tensor.matmul(out=ibp[:], lhsT=selbc[0:8, :],
                                 rhs=inv[0:8, :], start=True, stop=True)
                nc.vector.tensor_copy(out=mbc[:, c0:c1], in_=mbp[:])
                nc.vector.tensor_copy(out=ibc[:, c0:c1], in_=ibp[:])

            def normalize_tile(ti):
                nfgf = 10 if ti < 25 else 6
                ncol = nfgf * 48
                ssl = sym[:, ti * 480:ti * 480 + ncol]
                t1 = wpool.tile([128, 480], f32, tag="nt")
                nc.vector.tensor_tensor(
                    out=t1[:, :ncol], in0=ssl,
                    in1=_mk_ap(mbc[:], ti * 10, [[1, nfgf], [0, 48]]),
                    op=AL.subtract)
                nc.vector.tensor_tensor(
                    out=ssl, in0=t1[:, :ncol],
                    in1=_mk_ap(ibc[:], ti * 10, [[1, nfgf], [0, 48]]),
                    op=AL.mult)

            for ti in range(NTILE):
                ngrp = GRP_T if ti < 25 else 24
                nfgf = 10 if ti < 25 else 6
                ncol = nfgf * 48
                wmask = wpool.tile([128, GRP_T * 32], bf16, tag="wmask")
                in0 = _mk_ap(zt[:], ti * GRP_T, [[1, ngrp], [0, 32]])
                t0 = _mk_ap(tcode[:], 0, [[0, ngrp], [1, 32]])
                nc.vector.tensor_tensor(
                    out=wmask[:, :ngrp * 32], in0=in0, in1=t0, op=AL.is_equal)
                stp = ppool.tile([128, 480], f32, tag="stp")
                for gi in range(ngrp):
                    g = ti * GRP_T + gi
                    gp, gf = gi % 4, gi // 4
                    rhs = _mk_ap(rsf[:], g, [[AH, P]])
                    nc.tensor.matmul(
                        out=stp[32 * gp:32 * gp + 32, gf * 48:(gf + 1) * 48],
                        lhsT=wmask[:, gi * 32:(gi + 1) * 32],
                        rhs=rhs, start=True, stop=True,
                        tile_position=(0, 32 * gp))
                # park + square on idle ACT; fold rows via sel8 matmul, then
                # tiny per-gf reduces on the [8, ncol] results
                ssl = sym[:, ti * 480:ti * 480 + ncol]
                nc.scalar.activation(out=ssl, in_=stp[:, :ncol], func=AF.Copy)
                sqt = wpool.tile([128, 480], bf16, tag="sqt")
                nc.scalar.activation(out=sqt[:, :ncol], in_=stp[:, :ncol],
                                     func=AF.Square)
                y1 = ypool.tile([128, 480], f32, tag="y1")
                y2 = ypool.tile([128, 480], f32, tag="y2")
                nc.tensor.matmul(out=y1[0:8, :ncol], lhsT=sel8[:, 0:8],
                                 rhs=ssl, start=True, stop=True)
                nc.tensor.matmul(out=y2[0:8, :ncol], lhsT=sel8[:, 0:8],
                                 rhs=sqt[:, :ncol], start=True, stop=True)
                nc.vector.tensor_reduce(
                    out=rs1[0:8, ti * 10:ti * 10 + nfgf],
                    in_=_mk_ap(y1[0:8, :], 0, [[48, nfgf], [1, 48]]),
                    axis=mybir.AxisListType.X, op=AL.add)
                nc.vector.tensor_reduce(
                    out=rs2[0:8, ti * 10:ti * 10 + nfgf],
                    in_=_mk_ap(y2[0:8, :], 0, [[48, nfgf], [1, 48]]),
                    axis=mybir.AxisListType.X, op=AL.add)
                if ti == HALF_T - 1:
                    stats_half(0)
                    epilogue_half(0)
            stats_half(1)
            for ti in range(HALF_T):
                normalize_tile(ti)
            epilogue_half(1)
            for ti in range(HALF_T, NTILE):
                normalize_tile(ti)
            nc.sync.dma_start(out=out_ext[:], in_=sym[:])

    _fix_sync_waits(nc, spares, relay_sem)
    return nc


# ---------------------------------------------------------------- host driver
def _host_tables(rsv, rev):
    from ml_dtypes import bfloat16
    LN2 = float(np.log(2.0))
    tcode = np.full((128, 32), -1.0, np.float32)
    for al in range(4):
        for t5 in range(T):
            tcode[al * 32:(al + 1) * 32, al * 5 + t5] = float(ATOM_TYPES[t5])
    sel8 = np.zeros((128, 8), np.float32)
    selbc = np.zeros((8, 128), np.float32)
    for gp in range(4):
        for al in range(4):
            for t5 in range(T):
                row = 32 * gp + 5 * al + t5
                col = 4 * (al % 2) + gp
                sel8[row, col] = 1.0
                selbc[col, row] = 1.0
    biases = np.zeros((128, 98), np.float32)
    for p in range(P):
        biases[:, p] = -float(rsv[p])
        biases[:, 48 + p] = LN2 - float(rev[p]) * float(rsv[p]) ** 2
    biases[:, 96] = float(np.pi / 2.0)
    biases[:, 97] = LN2
    # output unscramble: [128, OUTW] -> [4096, 240]
    a = np.arange(A)
    al = a // AH
    ah = a % AH
    ti = ah // 40
    rem = ah % 40
    gf = rem // 4
    gp = rem % 4
    c = np.arange(C_OUT)
    t5 = c // P
    p = c % P
    rows = (32 * gp[:, None] + 5 * al[:, None] + t5[None, :]).astype(np.int64)
    cols = ((ti * 480 + gf * 48)[:, None] + p[None, :]).astype(np.int64)
    return (tcode.astype(bfloat16), sel8.astype(bfloat16), selbc, biases,
            rows, cols)


def kernel(X, rc, rs, re, Nbrs, Nbrs_Z):
    from ml_dtypes import bfloat16
    X = np.asarray(X, np.float32)
    rc = np.asarray(rc, np.float32).ravel()
    rs = np.asarray(rs, np.float32).ravel()
    re = np.asarray(re, np.float32).ravel()
    Nbrs = np.asarray(Nbrs, np.int32)
    Nbrs_Z = np.asarray(Nbrs_Z, np.int32)

    nc = build_nc(rc, rs, re)
    tcode, sel8, selbc, biases, orows, ocols = _host_tables(rs, re)

    # per-(a,m)-tile layouts: partition p = (a//1024)*32 + m, free = a % 1024
    in_maps = []
    for core in range(NC_CORES):
        bsl = slice(core * B_LOC, (core + 1) * B_LOC)
        Xc = X[bsl].reshape(A, 3)                       # a = b_loc*2048 + n
        Nb = Nbrs[bsl].reshape(A, M)
        Zb = Nbrs_Z[bsl].reshape(A, M)
        gidx = Nb + (np.arange(A)[:, None] // N) * N    # [A, M] global rows
        a_hi = np.arange(A) % AH
        part = ((np.arange(A) // AH)[:, None] * 32
                + np.arange(M)[None]).astype(np.int32)
        zt = np.zeros((128, AH), np.float32)
        zt[part.ravel(), np.repeat(a_hi, M)] = Zb.ravel().astype(np.float32)
        D = Xc[gidx] - Xc[:, None, :]                   # [A, M, 3]
        Rv = np.sqrt(np.sum(D * D, axis=2, dtype=np.float32))
        rr = np.zeros((128, AH), np.float32)
        rr[part.ravel(), np.repeat(a_hi, M)] = Rv.ravel()
        in_maps.append({
            "rr": rr, "zt": zt.astype(bfloat16), "tcode": tcode,
            "sel8": sel8, "selbc": selbc, "biases": biases,
        })

    res = run_bass_kernel_spmd(nc, in_maps, core_ids=list(range(NC_CORES)),
                               trace=_TRACE[0])
    if _TRACE[0]:
        kernel.last_exec_ns = res.exec_time_ns
        kernel.last_profile = res

    out = np.zeros((B, N, C_OUT), np.float32)
    for core in range(NC_CORES):
        o = np.asarray(res.results[core]["out"]).astype(np.float32)
        out[core * B_LOC:(core + 1) * B_LOC] = o[orows, ocols].reshape(
            B_LOC, N, C_OUT)
    return out


# revision 24
# speedup vs baseline: 2.9784x; 1.0514x over previous
"""AtomicConvolution Trainium2 kernel (8 NeuronCores, data-parallel over B).

Pipeline per core (2 complexes, 4096 atoms, layout [par=(a_lo*32+m), free=a_hi]):
  host computes R (gather + norm) -> radial fn on device in two table-set
  batched ACT phases (Square+Exp phase, then Sin phase; half-angle form
  rsf = (sin(pi/2 - theta/2))^2 * 2*exp(..) avoids the slow 3-operand DVE
  op) writing contiguous-per-p rsf (bf16) -> per-atom-group masked type
  reduction on TensorE (block-diagonal 0/1 weights from is_equal) -> PSUM
  -> sym parked in SBUF (bf16, PSUM-native layout; host unscrambles) ->
  per-atom BN stats via selector matmuls + split AllReduce (first half
  overlaps the remaining tile loop) -> normalize in place -> one out DMA.
  Stored rsf = +2*f; BN epilogue uses eps*4 to compensate.
"""
import sys
import types
import numpy as np

ATOM_TYPES = (1, 6, 7, 8, 16)
BN_EPS = 1e-5
B, N, M, P = 16, 2048, 32, 48
T = len(ATOM_TYPES)
NC_CORES = 8
B_LOC = B // NC_CORES            # 2 complexes per core
A = B_LOC * N                    # 4096 atoms per core
AH = A // 4                      # 1024 free columns
C_OUT = P * T                    # 240 channels
NTILE = 26                       # ceil(1024 / 40) psum tiles
OUTW = NTILE * 480               # 12480 staged output columns
DVE_PREP = 20                    # how many p's compute the exp arg on DVE
HALF_T = 20                      # collective split point (tiles 0..19 | 20..25)
NMC = 20                         # mask columns per group (al*5+t5)

_TRACE = [False]

# ---------------------------------------------------------------- env patches
import concourse.bass as bass
import concourse.mybir as mybir
import concourse.tile as tile
import concourse.bass_utils as bu
from concourse.bass_utils import run_bass_kernel_spmd
from concourse.tile import TileContext, add_dep_helper


def _patch_tile_tail_drain():
    tile_mod = tile
    ScopedClock = None
    for _n in dir(tile_mod):
        if "ScopedClock" in _n:
            ScopedClock = getattr(tile_mod, _n)

    def _drain(self, tick_clock, wait_clock):
        nc = self.nc
        nops = [nc.sync.nop(nofuse=True) for _ in range(30)]
        drain_inst = nc.sync.drain()
        wait_clock.add_sem_waits(
            drain_inst.ins, ScopedClock({None: tick_clock.global_clock})
        )
        si = drain_inst.ins.sync_info
        if si is not None and si.on_wait and len(si.on_wait) > 1:
            waits = list(si.on_wait)
            si.on_wait = waits[:1]
            rest = waits[1:]
            assert len(rest) <= len(nops)
            for i, nop in enumerate(nops):
                chunk = rest[i:i + 1]
                if not chunk:
                    break
                nsi = nop.ins.sync_info
                if nsi is None:
                    nop.ins.sync_info = mybir.SyncInfo(on_wait=chunk, on_update=[])
                else:
                    nsi.on_wait = chunk
        nc.all_engine_barrier()
        popped = nc._tile_sem_poison_stack.pop()
        assert popped is self._sem_poison
        nc.clear_and_free_semaphores(list(self.sems.allocated().values()))
        nc.all_engine_barrier()

    TileContext._drain_and_barrier = _drain


WAIT_CAP = 1


def _make_spare_nops(nc, counts):
    # SP-engine carrier nops: the only engine whose sequencer NoOp reliably
    # encodes with sem waits in this walrus build.
    return {"carriers": [nc.sync.nop(nofuse=True) for _ in range(4000)]}


def _fix_sync_waits(nc, spares, relay):
    clr = nc.sync.sem_clear(relay)
    relay_count = [0]
    carriers = spares["carriers"]
    spare_names = {c.ins.name for c in carriers}
    # move the freshly-appended clear to the very beginning of the first block
    fn0 = nc.m.functions[0]
    for bb in fn0.blocks:
        if clr.ins in bb.instructions:
            bb.instructions.remove(clr.ins)
    fn0.blocks[0].instructions.insert(0, clr.ins)
    for fn in nc.m.functions:
        for bb in fn.blocks:
            bb.instructions[:] = [
                i for i in bb.instructions if i.name not in spare_names
            ]
    for fn in nc.m.functions:
        for bb in fn.blocks:
            new = []
            for inst in bb.instructions:
                si = inst.sync_info
                waits = list(si.on_wait) if si is not None and si.on_wait else []
                if len(waits) > WAIT_CAP:
                    for w in waits:
                        assert carriers, "out of relay carriers"
                        car = carriers.pop()
                        car.then_inc(relay, 1)
                        car.ins.sync_info.on_wait = [w]
                        relay_count[0] += 1
                        new.append(car.ins)
                    si.on_wait = [mybir.SyncWait(
                        sync_type="semaphore", id=relay.num,
                        ant_name=relay.name, wait_mode="sem-ge-imm",
                        wait_value=relay_count[0], wait_reg=None)]
                new.append(inst)
            bb.instructions[:] = new


def _patch_walrus_dyndma(size=16384):
    if getattr(bu.run_command, "_walrus_patched", False):
        return
    _orig = bu.run_command

    def run2(cmd, cwd=None, **kw):
        try:
            if cmd and "walrus_driver" in str(cmd[0]) and any(
                "codegen" in str(c) for c in cmd
            ):
                cmd = list(cmd) + [
                    f"--dynamic-dma-scratch-size-per-partition={size}"
                ]
        except Exception:
            pass
        return _orig(cmd, cwd=cwd, **kw)

    run2._walrus_patched = True
    bu.run_command = run2


def _install_ntff_hook():
    if "antenv.axon_hooks" in sys.modules:
        return
    try:
        from trn_agent_boot.trn_boot import _ntff_profile_via_ctypes
        hook = _ntff_profile_via_ctypes("/opt/axon/libaxon_pjrt.so")
    except Exception:
        hook = None
    m = types.ModuleType("antenv.axon_hooks")
    m._hook = hook
    m.get_axon_ntff_profile_hook = lambda: m._hook
    m.set_axon_ntff_profile_hook = lambda h: setattr(m, "_hook", h)
    sys.modules["antenv.axon_hooks"] = m
    try:
        import antenv
        antenv.axon_hooks = m
    except Exception:
        pass


_patch_tile_tail_drain()
_patch_walrus_dyndma()
_install_ntff_hook()

DT = mybir.dt


def _mk_ap(base_ap, off_elems, free_dims):
    return bass.AP(base_ap.tensor, base_ap.offset + off_elems,
                   [base_ap.ap[0]] + free_dims)


# ---------------------------------------------------------------- bass build
def build_nc(rcv, rsv, rev):
    nc = bass.Bass(dynamic_dma_scratch_size=8192)
    f32, bf16 = DT.float32, DT.bfloat16

    PIH = float(np.pi / 2.0)
    AL = mybir.AluOpType
    AF = mybir.ActivationFunctionType

    rr_ext = nc.declare_dram_parameter("rr", [128, AH], f32, isOutput=False)
    zt_ext = nc.declare_dram_parameter("zt", [128, AH], bf16, isOutput=False)
    tc_ext = nc.declare_dram_parameter("tcode", [128, NMC], bf16,
                                       isOutput=False)
    s8_ext = nc.declare_dram_parameter("sel8", [128, 8], bf16, isOutput=False)
    sb_ext = nc.declare_dram_parameter("selbc", [8, 128], f32, isOutput=False)
    # bias table: col p -> -rs_p (Square bias), col 48+p -> exp-path bias,
    # col 96 -> +pi/2 (Sin), col 97 -> ln2
    bi_ext = nc.declare_dram_parameter("biases", [128, 98], f32, isOutput=False)
    out_ext = nc.declare_dram_parameter("out", [128, OUTW], bf16, isOutput=True)

    st_w = [2 * HALF_T * 10, 2 * (260 - HALF_T * 10)]
    st_in = [nc.dram_tensor(f"st_in{h}", [8, st_w[h]], f32) for h in range(2)]
    st_out = [nc.dram_tensor(f"st_out{h}", [8, st_w[h]], f32,
                             addr_space="Shared") for h in range(2)]

    relay_sem = nc.semaphore("wait_relay").__enter__()
    with TileContext(nc) as tc:
        spares = _make_spare_nops(nc, {})
        with tc.tile_pool(name="main", bufs=1) as pool, \
             tc.tile_pool(name="work", bufs=2) as wpool, \
             tc.tile_pool(name="uprep", bufs=4) as upool, \
             tc.tile_pool(name="rcap", bufs=6) as rpool, \
             tc.tile_pool(name="psum", bufs=2, space="PSUM") as ppool, \
             tc.tile_pool(name="py", bufs=2, space="PSUM") as ypool, \
             tc.tile_pool(name="pstat", bufs=1, space="PSUM") as spool:

            # ---- loads
            zt = pool.tile([128, AH], bf16)
            nc.sync.dma_start(out=zt[:], in_=zt_ext[:])
            tcode = pool.tile([128, NMC], bf16)
            nc.sync.dma_start(out=tcode[:], in_=tc_ext[:])
            sel8 = pool.tile([128, 8], bf16)
            nc.sync.dma_start(out=sel8[:], in_=s8_ext[:])
            selbc = pool.tile([128, 128], f32)
            nc.sync.dma_start(out=selbc[0:8, :], in_=sb_ext[:])
            bia = pool.tile([128, 98], f32)
            nc.sync.dma_start(out=bia[:], in_=bi_ext[:])
            rr = pool.tile([128, AH], f32)
            nc.sync.dma_start(out=rr[:], in_=rr_ext[:])
            r2 = pool.tile([128, AH], f32)
            nc.vector.tensor_tensor(out=r2[:], in0=rr[:], in1=rr[:],
                                    op=AL.mult)

            # rsf layout: contiguous per p -> col = p*AH + a_hi
            rsf = pool.tile([128, P * AH], bf16)

            # ---- phase A: kk'_p = 2*exp(-re*(R-rs)^2), bf16.
            # ACT-path p's (Square+Exp, one table set) run first so ACT
            # starts immediately; DVE_PREP p's build the exp arg on DVE
            # meanwhile (deeper uprep pool so DVE runs ahead).
            last_exp = None
            keys = []
            for p in range(P):
                if p < DVE_PREP:
                    keys.append(((p + 0.5) / DVE_PREP, 1, p))
                else:
                    keys.append(((p - DVE_PREP + 0.5) / (P - DVE_PREP), 0, p))
            p_order = [p for _, _, p in sorted(keys)]
            for p in p_order:
                re_p, rs_p = float(rev[p]), float(rsv[p])
                if p < DVE_PREP:
                    t1 = wpool.tile([128, AH], f32, tag="t1")
                    nc.vector.tensor_scalar(
                        out=t1[:], in0=rr[:], scalar1=-2.0 * rs_p,
                        scalar2=None, op0=AL.mult)
                    u = upool.tile([128, AH], f32, tag="u")
                    nc.vector.tensor_tensor(out=u[:], in0=t1[:], in1=r2[:],
                                            op=AL.add)
                    # exp(-re*u + (ln2 - re*rs^2)) = 2*exp(-re*(R-rs)^2)
                    ei = nc.scalar.activation(
                        out=rsf[:, p * AH:(p + 1) * AH], in_=u[:],
                        func=AF.Exp, scale=-re_p,
                        bias=bia[:, 48 + p:49 + p])
                else:
                    ua = wpool.tile([128, AH], f32, tag="ua")
                    nc.scalar.activation(out=ua[:], in_=rr[:], func=AF.Square,
                                         bias=bia[:, p:p + 1])
                    ei = nc.scalar.activation(
                        out=rsf[:, p * AH:(p + 1) * AH], in_=ua[:],
                        func=AF.Exp, scale=-re_p,
                        bias=bia[:, 97:98])
                last_exp = ei

            # ---- phase B: s = sin(pi/2 - pi*min(R,rc)/(2rc)) (>=0, LUT-safe)
            # rsf *= s*s  ->  rsf = 2*f_p.  Sins forced after all Exps so the
            # ACT table set switches exactly once; rcap pool is deep so DVE
            # computes sin args well ahead.
            for p in range(P):
                rc_p = float(rcv[p])
                rt = rpool.tile([128, AH], bf16, tag="rt")
                nc.vector.tensor_scalar(
                    out=rt[:], in0=rr[:], scalar1=rc_p,
                    scalar2=float(np.pi / (2.0 * rc_p)),
                    op0=AL.min, op1=AL.mult)
                cs = wpool.tile([128, AH], bf16, tag="cs")
                si = nc.scalar.activation(out=cs[:], in_=rt[:], func=AF.Sin,
                                          scale=-1.0, bias=bia[:, 96:97])
                add_dep_helper(si.ins, last_exp.ins,
                               reason="keep Sin phase after Exp phase")
                s2t = wpool.tile([128, AH], bf16, tag="s2t")
                nc.vector.tensor_tensor(out=s2t[:], in0=cs[:], in1=cs[:],
                                        op=AL.mult)
                psl = rsf[:, p * AH:(p + 1) * AH]
                nc.vector.tensor_tensor(out=psl, in0=s2t[:], in1=psl,
                                        op=AL.mult)

            # ---- TensorE masked reduction; sym parked in SBUF (bf16)
            # group g = one a_hi; psum tile: rows 32*gp + (al*5+t5),
            # cols gf*48 + p, for g = ti*40 + gf*4 + gp
            sym = pool.tile([128, OUTW], bf16)
            rs1 = pool.tile([128, 260], f32)
            rs2 = pool.tile([128, 260], f32)
            mbc = pool.tile([128, 260], f32)
            ibc = pool.tile([128, 260], f32)
            sall = [None, None]
            GRP_T = 40

            def stats_half(h):
                # per-n stats already folded per tile; just ship + AllReduce
                c0, c1 = (0, HALF_T * 10) if h == 0 else (HALF_T * 10, 260)
                w = c1 - c0
                nc.sync.dma_start(out=st_in[h][:, 0:w], in_=rs1[0:8, c0:c1])
                nc.sync.dma_start(out=st_in[h][:, w:2 * w],
                                  in_=rs2[0:8, c0:c1])
                nc.gpsimd.collective_compute(
                    "AllReduce", AL.add,
                    ins=[st_in[h][:]], outs=[st_out[h][:]],
                    replica_groups=[list(range(NC_CORES))])
                sa = pool.tile([128, 2 * w], f32)
                nc.sync.dma_start(out=sa[0:8, :], in_=st_out[h][:])
                sall[h] = sa

            def epilogue_half(h):
                # mean/inv on [8,w], broadcast to 128 rows via matmul
                c0, c1 = (0, HALF_T * 10) if h == 0 else (HALF_T * 10, 260)
                w = c1 - c0
                sa = sall[h]
                inv_n = 1.0 / (B * C_OUT)
                mean = wpool.tile([128, 200], f32, tag="mean")
                nc.vector.tensor_scalar(out=mean[0:8, :w], in0=sa[0:8, 0:w],
                                        scalar1=inv_n, scalar2=None,
                                        op0=AL.mult)
                vpe = wpool.tile([128, 200], f32, tag="vpe")
                nc.vector.tensor_scalar(out=vpe[0:8, :w], in0=sa[0:8, w:2 * w],
                                        scalar1=inv_n, scalar2=None,
                                        op0=AL.mult)
                msq = wpool.tile([128, 200], f32, tag="msq")
                nc.vector.tensor_tensor(out=msq[0:8, :w], in0=mean[0:8, :w],
                                        in1=mean[0:8, :w], op=AL.mult)
                nc.vector.tensor_tensor(out=vpe[0:8, :w], in0=vpe[0:8, :w],
                                        in1=msq[0:8, :w], op=AL.subtract)
                nc.vector.tensor_scalar(out=vpe[0:8, :w], in0=vpe[0:8, :w],
                                        scalar1=float(4.0 * BN_EPS),
                                        scalar2=None, op0=AL.add)
                sdev = wpool.tile([128, 200], f32, tag="sdev")
                nc.scalar.activation(out=sdev[0:8, :w], in_=vpe[0:8, :w],
                                     func=AF.Sqrt)
                inv = wpool.tile([128, 200], f32, tag="inv")
                nc.vector.reciprocal(out=inv[0:8, :w], in_=sdev[0:8, :w])
                mbp = spool.tile([128, 200], f32, tag="mbp")
                ibp = spool.tile([128, 200], f32, tag="ibp")
                nc.tensor.matmul(out=mbp[:, :w], lhsT=selbc[0:8, :],
                                 rhs=mean[0:8, :w], start=True, stop=True)
                nc.tensor.matmul(out=ibp[:, :w], lhsT=selbc[0:8, :],
                                 rhs=inv[0:8, :w], start=True, stop=True)
                nc.vector.tensor_copy(out=mbc[:, c0:c1], in_=mbp[:, :w])
                nc.vector.tensor_copy(out=ibc[:, c0:c1], in_=ibp[:, :w])

            def normalize_tile(ti):
                nfgf = 10 if ti < 25 else 6
                ncol = nfgf * 48
                ssl = sym[:, ti * 480:ti * 480 + ncol]
                t1 = wpool.tile([128, 480], f32, tag="nt")
                nc.vector.tensor_tensor(
                    out=t1[:, :ncol], in0=ssl,
                    in1=_mk_ap(mbc[:], ti * 10, [[1, nfgf], [0, 48]]),
                    op=AL.subtract)
                nc.vector.tensor_tensor(
                    out=ssl, in0=t1[:, :ncol],
                    in1=_mk_ap(ibc[:], ti * 10, [[1, nfgf], [0, 48]]),
                    op=AL.mult)

            for ti in range(NTILE):
                ngrp = GRP_T if ti < 25 else 24
                nfgf = 10 if ti < 25 else 6
                ncol = nfgf * 48
                wmask = wpool.tile([128, GRP_T * NMC], bf16, tag="wmask")
                in0 = _mk_ap(zt[:], ti * GRP_T, [[1, ngrp], [0, NMC]])
                t0 = _mk_ap(tcode[:], 0, [[0, ngrp], [1, NMC]])
                nc.vector.tensor_tensor(
                    out=wmask[:, :ngrp * NMC], in0=in0, in1=t0, op=AL.is_equal)
                stp = ppool.tile([128, 480], f32, tag="stp")
                if ti < 2:
                    # 20-col masks leave psum rows 20..31 of each strip
                    # unwritten; clear once so parked garbage is finite
                    nc.vector.memset(stp[:], 0.0)
                for gi in range(ngrp):
                    g = ti * GRP_T + gi
                    gp, gf = gi % 4, gi // 4
                    rhs = _mk_ap(rsf[:], g, [[AH, P]])
                    nc.tensor.matmul(
                        out=stp[32 * gp:32 * gp + 20, gf * 48:(gf + 1) * 48],
                        lhsT=wmask[:, gi * NMC:(gi + 1) * NMC],
                        rhs=rhs, start=True, stop=True,
                        tile_position=(0, 32 * gp))
                # park + square on idle ACT; fold rows via sel8 matmul, then
                # tiny per-gf reduces on the [8, ncol] results
                ssl = sym[:, ti * 480:ti * 480 + ncol]
                nc.scalar.activation(out=ssl, in_=stp[:, :ncol], func=AF.Copy)
                sqt = wpool.tile([128, 480], bf16, tag="sqt")
                nc.scalar.activation(out=sqt[:, :ncol], in_=stp[:, :ncol],
                                     func=AF.Square)
                y1 = ypool.tile([128, 480], f32, tag="y1")
                y2 = ypool.tile([128, 480], f32, tag="y2")
                nc.tensor.matmul(out=y1[0:8, :ncol], lhsT=sel8[:, 0:8],
                                 rhs=ssl, start=True, stop=True)
                nc.tensor.matmul(out=y2[0:8, :ncol], lhsT=sel8[:, 0:8],
                                 rhs=sqt[:, :ncol], start=True, stop=True)
                nc.vector.tensor_reduce(
                    out=rs1[0:8, ti * 10:ti * 10 + nfgf],
                    in_=_mk_ap(y1[0:8, :], 0, [[48, nfgf], [1, 48]]),
                    axis=mybir.AxisListType.X, op=AL.add)
                nc.vector.tensor_reduce(
                    out=rs2[0:8, ti * 10:ti * 10 + nfgf],
                    in_=_mk_ap(y2[0:8, :], 0, [[48, nfgf], [1, 48]]),
                    axis=mybir.AxisListType.X, op=AL.add)
                if ti == HALF_T - 1:
                    stats_half(0)
                    epilogue_half(0)
            stats_half(1)
            for ti in range(HALF_T):
                normalize_tile(ti)
            epilogue_half(1)
            for ti in range(HALF_T, NTILE):
                normalize_tile(ti)
            nc.sync.dma_start(out=out_ext[:], in_=sym[:])

    _fix_sync_waits(nc, spares, relay_sem)
    return nc


# ---------------------------------------------------------------- host driver
def _host_tables(rsv, rev):
    from ml_dtypes import bfloat16
    LN2 = float(np.log(2.0))
    tcode = np.full((128, NMC), -1.0, np.float32)
    for al in range(4):
        for t5 in range(T):
            tcode[al * 32:(al + 1) * 32, al * 5 + t5] = float(ATOM_TYPES[t5])
    sel8 = np.zeros((128, 8), np.float32)
    selbc = np.zeros((8, 128), np.float32)
    for gp in range(4):
        for al in range(4):
            for t5 in range(T):
                row = 32 * gp + 5 * al + t5
                col = 4 * (al % 2) + gp
                sel8[row, col] = 1.0
                selbc[col, row] = 1.0
    biases = np.zeros((128, 98), np.float32)
    for p in range(P):
        biases[:, p] = -float(rsv[p])
        biases[:, 48 + p] = LN2 - float(rev[p]) * float(rsv[p]) ** 2
    biases[:, 96] = float(np.pi / 2.0)
    biases[:, 97] = LN2
    # output unscramble: [128, OUTW] -> [4096, 240]
    a = np.arange(A)
    al = a // AH
    ah = a % AH
    ti = ah // 40
    rem = ah % 40
    gf = rem // 4
    gp = rem % 4
    c = np.arange(C_OUT)
    t5 = c // P
    p = c % P
    rows = (32 * gp[:, None] + 5 * al[:, None] + t5[None, :]).astype(np.int64)
    cols = ((ti * 480 + gf * 48)[:, None] + p[None, :]).astype(np.int64)
    return (tcode.astype(bfloat16), sel8.astype(bfloat16), selbc, biases,
            rows, cols)


def kernel(X, rc, rs, re, Nbrs, Nbrs_Z):
    from ml_dtypes import bfloat16
    X = np.asarray(X, np.float32)
    rc = np.asarray(rc, np.float32).ravel()
    rs = np.asarray(rs, np.float32).ravel()
    re = np.asarray(re, np.float32).ravel()
    Nbrs = np.asarray(Nbrs, np.int32)
    Nbrs_Z = np.asarray(Nbrs_Z, np.int32)

    nc = build_nc(rc, rs, re)
    tcode, sel8, selbc, biases, orows, ocols = _host_tables(rs, re)

    # per-(a,m)-tile layouts: partition p = (a//1024)*32 + m, free = a % 1024
    in_maps = []
    for core in range(NC_CORES):
        bsl = slice(core * B_LOC, (core + 1) * B_LOC)
        Xc = X[bsl].reshape(A, 3)                       # a = b_loc*2048 + n
        Nb = Nbrs[bsl].reshape(A, M)
        Zb = Nbrs_Z[bsl].reshape(A, M)
        gidx = Nb + (np.arange(A)[:, None] // N) * N    # [A, M] global rows
        a_hi = np.arange(A) % AH
        part = ((np.arange(A) // AH)[:, None] * 32
                + np.arange(M)[None]).astype(np.int32)
        zt = np.zeros((128, AH), np.float32)
        zt[part.ravel(), np.repeat(a_hi, M)] = Zb.ravel().astype(np.float32)
        D = Xc[gidx] - Xc[:, None, :]                   # [A, M, 3]
        Rv = np.sqrt(np.sum(D * D, axis=2, dtype=np.float32))
        rr = np.zeros((128, AH), np.float32)
        rr[part.ravel(), np.repeat(a_hi, M)] = Rv.ravel()
        in_maps.append({
            "rr": rr, "zt": zt.astype(bfloat16), "tcode": tcode,
            "sel8": sel8, "selbc": selbc, "biases": biases,
        })

    res = run_bass_kernel_spmd(nc, in_maps, core_ids=list(range(NC_CORES)),
                               trace=_TRACE[0])
    if _TRACE[0]:
        kernel.last_exec_ns = res.exec_time_ns
        kernel.last_profile = res

    out = np.zeros((B, N, C_OUT), np.float32)
    for core in range(NC_CORES):
        o = np.asarray(res.results[core]["out"]).astype(np.float32)
        out[core * B_LOC:(core + 1) * B_LOC] = o[orows, ocols].reshape(
            B_LOC, N, C_OUT)
    return out


# revision 30
# speedup vs baseline: 3.1228x; 1.0485x over previous
"""AtomicConvolution Trainium2 kernel (8 NeuronCores, data-parallel over B).

Pipeline per core (2 complexes, 4096 atoms, layout [par=(a_lo*32+m), free=a_hi]):
  host computes R (gather + norm) -> radial fn on device in two table-set
  batched ACT phases (Square+Exp phase, then Sin phase; half-angle form
  rsf = (sin(pi/2 - theta/2))^2 * 2*exp(..) avoids the slow 3-operand DVE
  op) writing contiguous-per-p rsf (bf16) -> per-atom-group masked type
  reduction on TensorE (block-diagonal 0/1 weights from is_equal) -> PSUM
  -> sym parked in SBUF (bf16, PSUM-native layout; host unscrambles) ->
  per-atom BN stats via selector matmuls + split AllReduce (first half
  overlaps the remaining tile loop) -> normalize in place -> one out DMA.
  Stored rsf = +2*f; BN epilogue uses eps*4 to compensate.
"""
import sys
import types
import numpy as np

ATOM_TYPES = (1, 6, 7, 8, 16)
BN_EPS = 1e-5
B, N, M, P = 16, 2048, 32, 48
T = len(ATOM_TYPES)
NC_CORES = 8
B_LOC = B // NC_CORES            # 2 complexes per core
A = B_LOC * N                    # 4096 atoms per core
AH = A // 4                      # 1024 free columns
C_OUT = P * T                    # 240 channels
NTILE = 26                       # ceil(1024 / 40) psum tiles
OUTW = NTILE * 480               # 12480 staged output columns
DVE_PREP = 26                    # how many p's compute the exp arg on DVE
ACT_SQ = 16                      # how many p's square the sin on ACT
HALF_T = 20                      # collective split point (tiles 0..19 | 20..25)
NMC = 20                         # mask columns per group (al*5+t5)

_TRACE = [False]

# ---------------------------------------------------------------- env patches
import concourse.bass as bass
import concourse.mybir as mybir
import concourse.tile as tile
import concourse.bass_utils as bu
from concourse.bass_utils import run_bass_kernel_spmd
from concourse.tile import TileContext, add_dep_helper


def _patch_tile_tail_drain():
    tile_mod = tile
    ScopedClock = None
    for _n in dir(tile_mod):
        if "ScopedClock" in _n:
            ScopedClock = getattr(tile_mod, _n)

    def _drain(self, tick_clock, wait_clock):
        nc = self.nc
        nops = [nc.sync.nop(nofuse=True) for _ in range(30)]
        drain_inst = nc.sync.drain()
        wait_clock.add_sem_waits(
            drain_inst.ins, ScopedClock({None: tick_clock.global_clock})
        )
        si = drain_inst.ins.sync_info
        if si is not None and si.on_wait and len(si.on_wait) > 1:
            waits = list(si.on_wait)
            si.on_wait = waits[:1]
            rest = waits[1:]
            assert len(rest) <= len(nops)
            for i, nop in enumerate(nops):
                chunk = rest[i:i + 1]
                if not chunk:
                    break
                nsi = nop.ins.sync_info
                if nsi is None:
                    nop.ins.sync_info = mybir.SyncInfo(on_wait=chunk, on_update=[])
                else:
                    nsi.on_wait = chunk
        nc.all_engine_barrier()
        popped = nc._tile_sem_poison_stack.pop()
        assert popped is self._sem_poison
        nc.clear_and_free_semaphores(list(self.sems.allocated().values()))
        nc.all_engine_barrier()

    TileContext._drain_and_barrier = _drain


WAIT_CAP = 1


def _make_spare_nops(nc, counts):
    # SP-engine carrier nops: the only engine whose sequencer NoOp reliably
    # encodes with sem waits in this walrus build.
    return {"carriers": [nc.sync.nop(nofuse=True) for _ in range(4000)]}


def _fix_sync_waits(nc, spares, relay):
    clr = nc.sync.sem_clear(relay)
    relay_count = [0]
    carriers = spares["carriers"]
    spare_names = {c.ins.name for c in carriers}
    # move the freshly-appended clear to the very beginning of the first block
    fn0 = nc.m.functions[0]
    for bb in fn0.blocks:
        if clr.ins in bb.instructions:
            bb.instructions.remove(clr.ins)
    fn0.blocks[0].instructions.insert(0, clr.ins)
    for fn in nc.m.functions:
        for bb in fn.blocks:
            bb.instructions[:] = [
                i for i in bb.instructions if i.name not in spare_names
            ]
    for fn in nc.m.functions:
        for bb in fn.blocks:
            new = []
            for inst in bb.instructions:
                si = inst.sync_info
                waits = list(si.on_wait) if si is not None and si.on_wait else []
                if len(waits) > WAIT_CAP:
                    for w in waits:
                        assert carriers, "out of relay carriers"
                        car = carriers.pop()
                        car.then_inc(relay, 1)
                        car.ins.sync_info.on_wait = [w]
                        relay_count[0] += 1
                        new.append(car.ins)
                    si.on_wait = [mybir.SyncWait(
                        sync_type="semaphore", id=relay.num,
                        ant_name=relay.name, wait_mode="sem-ge-imm",
                        wait_value=relay_count[0], wait_reg=None)]
                new.append(inst)
            bb.instructions[:] = new


def _patch_walrus_dyndma(size=16384):
    if getattr(bu.run_command, "_walrus_patched", False):
        return
    _orig = bu.run_command

    def run2(cmd, cwd=None, **kw):
        try:
            if cmd and "walrus_driver" in str(cmd[0]) and any(
                "codegen" in str(c) for c in cmd
            ):
                cmd = list(cmd) + [
                    f"--dynamic-dma-scratch-size-per-partition={size}"
                ]
        except Exception:
            pass
        return _orig(cmd, cwd=cwd, **kw)

    run2._walrus_patched = True
    bu.run_command = run2


def _install_ntff_hook():
    if "antenv.axon_hooks" in sys.modules:
        return
    try:
        from trn_agent_boot.trn_boot import _ntff_profile_via_ctypes
        hook = _ntff_profile_via_ctypes("/opt/axon/libaxon_pjrt.so")
    except Exception:
        hook = None
    m = types.ModuleType("antenv.axon_hooks")
    m._hook = hook
    m.get_axon_ntff_profile_hook = lambda: m._hook
    m.set_axon_ntff_profile_hook = lambda h: setattr(m, "_hook", h)
    sys.modules["antenv.axon_hooks"] = m
    try:
        import antenv
        antenv.axon_hooks = m
    except Exception:
        pass


_patch_tile_tail_drain()
_patch_walrus_dyndma()
_install_ntff_hook()

DT = mybir.dt


def _mk_ap(base_ap, off_elems, free_dims):
    return bass.AP(base_ap.tensor, base_ap.offset + off_elems,
                   [base_ap.ap[0]] + free_dims)


# ---------------------------------------------------------------- bass build
def build_nc(rcv, rsv, rev):
    nc = bass.Bass(dynamic_dma_scratch_size=8192)
    f32, bf16 = DT.float32, DT.bfloat16

    PIH = float(np.pi / 2.0)
    AL = mybir.AluOpType
    AF = mybir.ActivationFunctionType

    rr_ext = nc.declare_dram_parameter("rr", [128, AH], f32, isOutput=False)
    zt_ext = nc.declare_dram_parameter("zt", [128, AH], bf16, isOutput=False)
    tc_ext = nc.declare_dram_parameter("tcode", [128, NMC], bf16,
                                       isOutput=False)
    s8_ext = nc.declare_dram_parameter("sel8", [128, 8], bf16, isOutput=False)
    sb_ext = nc.declare_dram_parameter("selbc", [8, 128], f32, isOutput=False)
    # bias table: col p -> -rs_p (Square bias), col 48+p -> exp-path bias,
    # col 96 -> +pi/2 (Sin), col 97 -> ln2
    bi_ext = nc.declare_dram_parameter("biases", [128, 98], f32, isOutput=False)
    out_ext = nc.declare_dram_parameter("out", [128, OUTW], bf16, isOutput=True)

    st_w = [2 * HALF_T * 10, 2 * (260 - HALF_T * 10)]
    st_in = [nc.dram_tensor(f"st_in{h}", [8, st_w[h]], f32) for h in range(2)]
    st_out = [nc.dram_tensor(f"st_out{h}", [8, st_w[h]], f32,
                             addr_space="Shared") for h in range(2)]

    relay_sem = nc.semaphore("wait_relay").__enter__()
    with TileContext(nc) as tc:
        spares = _make_spare_nops(nc, {})
        with tc.tile_pool(name="main", bufs=1) as pool, \
             tc.tile_pool(name="work", bufs=2) as wpool, \
             tc.tile_pool(name="uprep", bufs=4) as upool, \
             tc.tile_pool(name="rcap", bufs=6) as rpool, \
             tc.tile_pool(name="psum", bufs=2, space="PSUM") as ppool, \
             tc.tile_pool(name="py", bufs=2, space="PSUM") as ypool, \
             tc.tile_pool(name="pstat", bufs=1, space="PSUM") as spool:

            # ---- loads
            zt = pool.tile([128, AH], bf16)
            nc.sync.dma_start(out=zt[:], in_=zt_ext[:])
            tcode = pool.tile([128, NMC], bf16)
            nc.sync.dma_start(out=tcode[:], in_=tc_ext[:])
            sel8 = pool.tile([128, 8], bf16)
            nc.sync.dma_start(out=sel8[:], in_=s8_ext[:])
            selbc = pool.tile([128, 128], f32)
            nc.sync.dma_start(out=selbc[0:8, :], in_=sb_ext[:])
            bia = pool.tile([128, 98], f32)
            nc.sync.dma_start(out=bia[:], in_=bi_ext[:])
            rr = pool.tile([128, AH], f32)
            nc.sync.dma_start(out=rr[:], in_=rr_ext[:])
            r2 = pool.tile([128, AH], f32)
            nc.vector.tensor_tensor(out=r2[:], in0=rr[:], in1=rr[:],
                                    op=AL.mult)

            # rsf layout: contiguous per p -> col = p*AH + a_hi
            rsf = pool.tile([128, P * AH], bf16)

            # ---- phase A: kk'_p = 2*exp(-re*(R-rs)^2), bf16.
            # ACT-path p's (Square+Exp, one table set) run first so ACT
            # starts immediately; DVE_PREP p's build the exp arg on DVE
            # meanwhile (deeper uprep pool so DVE runs ahead).
            last_exp = None
            keys = []
            for p in range(P):
                if p < DVE_PREP:
                    keys.append(((p + 0.5) / DVE_PREP, 1, p))
                else:
                    keys.append(((p - DVE_PREP + 0.5) / (P - DVE_PREP), 0, p))
            p_order = [p for _, _, p in sorted(keys)]
            for p in p_order:
                re_p, rs_p = float(rev[p]), float(rsv[p])
                if p < DVE_PREP:
                    t1 = wpool.tile([128, AH], f32, tag="t1")
                    nc.vector.tensor_scalar(
                        out=t1[:], in0=rr[:], scalar1=-2.0 * rs_p,
                        scalar2=None, op0=AL.mult)
                    u = upool.tile([128, AH], f32, tag="u")
                    nc.vector.tensor_tensor(out=u[:], in0=t1[:], in1=r2[:],
                                            op=AL.add)
                    # exp(-re*u + (ln2 - re*rs^2)) = 2*exp(-re*(R-rs)^2)
                    ei = nc.scalar.activation(
                        out=rsf[:, p * AH:(p + 1) * AH], in_=u[:],
                        func=AF.Exp, scale=-re_p,
                        bias=bia[:, 48 + p:49 + p])
                else:
                    ua = wpool.tile([128, AH], f32, tag="ua")
                    nc.scalar.activation(out=ua[:], in_=rr[:], func=AF.Square,
                                         bias=bia[:, p:p + 1])
                    ei = nc.scalar.activation(
                        out=rsf[:, p * AH:(p + 1) * AH], in_=ua[:],
                        func=AF.Exp, scale=-re_p,
                        bias=bia[:, 97:98])
                last_exp = ei

            # ---- phase B: s = sin(pi/2 - pi*min(R,rc)/(2rc)) (>=0, LUT-safe)
            # rsf *= s*s  ->  rsf = 2*f_p.  Sins forced after all Exps so the
            # ACT table set switches exactly once; rcap pool is deep so DVE
            # computes sin args well ahead.
            for p in range(P):
                rc_p = float(rcv[p])
                rt = rpool.tile([128, AH], bf16, tag="rt")
                nc.vector.tensor_scalar(
                    out=rt[:], in0=rr[:], scalar1=rc_p,
                    scalar2=float(np.pi / (2.0 * rc_p)),
                    op0=AL.min, op1=AL.mult)
                cs = wpool.tile([128, AH], bf16, tag="cs")
                si = nc.scalar.activation(out=cs[:], in_=rt[:], func=AF.Sin,
                                          scale=-1.0, bias=bia[:, 96:97])
                add_dep_helper(si.ins, last_exp.ins,
                               reason="keep Sin phase after Exp phase")
                s2t = wpool.tile([128, AH], bf16, tag="s2t")
                if p < ACT_SQ:
                    # Square is in every table set -> no reload
                    nc.scalar.activation(out=s2t[:], in_=cs[:],
                                         func=AF.Square)
                else:
                    nc.vector.tensor_tensor(out=s2t[:], in0=cs[:], in1=cs[:],
                                            op=AL.mult)
                psl = rsf[:, p * AH:(p + 1) * AH]
                nc.vector.tensor_tensor(out=psl, in0=s2t[:], in1=psl,
                                        op=AL.mult)

            # ---- TensorE masked reduction; sym parked in SBUF (bf16)
            # group g = one a_hi; psum tile: rows 32*gp + (al*5+t5),
            # cols gf*48 + p, for g = ti*40 + gf*4 + gp
            sym = pool.tile([128, OUTW], bf16)
            rst = pool.tile([128, 260], f32)   # rows 0:8 = s1, 32:40 = s2
            mbc = pool.tile([128, 260], f32)
            ibc = pool.tile([128, 260], f32)
            sall = [None, None]
            GRP_T = 40

            def stats_half(h):
                # per-n stats already folded per tile; just ship + AllReduce
                c0, c1 = (0, HALF_T * 10) if h == 0 else (HALF_T * 10, 260)
                w = c1 - c0
                nc.sync.dma_start(out=st_in[h][:, 0:w], in_=rst[0:8, c0:c1])
                nc.sync.dma_start(out=st_in[h][:, w:2 * w],
                                  in_=rst[32:40, c0:c1])
                nc.gpsimd.collective_compute(
                    "AllReduce", AL.add,
                    ins=[st_in[h][:]], outs=[st_out[h][:]],
                    replica_groups=[list(range(NC_CORES))])
                sa = pool.tile([128, 2 * w], f32)
                nc.sync.dma_start(out=sa[0:8, :], in_=st_out[h][:])
                sall[h] = sa

            def epilogue_half(h):
                # mean/inv on [8,w], broadcast to 128 rows via matmul
                c0, c1 = (0, HALF_T * 10) if h == 0 else (HALF_T * 10, 260)
                w = c1 - c0
                sa = sall[h]
                inv_n = 1.0 / (B * C_OUT)
                mean = wpool.tile([128, 200], f32, tag="mean")
                nc.vector.tensor_scalar(out=mean[0:8, :w], in0=sa[0:8, 0:w],
                                        scalar1=inv_n, scalar2=None,
                                        op0=AL.mult)
                vpe = wpool.tile([128, 200], f32, tag="vpe")
                nc.vector.tensor_scalar(out=vpe[0:8, :w], in0=sa[0:8, w:2 * w],
                                        scalar1=inv_n, scalar2=None,
                                        op0=AL.mult)
                msq = wpool.tile([128, 200], f32, tag="msq")
                nc.vector.tensor_tensor(out=msq[0:8, :w], in0=mean[0:8, :w],
                                        in1=mean[0:8, :w], op=AL.mult)
                nc.vector.tensor_tensor(out=vpe[0:8, :w], in0=vpe[0:8, :w],
                                        in1=msq[0:8, :w], op=AL.subtract)
                nc.vector.tensor_scalar(out=vpe[0:8, :w], in0=vpe[0:8, :w],
                                        scalar1=float(4.0 * BN_EPS),
                                        scalar2=None, op0=AL.add)
                sdev = wpool.tile([128, 200], f32, tag="sdev")
                nc.scalar.activation(out=sdev[0:8, :w], in_=vpe[0:8, :w],
                                     func=AF.Sqrt)
                inv = wpool.tile([128, 200], f32, tag="inv")
                nc.vector.reciprocal(out=inv[0:8, :w], in_=sdev[0:8, :w])
                mbp = spool.tile([128, 200], f32, tag="mbp")
                ibp = spool.tile([128, 200], f32, tag="ibp")
                nc.tensor.matmul(out=mbp[:, :w], lhsT=selbc[0:8, :],
                                 rhs=mean[0:8, :w], start=True, stop=True)
                nc.tensor.matmul(out=ibp[:, :w], lhsT=selbc[0:8, :],
                                 rhs=inv[0:8, :w], start=True, stop=True)
                nc.vector.tensor_copy(out=mbc[:, c0:c1], in_=mbp[:, :w])
                nc.vector.tensor_copy(out=ibc[:, c0:c1], in_=ibp[:, :w])

            def normalize_tile(ti):
                nfgf = 10 if ti < 25 else 6
                ncol = nfgf * 48
                ssl = sym[:, ti * 480:ti * 480 + ncol]
                t1 = wpool.tile([128, 480], f32, tag="nt")
                nc.vector.tensor_tensor(
                    out=t1[:, :ncol], in0=ssl,
                    in1=_mk_ap(mbc[:], ti * 10, [[1, nfgf], [0, 48]]),
                    op=AL.subtract)
                nc.vector.tensor_tensor(
                    out=ssl, in0=t1[:, :ncol],
                    in1=_mk_ap(ibc[:], ti * 10, [[1, nfgf], [0, 48]]),
                    op=AL.mult)

            for ti in range(NTILE):
                ngrp = GRP_T if ti < 25 else 24
                nfgf = 10 if ti < 25 else 6
                ncol = nfgf * 48
                wmask = wpool.tile([128, GRP_T * NMC], bf16, tag="wmask")
                in0 = _mk_ap(zt[:], ti * GRP_T, [[1, ngrp], [0, NMC]])
                t0 = _mk_ap(tcode[:], 0, [[0, ngrp], [1, NMC]])
                nc.vector.tensor_tensor(
                    out=wmask[:, :ngrp * NMC], in0=in0, in1=t0, op=AL.is_equal)
                stp = ppool.tile([128, 480], f32, tag="stp")
                if ti < 2:
                    # 20-col masks leave psum rows 20..31 of each strip
                    # unwritten; clear once so parked garbage is finite
                    nc.vector.memset(stp[:], 0.0)
                for gi in range(ngrp):
                    g = ti * GRP_T + gi
                    gp, gf = gi % 4, gi // 4
                    rhs = _mk_ap(rsf[:], g, [[AH, P]])
                    nc.tensor.matmul(
                        out=stp[32 * gp:32 * gp + 20, gf * 48:(gf + 1) * 48],
                        lhsT=wmask[:, gi * NMC:(gi + 1) * NMC],
                        rhs=rhs, start=True, stop=True,
                        tile_position=(0, 32 * gp))
                # park + square on idle ACT; fold rows via sel8 matmul, then
                # tiny per-gf reduces on the [8, ncol] results
                ssl = sym[:, ti * 480:ti * 480 + ncol]
                nc.scalar.activation(out=ssl, in_=stp[:, :ncol], func=AF.Copy)
                sqt = wpool.tile([128, 480], bf16, tag="sqt")
                nc.scalar.activation(out=sqt[:, :ncol], in_=stp[:, :ncol],
                                     func=AF.Square)
                y1 = ypool.tile([128, 480], f32, tag="y1")
                if ti < 2:
                    nc.vector.memset(y1[:], 0.0)
                nc.tensor.matmul(out=y1[0:8, :ncol], lhsT=sel8[:, 0:8],
                                 rhs=ssl, start=True, stop=True)
                nc.tensor.matmul(out=y1[32:40, :ncol], lhsT=sel8[:, 0:8],
                                 rhs=sqt[:, :ncol], start=True, stop=True,
                                 tile_position=(0, 32))
                nc.vector.tensor_reduce(
                    out=rst[0:40, ti * 10:ti * 10 + nfgf],
                    in_=_mk_ap(y1[0:40, :], 0, [[48, nfgf], [1, 48]]),
                    axis=mybir.AxisListType.X, op=AL.add)
                if ti == HALF_T - 1:
                    stats_half(0)
                    epilogue_half(0)
            stats_half(1)
            for ti in range(HALF_T):
                normalize_tile(ti)
            ch = HALF_T * 480
            nc.sync.dma_start(out=out_ext[:, 0:ch], in_=sym[:, 0:ch])
            epilogue_half(1)
            for ti in range(HALF_T, NTILE):
                normalize_tile(ti)
            nc.sync.dma_start(out=out_ext[:, ch:OUTW], in_=sym[:, ch:OUTW])

    _fix_sync_waits(nc, spares, relay_sem)
    return nc


# ---------------------------------------------------------------- host driver
def _host_tables(rsv, rev):
    from ml_dtypes import bfloat16
    LN2 = float(np.log(2.0))
    tcode = np.full((128, NMC), -1.0, np.float32)
    for al in range(4):
        for t5 in range(T):
            tcode[al * 32:(al + 1) * 32, al * 5 + t5] = float(ATOM_TYPES[t5])
    sel8 = np.zeros((128, 8), np.float32)
    selbc = np.zeros((8, 128), np.float32)
    for gp in range(4):
        for al in range(4):
            for t5 in range(T):
                row = 32 * gp + 5 * al + t5
                col = 4 * (al % 2) + gp
                sel8[row, col] = 1.0
                selbc[col, row] = 1.0
    biases = np.zeros((128, 98), np.float32)
    for p in range(P):
        biases[:, p] = -float(rsv[p])
        biases[:, 48 + p] = LN2 - float(rev[p]) * float(rsv[p]) ** 2
    biases[:, 96] = float(np.pi / 2.0)
    biases[:, 97] = LN2
    # output unscramble: [128, OUTW] -> [4096, 240]
    a = np.arange(A)
    al = a // AH
    ah = a % AH
    ti = ah // 40
    rem = ah % 40
    gf = rem // 4
    gp = rem % 4
    c = np.arange(C_OUT)
    t5 = c // P
    p = c % P
    rows = (32 * gp[:, None] + 5 * al[:, None] + t5[None, :]).astype(np.int64)
    cols = ((ti * 480 + gf * 48)[:, None] + p[None, :]).astype(np.int64)
    return (tcode.astype(bfloat16), sel8.astype(bfloat16), selbc, biases,
            rows, cols)


def kernel(X, rc, rs, re, Nbrs, Nbrs_Z):
    from ml_dtypes import bfloat16
    X = np.asarray(X, np.float32)
    rc = np.asarray(rc, np.float32).ravel()
    rs = np.asarray(rs, np.float32).ravel()
    re = np.asarray(re, np.float32).ravel()
    Nbrs = np.asarray(Nbrs, np.int32)
    Nbrs_Z = np.asarray(Nbrs_Z, np.int32)

    nc = build_nc(rc, rs, re)
    tcode, sel8, selbc, biases, orows, ocols = _host_tables(rs, re)

    # per-(a,m)-tile layouts: partition p = (a//1024)*32 + m, free = a % 1024
    in_maps = []
    for core in range(NC_CORES):
        bsl = slice(core * B_LOC, (core + 1) * B_LOC)
        Xc = X[bsl].reshape(A, 3)                       # a = b_loc*2048 + n
        Nb = Nbrs[bsl].reshape(A, M)
        Zb = Nbrs_Z[bsl].reshape(A, M)
        gidx = Nb + (np.arange(A)[:, None] // N) * N    # [A, M] global rows
        a_hi = np.arange(A) % AH
        part = ((np.arange(A) // AH)[:, None] * 32
                + np.arange(M)[None]).astype(np.int32)
        zt = np.zeros((128, AH), np.float32)
        zt[part.ravel(), np.repeat(a_hi, M)] = Zb.ravel().astype(np.float32)
        D = Xc[gidx] - Xc[:, None, :]                   # [A, M, 3]
        Rv = np.sqrt(np.sum(D * D, axis=2, dtype=np.float32))
        rr = np.zeros((128, AH), np.float32)
        rr[part.ravel(), np.repeat(a_hi, M)] = Rv.ravel()
        in_maps.append({
            "rr": rr, "zt": zt.astype(bfloat16), "tcode": tcode,
            "sel8": sel8, "selbc": selbc, "biases": biases,
        })

    res = run_bass_kernel_spmd(nc, in_maps, core_ids=list(range(NC_CORES)),
                               trace=_TRACE[0])
    if _TRACE[0]:
        kernel.last_exec_ns = res.exec_time_ns
        kernel.last_profile = res

    out = np.zeros((B, N, C_OUT), np.float32)
    for core in range(NC_CORES):
        o = np.asarray(res.results[core]["out"]).astype(np.float32)
        out[core * B_LOC:(core + 1) * B_LOC] = o[orows, ocols].reshape(
            B_LOC, N, C_OUT)
    return out


# revision 32
# speedup vs baseline: 3.1452x; 1.0072x over previous
"""AtomicConvolution Trainium2 kernel (8 NeuronCores, data-parallel over B).

Pipeline per core (2 complexes, 4096 atoms, layout [par=(a_lo*32+m), free=a_hi]):
  host computes R (gather + norm) -> radial fn on device in two table-set
  batched ACT phases (Square+Exp phase, then Sin phase; half-angle form
  rsf = (sin(pi/2 - theta/2))^2 * 2*exp(..) avoids the slow 3-operand DVE
  op) writing contiguous-per-p rsf (bf16) -> per-atom-group masked type
  reduction on TensorE (block-diagonal 0/1 weights from is_equal) -> PSUM
  -> sym parked in SBUF (bf16, PSUM-native layout; host unscrambles) ->
  per-atom BN stats via selector matmuls + split AllReduce (first half
  overlaps the remaining tile loop) -> normalize in place -> one out DMA.
  Stored rsf = +2*f; BN epilogue uses eps*4 to compensate.
"""
import sys
import types
import numpy as np

ATOM_TYPES = (1, 6, 7, 8, 16)
BN_EPS = 1e-5
B, N, M, P = 16, 2048, 32, 48
T = len(ATOM_TYPES)
NC_CORES = 8
B_LOC = B // NC_CORES            # 2 complexes per core
A = B_LOC * N                    # 4096 atoms per core
AH = A // 4                      # 1024 free columns
C_OUT = P * T                    # 240 channels
NTILE = 26                       # ceil(1024 / 40) psum tiles
OUTW = NTILE * 480               # 12480 staged output columns
DVE_PREP = 34                    # how many p's compute the exp arg on DVE
ACT_SQ = 26                      # how many p's square the sin on ACT
HALF_T = 16                      # collective split point (tiles 0..15 | 16..25)
NMC = 20                         # mask columns per group (al*5+t5)

_TRACE = [False]

# ---------------------------------------------------------------- env patches
import concourse.bass as bass
import concourse.mybir as mybir
import concourse.tile as tile
import concourse.bass_utils as bu
from concourse.bass_utils import run_bass_kernel_spmd
from concourse.tile import TileContext, add_dep_helper


def _patch_tile_tail_drain():
    tile_mod = tile
    ScopedClock = None
    for _n in dir(tile_mod):
        if "ScopedClock" in _n:
            ScopedClock = getattr(tile_mod, _n)

    def _drain(self, tick_clock, wait_clock):
        nc = self.nc
        nops = [nc.sync.nop(nofuse=True) for _ in range(30)]
        drain_inst = nc.sync.drain()
        wait_clock.add_sem_waits(
            drain_inst.ins, ScopedClock({None: tick_clock.global_clock})
        )
        si = drain_inst.ins.sync_info
        if si is not None and si.on_wait and len(si.on_wait) > 1:
            waits = list(si.on_wait)
            si.on_wait = waits[:1]
            rest = waits[1:]
            assert len(rest) <= len(nops)
            for i, nop in enumerate(nops):
                chunk = rest[i:i + 1]
                if not chunk:
                    break
                nsi = nop.ins.sync_info
                if nsi is None:
                    nop.ins.sync_info = mybir.SyncInfo(on_wait=chunk, on_update=[])
                else:
                    nsi.on_wait = chunk
        nc.all_engine_barrier()
        popped = nc._tile_sem_poison_stack.pop()
        assert popped is self._sem_poison
        nc.clear_and_free_semaphores(list(self.sems.allocated().values()))
        nc.all_engine_barrier()

    TileContext._drain_and_barrier = _drain


WAIT_CAP = 1


def _make_spare_nops(nc, counts):
    # SP-engine carrier nops: the only engine whose sequencer NoOp reliably
    # encodes with sem waits in this walrus build.
    return {"carriers": [nc.sync.nop(nofuse=True) for _ in range(4000)]}


def _fix_sync_waits(nc, spares, relay):
    clr = nc.sync.sem_clear(relay)
    relay_count = [0]
    carriers = spares["carriers"]
    spare_names = {c.ins.name for c in carriers}
    # move the freshly-appended clear to the very beginning of the first block
    fn0 = nc.m.functions[0]
    for bb in fn0.blocks:
        if clr.ins in bb.instructions:
            bb.instructions.remove(clr.ins)
    fn0.blocks[0].instructions.insert(0, clr.ins)
    for fn in nc.m.functions:
        for bb in fn.blocks:
            bb.instructions[:] = [
                i for i in bb.instructions if i.name not in spare_names
            ]
    for fn in nc.m.functions:
        for bb in fn.blocks:
            new = []
            for inst in bb.instructions:
                si = inst.sync_info
                waits = list(si.on_wait) if si is not None and si.on_wait else []
                if len(waits) > WAIT_CAP:
                    for w in waits:
                        assert carriers, "out of relay carriers"
                        car = carriers.pop()
                        car.then_inc(relay, 1)
                        car.ins.sync_info.on_wait = [w]
                        relay_count[0] += 1
                        new.append(car.ins)
                    si.on_wait = [mybir.SyncWait(
                        sync_type="semaphore", id=relay.num,
                        ant_name=relay.name, wait_mode="sem-ge-imm",
                        wait_value=relay_count[0], wait_reg=None)]
                new.append(inst)
            bb.instructions[:] = new


def _patch_walrus_dyndma(size=16384):
    if getattr(bu.run_command, "_walrus_patched", False):
        return
    _orig = bu.run_command

    def run2(cmd, cwd=None, **kw):
        try:
            if cmd and "walrus_driver" in str(cmd[0]) and any(
                "codegen" in str(c) for c in cmd
            ):
                cmd = list(cmd) + [
                    f"--dynamic-dma-scratch-size-per-partition={size}"
                ]
        except Exception:
            pass
        return _orig(cmd, cwd=cwd, **kw)

    run2._walrus_patched = True
    bu.run_command = run2


def _install_ntff_hook():
    if "antenv.axon_hooks" in sys.modules:
        return
    try:
        from trn_agent_boot.trn_boot import _ntff_profile_via_ctypes
        hook = _ntff_profile_via_ctypes("/opt/axon/libaxon_pjrt.so")
    except Exception:
        hook = None
    m = types.ModuleType("antenv.axon_hooks")
    m._hook = hook
    m.get_axon_ntff_profile_hook = lambda: m._hook
    m.set_axon_ntff_profile_hook = lambda h: setattr(m, "_hook", h)
    sys.modules["antenv.axon_hooks"] = m
    try:
        import antenv
        antenv.axon_hooks = m
    except Exception:
        pass


_patch_tile_tail_drain()
_patch_walrus_dyndma()
_install_ntff_hook()

DT = mybir.dt


def _mk_ap(base_ap, off_elems, free_dims):
    return bass.AP(base_ap.tensor, base_ap.offset + off_elems,
                   [base_ap.ap[0]] + free_dims)


# ---------------------------------------------------------------- bass build
def build_nc(rcv, rsv, rev):
    nc = bass.Bass(dynamic_dma_scratch_size=8192)
    f32, bf16 = DT.float32, DT.bfloat16

    PIH = float(np.pi / 2.0)
    AL = mybir.AluOpType
    AF = mybir.ActivationFunctionType

    rr_ext = nc.declare_dram_parameter("rr", [128, AH], f32, isOutput=False)
    zt_ext = nc.declare_dram_parameter("zt", [128, AH], bf16, isOutput=False)
    tc_ext = nc.declare_dram_parameter("tcode", [128, NMC], bf16,
                                       isOutput=False)
    s8_ext = nc.declare_dram_parameter("sel8", [128, 8], bf16, isOutput=False)
    sb_ext = nc.declare_dram_parameter("selbc", [8, 128], f32, isOutput=False)
    # bias table: col p -> -rs_p (Square bias), col 48+p -> exp-path bias,
    # col 96 -> +pi/2 (Sin), col 97 -> ln2
    bi_ext = nc.declare_dram_parameter("biases", [128, 98], f32, isOutput=False)
    out_ext = nc.declare_dram_parameter("out", [128, OUTW], bf16, isOutput=True)

    st_w = [2 * HALF_T * 10, 2 * (260 - HALF_T * 10)]
    st_in = [nc.dram_tensor(f"st_in{h}", [8, st_w[h]], f32) for h in range(2)]
    st_out = [nc.dram_tensor(f"st_out{h}", [8, st_w[h]], f32,
                             addr_space="Shared") for h in range(2)]

    relay_sem = nc.semaphore("wait_relay").__enter__()
    with TileContext(nc) as tc:
        spares = _make_spare_nops(nc, {})
        with tc.tile_pool(name="main", bufs=1) as pool, \
             tc.tile_pool(name="work", bufs=2) as wpool, \
             tc.tile_pool(name="uprep", bufs=4) as upool, \
             tc.tile_pool(name="rcap", bufs=6) as rpool, \
             tc.tile_pool(name="psum", bufs=2, space="PSUM") as ppool, \
             tc.tile_pool(name="py", bufs=2, space="PSUM") as ypool, \
             tc.tile_pool(name="pstat", bufs=1, space="PSUM") as spool:

            # ---- loads (rr + biases first: they gate the radial phase)
            rr = pool.tile([128, AH], f32)
            nc.sync.dma_start(out=rr[:], in_=rr_ext[:])
            bia = pool.tile([128, 98], f32)
            nc.sync.dma_start(out=bia[:], in_=bi_ext[:])
            r2 = pool.tile([128, AH], f32)
            nc.vector.tensor_tensor(out=r2[:], in0=rr[:], in1=rr[:],
                                    op=AL.mult)
            zt = pool.tile([128, AH], bf16)
            nc.sync.dma_start(out=zt[:], in_=zt_ext[:])
            tcode = pool.tile([128, NMC], bf16)
            nc.sync.dma_start(out=tcode[:], in_=tc_ext[:])
            sel8 = pool.tile([128, 8], bf16)
            nc.sync.dma_start(out=sel8[:], in_=s8_ext[:])
            selbc = pool.tile([128, 128], f32)
            nc.sync.dma_start(out=selbc[0:8, :], in_=sb_ext[:])

            # rsf layout: contiguous per p -> col = p*AH + a_hi
            rsf = pool.tile([128, P * AH], bf16)

            # ---- phase A: kk'_p = 2*exp(-re*(R-rs)^2), bf16.
            # ACT-path p's (Square+Exp, one table set) run first so ACT
            # starts immediately; DVE_PREP p's build the exp arg on DVE
            # meanwhile (deeper uprep pool so DVE runs ahead).
            last_exp = None
            keys = []
            for p in range(P):
                if p < DVE_PREP:
                    keys.append(((p + 0.5) / DVE_PREP, 1, p))
                else:
                    keys.append(((p - DVE_PREP + 0.5) / (P - DVE_PREP), 0, p))
            p_order = [p for _, _, p in sorted(keys)]
            for p in p_order:
                re_p, rs_p = float(rev[p]), float(rsv[p])
                if p < DVE_PREP:
                    t1 = wpool.tile([128, AH], f32, tag="t1")
                    nc.vector.tensor_scalar(
                        out=t1[:], in0=rr[:], scalar1=-2.0 * rs_p,
                        scalar2=None, op0=AL.mult)
                    u = upool.tile([128, AH], f32, tag="u")
                    nc.vector.tensor_tensor(out=u[:], in0=t1[:], in1=r2[:],
                                            op=AL.add)
                    # exp(-re*u + (ln2 - re*rs^2)) = 2*exp(-re*(R-rs)^2)
                    ei = nc.scalar.activation(
                        out=rsf[:, p * AH:(p + 1) * AH], in_=u[:],
                        func=AF.Exp, scale=-re_p,
                        bias=bia[:, 48 + p:49 + p])
                else:
                    ua = wpool.tile([128, AH], f32, tag="ua")
                    nc.scalar.activation(out=ua[:], in_=rr[:], func=AF.Square,
                                         bias=bia[:, p:p + 1])
                    ei = nc.scalar.activation(
                        out=rsf[:, p * AH:(p + 1) * AH], in_=ua[:],
                        func=AF.Exp, scale=-re_p,
                        bias=bia[:, 97:98])
                last_exp = ei

            # ---- phase B: s = sin(pi/2 - pi*min(R,rc)/(2rc)) (>=0, LUT-safe)
            # rsf *= s*s  ->  rsf = 2*f_p.  Sins forced after all Exps so the
            # ACT table set switches exactly once; rcap pool is deep so DVE
            # computes sin args well ahead.
            for p in range(P):
                rc_p = float(rcv[p])
                rt = rpool.tile([128, AH], bf16, tag="rt")
                nc.vector.tensor_scalar(
                    out=rt[:], in0=rr[:], scalar1=rc_p,
                    scalar2=float(np.pi / (2.0 * rc_p)),
                    op0=AL.min, op1=AL.mult)
                cs = wpool.tile([128, AH], bf16, tag="cs")
                si = nc.scalar.activation(out=cs[:], in_=rt[:], func=AF.Sin,
                                          scale=-1.0, bias=bia[:, 96:97])
                add_dep_helper(si.ins, last_exp.ins,
                               reason="keep Sin phase after Exp phase")
                s2t = wpool.tile([128, AH], bf16, tag="s2t")
                if p < ACT_SQ:
                    # Square is in every table set -> no reload
                    nc.scalar.activation(out=s2t[:], in_=cs[:],
                                         func=AF.Square)
                else:
                    nc.vector.tensor_tensor(out=s2t[:], in0=cs[:], in1=cs[:],
                                            op=AL.mult)
                psl = rsf[:, p * AH:(p + 1) * AH]
                nc.vector.tensor_tensor(out=psl, in0=s2t[:], in1=psl,
                                        op=AL.mult)

            # ---- TensorE masked reduction; sym parked in SBUF (bf16)
            # group g = one a_hi; psum tile: rows 32*gp + (al*5+t5),
            # cols gf*48 + p, for g = ti*40 + gf*4 + gp
            sym = pool.tile([128, OUTW], bf16)
            rst = pool.tile([128, 260], f32)   # rows 0:8 = s1, 32:40 = s2
            mbc = pool.tile([128, 260], f32)
            ibc = pool.tile([128, 260], f32)
            sall = [None, None]
            GRP_T = 40

            def stats_half(h):
                # per-n stats already folded per tile; just ship + AllReduce
                c0, c1 = (0, HALF_T * 10) if h == 0 else (HALF_T * 10, 260)
                w = c1 - c0
                nc.sync.dma_start(out=st_in[h][:, 0:w], in_=rst[0:8, c0:c1])
                nc.sync.dma_start(out=st_in[h][:, w:2 * w],
                                  in_=rst[32:40, c0:c1])
                nc.gpsimd.collective_compute(
                    "AllReduce", AL.add,
                    ins=[st_in[h][:]], outs=[st_out[h][:]],
                    replica_groups=[list(range(NC_CORES))])
                sa = pool.tile([128, 2 * w], f32)
                nc.sync.dma_start(out=sa[0:8, :], in_=st_out[h][:])
                sall[h] = sa

            def epilogue_half(h):
                # mean/inv on [8,w], broadcast to 128 rows via matmul
                c0, c1 = (0, HALF_T * 10) if h == 0 else (HALF_T * 10, 260)
                w = c1 - c0
                sa = sall[h]
                inv_n = 1.0 / (B * C_OUT)
                mean = wpool.tile([128, 200], f32, tag="mean")
                nc.vector.tensor_scalar(out=mean[0:8, :w], in0=sa[0:8, 0:w],
                                        scalar1=inv_n, scalar2=None,
                                        op0=AL.mult)
                vpe = wpool.tile([128, 200], f32, tag="vpe")
                nc.vector.tensor_scalar(out=vpe[0:8, :w], in0=sa[0:8, w:2 * w],
                                        scalar1=inv_n, scalar2=None,
                                        op0=AL.mult)
                msq = wpool.tile([128, 200], f32, tag="msq")
                nc.vector.tensor_tensor(out=msq[0:8, :w], in0=mean[0:8, :w],
                                        in1=mean[0:8, :w], op=AL.mult)
                nc.vector.tensor_tensor(out=vpe[0:8, :w], in0=vpe[0:8, :w],
                                        in1=msq[0:8, :w], op=AL.subtract)
                nc.vector.tensor_scalar(out=vpe[0:8, :w], in0=vpe[0:8, :w],
                                        scalar1=float(4.0 * BN_EPS),
                                        scalar2=None, op0=AL.add)
                sdev = wpool.tile([128, 200], f32, tag="sdev")
                nc.scalar.activation(out=sdev[0:8, :w], in_=vpe[0:8, :w],
                                     func=AF.Sqrt)
                inv = wpool.tile([128, 200], f32, tag="inv")
                nc.vector.reciprocal(out=inv[0:8, :w], in_=sdev[0:8, :w])
                mbp = spool.tile([128, 200], f32, tag="mbp")
                ibp = spool.tile([128, 200], f32, tag="ibp")
                nc.tensor.matmul(out=mbp[:, :w], lhsT=selbc[0:8, :],
                                 rhs=mean[0:8, :w], start=True, stop=True)
                nc.tensor.matmul(out=ibp[:, :w], lhsT=selbc[0:8, :],
                                 rhs=inv[0:8, :w], start=True, stop=True)
                nc.vector.tensor_copy(out=mbc[:, c0:c1], in_=mbp[:, :w])
                nc.vector.tensor_copy(out=ibc[:, c0:c1], in_=ibp[:, :w])

            def normalize_tile(ti):
                nfgf = 10 if ti < 25 else 6
                ncol = nfgf * 48
                ssl = sym[:, ti * 480:ti * 480 + ncol]
                t1 = wpool.tile([128, 480], f32, tag="nt")
                nc.vector.tensor_tensor(
                    out=t1[:, :ncol], in0=ssl,
                    in1=_mk_ap(mbc[:], ti * 10, [[1, nfgf], [0, 48]]),
                    op=AL.subtract)
                nc.vector.tensor_tensor(
                    out=ssl, in0=t1[:, :ncol],
                    in1=_mk_ap(ibc[:], ti * 10, [[1, nfgf], [0, 48]]),
                    op=AL.mult)

            for ti in range(NTILE):
                ngrp = GRP_T if ti < 25 else 24
                nfgf = 10 if ti < 25 else 6
                ncol = nfgf * 48
                wmask = wpool.tile([128, GRP_T * NMC], bf16, tag="wmask")
                in0 = _mk_ap(zt[:], ti * GRP_T, [[1, ngrp], [0, NMC]])
                t0 = _mk_ap(tcode[:], 0, [[0, ngrp], [1, NMC]])
                nc.vector.tensor_tensor(
                    out=wmask[:, :ngrp * NMC], in0=in0, in1=t0, op=AL.is_equal)
                stp = ppool.tile([128, 480], f32, tag="stp")
                if ti < 2:
                    # 20-col masks leave psum rows 20..31 of each strip
                    # unwritten; clear once so parked garbage is finite
                    nc.vector.memset(stp[:], 0.0)
                for gi in range(ngrp):
                    g = ti * GRP_T + gi
                    gp, gf = gi % 4, gi // 4
                    rhs = _mk_ap(rsf[:], g, [[AH, P]])
                    nc.tensor.matmul(
                        out=stp[32 * gp:32 * gp + 20, gf * 48:(gf + 1) * 48],
                        lhsT=wmask[:, gi * NMC:(gi + 1) * NMC],
                        rhs=rhs, start=True, stop=True,
                        tile_position=(0, 32 * gp))
                # park + square on idle ACT; fold rows via sel8 matmul, then
                # tiny per-gf reduces on the [8, ncol] results
                ssl = sym[:, ti * 480:ti * 480 + ncol]
                nc.scalar.activation(out=ssl, in_=stp[:, :ncol], func=AF.Copy)
                sqt = wpool.tile([128, 480], bf16, tag="sqt")
                nc.scalar.activation(out=sqt[:, :ncol], in_=stp[:, :ncol],
                                     func=AF.Square)
                y1 = ypool.tile([128, 480], f32, tag="y1")
                if ti < 2:
                    nc.vector.memset(y1[:], 0.0)
                nc.tensor.matmul(out=y1[0:8, :ncol], lhsT=sel8[:, 0:8],
                                 rhs=ssl, start=True, stop=True)
                nc.tensor.matmul(out=y1[32:40, :ncol], lhsT=sel8[:, 0:8],
                                 rhs=sqt[:, :ncol], start=True, stop=True,
                                 tile_position=(0, 32))
                nc.vector.tensor_reduce(
                    out=rst[0:40, ti * 10:ti * 10 + nfgf],
                    in_=_mk_ap(y1[0:40, :], 0, [[48, nfgf], [1, 48]]),
                    axis=mybir.AxisListType.X, op=AL.add)
                if ti == HALF_T - 1:
                    stats_half(0)
                    epilogue_half(0)
            stats_half(1)
            for ti in range(HALF_T):
                normalize_tile(ti)
            ch = HALF_T * 480
            nc.sync.dma_start(out=out_ext[:, 0:ch], in_=sym[:, 0:ch])
            epilogue_half(1)
            for ti in range(HALF_T, NTILE):
                normalize_tile(ti)
            nc.sync.dma_start(out=out_ext[:, ch:OUTW], in_=sym[:, ch:OUTW])

    _fix_sync_waits(nc, spares, relay_sem)
    return nc


# ---------------------------------------------------------------- host driver
def _host_tables(rsv, rev):
    from ml_dtypes import bfloat16
    LN2 = float(np.log(2.0))
    tcode = np.full((128, NMC), -1.0, np.float32)
    for al in range(4):
        for t5 in range(T):
            tcode[al * 32:(al + 1) * 32, al * 5 + t5] = float(ATOM_TYPES[t5])
    sel8 = np.zeros((128, 8), np.float32)
    selbc = np.zeros((8, 128), np.float32)
    for gp in range(4):
        for al in range(4):
            for t5 in range(T):
                row = 32 * gp + 5 * al + t5
                col = 4 * (al % 2) + gp
                sel8[row, col] = 1.0
                selbc[col, row] = 1.0
    biases = np.zeros((128, 98), np.float32)
    for p in range(P):
        biases[:, p] = -float(rsv[p])
        biases[:, 48 + p] = LN2 - float(rev[p]) * float(rsv[p]) ** 2
    biases[:, 96] = float(np.pi / 2.0)
    biases[:, 97] = LN2
    # output unscramble: [128, OUTW] -> [4096, 240]
    a = np.arange(A)
    al = a // AH
    ah = a % AH
    ti = ah // 40
    rem = ah % 40
    gf = rem // 4
    gp = rem % 4
    c = np.arange(C_OUT)
    t5 = c // P
    p = c % P
    rows = (32 * gp[:, None] + 5 * al[:, None] + t5[None, :]).astype(np.int64)
    cols = ((ti * 480 + gf * 48)[:, None] + p[None, :]).astype(np.int64)
    return (tcode.astype(bfloat16), sel8.astype(bfloat16), selbc, biases,
            rows, cols)


def kernel(X, rc, rs, re, Nbrs, Nbrs_Z):
    from ml_dtypes import bfloat16
    X = np.asarray(X, np.float32)
    rc = np.asarray(rc, np.float32).ravel()
    rs = np.asarray(rs, np.float32).ravel()
    re = np.asarray(re, np.float32).ravel()
    Nbrs = np.asarray(Nbrs, np.int32)
    Nbrs_Z = np.asarray(Nbrs_Z, np.int32)

    nc = build_nc(rc, rs, re)
    tcode, sel8, selbc, biases, orows, ocols = _host_tables(rs, re)

    # per-(a,m)-tile layouts: partition p = (a//1024)*32 + m, free = a % 1024
    in_maps = []
    for core in range(NC_CORES):
        bsl = slice(core * B_LOC, (core + 1) * B_LOC)
        Xc = X[bsl].reshape(A, 3)                       # a = b_loc*2048 + n
        Nb = Nbrs[bsl].reshape(A, M)
        Zb = Nbrs_Z[bsl].reshape(A, M)
        gidx = Nb + (np.arange(A)[:, None] // N) * N    # [A, M] global rows
        a_hi = np.arange(A) % AH
        part = ((np.arange(A) // AH)[:, None] * 32
                + np.arange(M)[None]).astype(np.int32)
        zt = np.zeros((128, AH), np.float32)
        zt[part.ravel(), np.repeat(a_hi, M)] = Zb.ravel().astype(np.float32)
        D = Xc[gidx] - Xc[:, None, :]                   # [A, M, 3]
        Rv = np.sqrt(np.sum(D * D, axis=2, dtype=np.float32))
        rr = np.zeros((128, AH), np.float32)
        rr[part.ravel(), np.repeat(a_hi, M)] = Rv.ravel()
        in_maps.append({
            "rr": rr, "zt": zt.astype(bfloat16), "tcode": tcode,
            "sel8": sel8, "selbc": selbc, "biases": biases,
        })

    res = run_bass_kernel_spmd(nc, in_maps, core_ids=list(range(NC_CORES)),
                               trace=_TRACE[0])
    if _TRACE[0]:
        kernel.last_exec_ns = res.exec_time_ns
        kernel.last_profile = res

    out = np.zeros((B, N, C_OUT), np.float32)
    for core in range(NC_CORES):
        o = np.asarray(res.results[core]["out"]).astype(np.float32)
        out[core * B_LOC:(core + 1) * B_LOC] = o[orows, ocols].reshape(
            B_LOC, N, C_OUT)
    return out


# revision 35
# speedup vs baseline: 3.3305x; 1.0589x over previous
"""AtomicConvolution Trainium2 kernel (8 NeuronCores, data-parallel over B).

Pipeline per core (2 complexes, 4096 atoms, layout [par=(a_lo*32+m), free=a_hi]):
  host computes R (gather + norm) -> radial fn on device in two table-set
  batched ACT phases (Square+Exp phase, then Sin phase; half-angle form
  rsf = (sin(pi/2 - theta/2))^2 * 2*exp(..) avoids the slow 3-operand DVE
  op) writing contiguous-per-p rsf (bf16) -> per-atom-group masked type
  reduction on TensorE (block-diagonal 0/1 weights from is_equal) -> PSUM
  -> sym parked in SBUF (bf16, PSUM-native layout; host unscrambles) ->
  per-atom BN stats via selector matmuls + split AllReduce (first half
  overlaps the remaining tile loop) -> normalize in place -> one out DMA.
  Stored rsf = +2*f; BN epilogue uses eps*4 to compensate.
"""
import sys
import types
import numpy as np

ATOM_TYPES = (1, 6, 7, 8, 16)
BN_EPS = 1e-5
B, N, M, P = 16, 2048, 32, 48
T = len(ATOM_TYPES)
NC_CORES = 8
B_LOC = B // NC_CORES            # 2 complexes per core
A = B_LOC * N                    # 4096 atoms per core
AH = A // 4                      # 1024 free columns
C_OUT = P * T                    # 240 channels
NTILE = 26                       # ceil(1024 / 40) psum tiles
OUTW = NTILE * 480               # 12480 staged output columns
DVE_PREP = 34                    # how many p's compute the exp arg on DVE
ACT_SQ = 26                      # how many p's square the sin on ACT
HALF_T = 16                      # collective split point (tiles 0..15 | 16..25)
NMC = 20                         # mask columns per group (al*5+t5)

_TRACE = [False]

# ---------------------------------------------------------------- env patches
import concourse.bass as bass
import concourse.mybir as mybir
import concourse.tile as tile
import concourse.bass_utils as bu
from concourse.bass_utils import run_bass_kernel_spmd
from concourse.tile import TileContext, add_dep_helper


def _patch_tile_tail_drain():
    tile_mod = tile
    ScopedClock = None
    for _n in dir(tile_mod):
        if "ScopedClock" in _n:
            ScopedClock = getattr(tile_mod, _n)

    def _drain(self, tick_clock, wait_clock):
        nc = self.nc
        nops = [nc.sync.nop(nofuse=True) for _ in range(30)]
        drain_inst = nc.sync.drain()
        wait_clock.add_sem_waits(
            drain_inst.ins, ScopedClock({None: tick_clock.global_clock})
        )
        si = drain_inst.ins.sync_info
        if si is not None and si.on_wait and len(si.on_wait) > 1:
            waits = list(si.on_wait)
            si.on_wait = waits[:1]
            rest = waits[1:]
            assert len(rest) <= len(nops)
            for i, nop in enumerate(nops):
                chunk = rest[i:i + 1]
                if not chunk:
                    break
                nsi = nop.ins.sync_info
                if nsi is None:
                    nop.ins.sync_info = mybir.SyncInfo(on_wait=chunk, on_update=[])
                else:
                    nsi.on_wait = chunk
        nc.all_engine_barrier()
        popped = nc._tile_sem_poison_stack.pop()
        assert popped is self._sem_poison
        nc.clear_and_free_semaphores(list(self.sems.allocated().values()))
        nc.all_engine_barrier()

    TileContext._drain_and_barrier = _drain


WAIT_CAP = 1


def _make_spare_nops(nc, counts):
    # SP-engine carrier nops: the only engine whose sequencer NoOp reliably
    # encodes with sem waits in this walrus build.
    return {"carriers": [nc.sync.nop(nofuse=True) for _ in range(4000)]}


def _fix_sync_waits(nc, spares, relay):
    clr = nc.sync.sem_clear(relay)
    relay_count = [0]
    carriers = spares["carriers"]
    spare_names = {c.ins.name for c in carriers}
    # move the freshly-appended clear to the very beginning of the first block
    fn0 = nc.m.functions[0]
    for bb in fn0.blocks:
        if clr.ins in bb.instructions:
            bb.instructions.remove(clr.ins)
    fn0.blocks[0].instructions.insert(0, clr.ins)
    for fn in nc.m.functions:
        for bb in fn.blocks:
            bb.instructions[:] = [
                i for i in bb.instructions if i.name not in spare_names
            ]
    for fn in nc.m.functions:
        for bb in fn.blocks:
            new = []
            for inst in bb.instructions:
                si = inst.sync_info
                waits = list(si.on_wait) if si is not None and si.on_wait else []
                if len(waits) > WAIT_CAP:
                    for w in waits:
                        assert carriers, "out of relay carriers"
                        car = carriers.pop()
                        car.then_inc(relay, 1)
                        car.ins.sync_info.on_wait = [w]
                        relay_count[0] += 1
                        new.append(car.ins)
                    si.on_wait = [mybir.SyncWait(
                        sync_type="semaphore", id=relay.num,
                        ant_name=relay.name, wait_mode="sem-ge-imm",
                        wait_value=relay_count[0], wait_reg=None)]
                new.append(inst)
            bb.instructions[:] = new


def _patch_walrus_dyndma(size=16384):
    if getattr(bu.run_command, "_walrus_patched", False):
        return
    _orig = bu.run_command

    def run2(cmd, cwd=None, **kw):
        try:
            if cmd and "walrus_driver" in str(cmd[0]) and any(
                "codegen" in str(c) for c in cmd
            ):
                cmd = list(cmd) + [
                    f"--dynamic-dma-scratch-size-per-partition={size}"
                ]
        except Exception:
            pass
        return _orig(cmd, cwd=cwd, **kw)

    run2._walrus_patched = True
    bu.run_command = run2


def _install_ntff_hook():
    if "antenv.axon_hooks" in sys.modules:
        return
    try:
        from trn_agent_boot.trn_boot import _ntff_profile_via_ctypes
        hook = _ntff_profile_via_ctypes("/opt/axon/libaxon_pjrt.so")
    except Exception:
        hook = None
    m = types.ModuleType("antenv.axon_hooks")
    m._hook = hook
    m.get_axon_ntff_profile_hook = lambda: m._hook
    m.set_axon_ntff_profile_hook = lambda h: setattr(m, "_hook", h)
    sys.modules["antenv.axon_hooks"] = m
    try:
        import antenv
        antenv.axon_hooks = m
    except Exception:
        pass


_patch_tile_tail_drain()
_patch_walrus_dyndma()
_install_ntff_hook()

DT = mybir.dt


def _mk_ap(base_ap, off_elems, free_dims):
    return bass.AP(base_ap.tensor, base_ap.offset + off_elems,
                   [base_ap.ap[0]] + free_dims)


# ---------------------------------------------------------------- bass build
def build_nc(rcv, rsv, rev):
    nc = bass.Bass(dynamic_dma_scratch_size=8192)
    f32, bf16 = DT.float32, DT.bfloat16

    PIH = float(np.pi / 2.0)
    AL = mybir.AluOpType
    AF = mybir.ActivationFunctionType

    rr_ext = nc.declare_dram_parameter("rr", [128, AH], f32, isOutput=False)
    zt_ext = nc.declare_dram_parameter("zt", [128, AH], bf16, isOutput=False)
    tc_ext = nc.declare_dram_parameter("tcode", [128, NMC], bf16,
                                       isOutput=False)
    s8_ext = nc.declare_dram_parameter("sel8", [128, 8], bf16, isOutput=False)
    sb_ext = nc.declare_dram_parameter("selbc", [8, 128], f32, isOutput=False)
    # bias table: col p -> -rs_p (Square bias), col 48+p -> exp-path bias,
    # col 96 -> +pi/2 (Sin), col 97 -> ln2
    bi_ext = nc.declare_dram_parameter("biases", [128, 98], f32, isOutput=False)
    out_ext = nc.declare_dram_parameter("out", [128, OUTW], bf16, isOutput=True)

    st_w = [2 * HALF_T * 10, 2 * (260 - HALF_T * 10)]
    st_in = [nc.dram_tensor(f"st_in{h}", [8, st_w[h]], f32) for h in range(2)]
    st_out = [nc.dram_tensor(f"st_out{h}", [8, st_w[h]], f32,
                             addr_space="Shared") for h in range(2)]

    relay_sem = nc.semaphore("wait_relay").__enter__()
    with TileContext(nc) as tc:
        spares = _make_spare_nops(nc, {})
        with tc.tile_pool(name="main", bufs=1) as pool, \
             tc.tile_pool(name="work", bufs=2) as wpool, \
             tc.tile_pool(name="uprep", bufs=4) as upool, \
             tc.tile_pool(name="rcap", bufs=6) as rpool, \
             tc.tile_pool(name="wm", bufs=3) as mpool, \
             tc.tile_pool(name="psum", bufs=3, space="PSUM") as ppool, \
             tc.tile_pool(name="py", bufs=2, space="PSUM") as ypool, \
             tc.tile_pool(name="pstat", bufs=1, space="PSUM") as spool:

            # ---- loads (rr + biases first: they gate the radial phase)
            rr = pool.tile([128, AH], f32)
            nc.sync.dma_start(out=rr[:], in_=rr_ext[:])
            bia = pool.tile([128, 98], f32)
            nc.sync.dma_start(out=bia[:], in_=bi_ext[:])
            r2 = pool.tile([128, AH], f32)
            nc.vector.tensor_tensor(out=r2[:], in0=rr[:], in1=rr[:],
                                    op=AL.mult)
            zt = pool.tile([128, AH], bf16)
            nc.sync.dma_start(out=zt[:], in_=zt_ext[:])
            tcode = pool.tile([128, NMC], bf16)
            nc.sync.dma_start(out=tcode[:], in_=tc_ext[:])
            sel8 = pool.tile([128, 8], bf16)
            nc.sync.dma_start(out=sel8[:], in_=s8_ext[:])
            selbc = pool.tile([128, 128], f32)
            nc.sync.dma_start(out=selbc[0:8, :], in_=sb_ext[:])

            # rsf layout: contiguous per p -> col = p*AH + a_hi
            rsf = pool.tile([128, P * AH], bf16)

            # ---- phase A: kk'_p = 2*exp(-re*(R-rs)^2), bf16.
            # ACT-path p's (Square+Exp, one table set) run first so ACT
            # starts immediately; DVE_PREP p's build the exp arg on DVE
            # meanwhile (deeper uprep pool so DVE runs ahead).
            last_exp = None
            keys = []
            for p in range(P):
                if p < DVE_PREP:
                    keys.append(((p + 0.5) / DVE_PREP, 1, p))
                else:
                    keys.append(((p - DVE_PREP + 0.5) / (P - DVE_PREP), 0, p))
            p_order = [p for _, _, p in sorted(keys)]
            for p in p_order:
                re_p, rs_p = float(rev[p]), float(rsv[p])
                if p < DVE_PREP:
                    t1 = wpool.tile([128, AH], f32, tag="t1")
                    nc.vector.tensor_scalar(
                        out=t1[:], in0=rr[:], scalar1=-2.0 * rs_p,
                        scalar2=None, op0=AL.mult)
                    u = upool.tile([128, AH], f32, tag="u")
                    nc.vector.tensor_tensor(out=u[:], in0=t1[:], in1=r2[:],
                                            op=AL.add)
                    # exp(-re*u + (ln2 - re*rs^2)) = 2*exp(-re*(R-rs)^2)
                    ei = nc.scalar.activation(
                        out=rsf[:, p * AH:(p + 1) * AH], in_=u[:],
                        func=AF.Exp, scale=-re_p,
                        bias=bia[:, 48 + p:49 + p])
                else:
                    ua = wpool.tile([128, AH], f32, tag="ua")
                    nc.scalar.activation(out=ua[:], in_=rr[:], func=AF.Square,
                                         bias=bia[:, p:p + 1])
                    ei = nc.scalar.activation(
                        out=rsf[:, p * AH:(p + 1) * AH], in_=ua[:],
                        func=AF.Exp, scale=-re_p,
                        bias=bia[:, 97:98])
                last_exp = ei

            # ---- phase B: s = sin(pi/2 - pi*min(R,rc)/(2rc)) (>=0, LUT-safe)
            # rsf *= s*s  ->  rsf = 2*f_p.  Sins forced after all Exps so the
            # ACT table set switches exactly once; rcap pool is deep so DVE
            # computes sin args well ahead.
            for p in range(P):
                rc_p = float(rcv[p])
                rt = rpool.tile([128, AH], bf16, tag="rt")
                nc.vector.tensor_scalar(
                    out=rt[:], in0=rr[:], scalar1=rc_p,
                    scalar2=float(np.pi / (2.0 * rc_p)),
                    op0=AL.min, op1=AL.mult)
                cs = wpool.tile([128, AH], bf16, tag="cs")
                si = nc.scalar.activation(out=cs[:], in_=rt[:], func=AF.Sin,
                                          scale=-1.0, bias=bia[:, 96:97])
                add_dep_helper(si.ins, last_exp.ins,
                               reason="keep Sin phase after Exp phase")
                s2t = wpool.tile([128, AH], bf16, tag="s2t")
                if p < ACT_SQ:
                    # Square is in every table set -> no reload
                    nc.scalar.activation(out=s2t[:], in_=cs[:],
                                         func=AF.Square)
                else:
                    nc.vector.tensor_tensor(out=s2t[:], in0=cs[:], in1=cs[:],
                                            op=AL.mult)
                psl = rsf[:, p * AH:(p + 1) * AH]
                nc.vector.tensor_tensor(out=psl, in0=s2t[:], in1=psl,
                                        op=AL.mult)

            # ---- TensorE masked reduction; sym parked in SBUF (bf16)
            # group g = one a_hi; psum tile: rows 32*gp + (al*5+t5),
            # cols gf*48 + p, for g = ti*40 + gf*4 + gp
            sym = pool.tile([128, OUTW], bf16)
            rst = pool.tile([128, 260], f32)   # rows 0:8 = s1, 32:40 = s2
            mbc = pool.tile([128, 260], f32)
            ibc = pool.tile([128, 260], f32)
            sall = [None, None]
            GRP_T = 40

            def stats_half(h):
                # per-n stats already folded per tile; just ship + AllReduce
                c0, c1 = (0, HALF_T * 10) if h == 0 else (HALF_T * 10, 260)
                w = c1 - c0
                nc.sync.dma_start(out=st_in[h][:, 0:w], in_=rst[0:8, c0:c1])
                nc.sync.dma_start(out=st_in[h][:, w:2 * w],
                                  in_=rst[32:40, c0:c1])
                nc.gpsimd.collective_compute(
                    "AllReduce", AL.add,
                    ins=[st_in[h][:]], outs=[st_out[h][:]],
                    replica_groups=[list(range(NC_CORES))])
                sa = pool.tile([128, 2 * w], f32)
                nc.sync.dma_start(out=sa[0:8, :], in_=st_out[h][:])
                sall[h] = sa

            def epilogue_half(h):
                # mean/inv on [8,w], broadcast to 128 rows via matmul
                c0, c1 = (0, HALF_T * 10) if h == 0 else (HALF_T * 10, 260)
                w = c1 - c0
                sa = sall[h]
                inv_n = 1.0 / (B * C_OUT)
                mean = wpool.tile([128, 200], f32, tag="mean")
                nc.vector.tensor_scalar(out=mean[0:8, :w], in0=sa[0:8, 0:w],
                                        scalar1=inv_n, scalar2=None,
                                        op0=AL.mult)
                vpe = wpool.tile([128, 200], f32, tag="vpe")
                nc.vector.tensor_scalar(out=vpe[0:8, :w], in0=sa[0:8, w:2 * w],
                                        scalar1=inv_n, scalar2=None,
                                        op0=AL.mult)
                msq = wpool.tile([128, 200], f32, tag="msq")
                nc.vector.tensor_tensor(out=msq[0:8, :w], in0=mean[0:8, :w],
                                        in1=mean[0:8, :w], op=AL.mult)
                nc.vector.tensor_tensor(out=vpe[0:8, :w], in0=vpe[0:8, :w],
                                        in1=msq[0:8, :w], op=AL.subtract)
                nc.vector.tensor_scalar(out=vpe[0:8, :w], in0=vpe[0:8, :w],
                                        scalar1=float(4.0 * BN_EPS),
                                        scalar2=None, op0=AL.add)
                sdev = wpool.tile([128, 200], f32, tag="sdev")
                nc.scalar.activation(out=sdev[0:8, :w], in_=vpe[0:8, :w],
                                     func=AF.Sqrt)
                inv = wpool.tile([128, 200], f32, tag="inv")
                nc.vector.reciprocal(out=inv[0:8, :w], in_=sdev[0:8, :w])
                mbp = spool.tile([128, 200], f32, tag="mbp")
                ibp = spool.tile([128, 200], f32, tag="ibp")
                nc.tensor.matmul(out=mbp[:, :w], lhsT=selbc[0:8, :],
                                 rhs=mean[0:8, :w], start=True, stop=True)
                nc.tensor.matmul(out=ibp[:, :w], lhsT=selbc[0:8, :],
                                 rhs=inv[0:8, :w], start=True, stop=True)
                nc.vector.tensor_copy(out=mbc[:, c0:c1], in_=mbp[:, :w])
                nc.vector.tensor_copy(out=ibc[:, c0:c1], in_=ibp[:, :w])

            def normalize_tile(ti):
                nfgf = 10 if ti < 25 else 6
                ncol = nfgf * 48
                ssl = sym[:, ti * 480:ti * 480 + ncol]
                t1 = wpool.tile([128, 480], f32, tag="nt")
                nc.vector.tensor_tensor(
                    out=t1[:, :ncol], in0=ssl,
                    in1=_mk_ap(mbc[:], ti * 10, [[1, nfgf], [0, 48]]),
                    op=AL.subtract)
                nc.vector.tensor_tensor(
                    out=ssl, in0=t1[:, :ncol],
                    in1=_mk_ap(ibc[:], ti * 10, [[1, nfgf], [0, 48]]),
                    op=AL.mult)

            for ti in range(NTILE):
                ngrp = GRP_T if ti < 25 else 24
                nfgf = 10 if ti < 25 else 6
                ncol = nfgf * 48
                wmask = mpool.tile([128, GRP_T * NMC], bf16, tag="wmask")
                in0 = _mk_ap(zt[:], ti * GRP_T, [[1, ngrp], [0, NMC]])
                t0 = _mk_ap(tcode[:], 0, [[0, ngrp], [1, NMC]])
                nc.vector.tensor_tensor(
                    out=wmask[:, :ngrp * NMC], in0=in0, in1=t0, op=AL.is_equal)
                stp = ppool.tile([128, 480], f32, tag="stp")
                if ti < 3:
                    # 20-col masks leave psum rows 20..31 of each strip
                    # unwritten; clear each rotating buffer once so parked
                    # garbage is finite
                    nc.vector.memset(stp[:], 0.0)
                for gi in range(ngrp):
                    g = ti * GRP_T + gi
                    gp, gf = gi % 4, gi // 4
                    rhs = _mk_ap(rsf[:], g, [[AH, P]])
                    nc.tensor.matmul(
                        out=stp[32 * gp:32 * gp + 20, gf * 48:(gf + 1) * 48],
                        lhsT=wmask[:, gi * NMC:(gi + 1) * NMC],
                        rhs=rhs, start=True, stop=True,
                        tile_position=(0, 32 * gp))
                # park + square on idle ACT; fold rows via sel8 matmul, then
                # tiny per-gf reduces on the [8, ncol] results
                ssl = sym[:, ti * 480:ti * 480 + ncol]
                nc.scalar.activation(out=ssl, in_=stp[:, :ncol], func=AF.Copy)
                sqt = wpool.tile([128, 480], bf16, tag="sqt")
                nc.scalar.activation(out=sqt[:, :ncol], in_=stp[:, :ncol],
                                     func=AF.Square)
                y1 = ypool.tile([128, 480], f32, tag="y1")
                if ti < 2:
                    nc.vector.memset(y1[:], 0.0)
                nc.tensor.matmul(out=y1[0:8, :ncol], lhsT=sel8[:, 0:8],
                                 rhs=ssl, start=True, stop=True)
                nc.tensor.matmul(out=y1[32:40, :ncol], lhsT=sel8[:, 0:8],
                                 rhs=sqt[:, :ncol], start=True, stop=True,
                                 tile_position=(0, 32))
                nc.vector.tensor_reduce(
                    out=rst[0:40, ti * 10:ti * 10 + nfgf],
                    in_=_mk_ap(y1[0:40, :], 0, [[48, nfgf], [1, 48]]),
                    axis=mybir.AxisListType.X, op=AL.add)
                if ti == HALF_T - 1:
                    stats_half(0)
                    epilogue_half(0)
            stats_half(1)
            for ti in range(HALF_T):
                normalize_tile(ti)
            ch = HALF_T * 480
            nc.sync.dma_start(out=out_ext[:, 0:ch], in_=sym[:, 0:ch])
            epilogue_half(1)
            for ti in range(HALF_T, NTILE):
                normalize_tile(ti)
            nc.sync.dma_start(out=out_ext[:, ch:OUTW], in_=sym[:, ch:OUTW])

    _fix_sync_waits(nc, spares, relay_sem)
    return nc


# ---------------------------------------------------------------- host driver
def _host_tables(rsv, rev):
    from ml_dtypes import bfloat16
    LN2 = float(np.log(2.0))
    tcode = np.full((128, NMC), -1.0, np.float32)
    for al in range(4):
        for t5 in range(T):
            tcode[al * 32:(al + 1) * 32, al * 5 + t5] = float(ATOM_TYPES[t5])
    sel8 = np.zeros((128, 8), np.float32)
    selbc = np.zeros((8, 128), np.float32)
    for gp in range(4):
        for al in range(4):
            for t5 in range(T):
                row = 32 * gp + 5 * al + t5
                col = 4 * (al % 2) + gp
                sel8[row, col] = 1.0
                selbc[col, row] = 1.0
    biases = np.zeros((128, 98), np.float32)
    for p in range(P):
        biases[:, p] = -float(rsv[p])
        biases[:, 48 + p] = LN2 - float(rev[p]) * float(rsv[p]) ** 2
    biases[:, 96] = float(np.pi / 2.0)
    biases[:, 97] = LN2
    # output unscramble: [128, OUTW] -> [4096, 240]
    a = np.arange(A)
    al = a // AH
    ah = a % AH
    ti = ah // 40
    rem = ah % 40
    gf = rem // 4
    gp = rem % 4
    c = np.arange(C_OUT)
    t5 = c // P
    p = c % P
    rows = (32 * gp[:, None] + 5 * al[:, None] + t5[None, :]).astype(np.int64)
    cols = ((ti * 480 + gf * 48)[:, None] + p[None, :]).astype(np.int64)
    return (tcode.astype(bfloat16), sel8.astype(bfloat16), selbc, biases,
            rows, cols)


def kernel(X, rc, rs, re, Nbrs, Nbrs_Z):
    from ml_dtypes import bfloat16
    X = np.asarray(X, np.float32)
    rc = np.asarray(rc, np.float32).ravel()
    rs = np.asarray(rs, np.float32).ravel()
    re = np.asarray(re, np.float32).ravel()
    Nbrs = np.asarray(Nbrs, np.int32)
    Nbrs_Z = np.asarray(Nbrs_Z, np.int32)

    nc = build_nc(rc, rs, re)
    tcode, sel8, selbc, biases, orows, ocols = _host_tables(rs, re)

    # per-(a,m)-tile layouts: partition p = (a//1024)*32 + m, free = a % 1024
    in_maps = []
    for core in range(NC_CORES):
        bsl = slice(core * B_LOC, (core + 1) * B_LOC)
        Xc = X[bsl].reshape(A, 3)                       # a = b_loc*2048 + n
        Nb = Nbrs[bsl].reshape(A, M)
        Zb = Nbrs_Z[bsl].reshape(A, M)
        gidx = Nb + (np.arange(A)[:, None] // N) * N    # [A, M] global rows
        a_hi = np.arange(A) % AH
        part = ((np.arange(A) // AH)[:, None] * 32
                + np.arange(M)[None]).astype(np.int32)
        zt = np.zeros((128, AH), np.float32)
        zt[part.ravel(), np.repeat(a_hi, M)] = Zb.ravel().astype(np.float32)
        D = Xc[gidx] - Xc[:, None, :]                   # [A, M, 3]
        Rv = np.sqrt(np.sum(D * D, axis=2, dtype=np.float32))
        rr = np.zeros((128, AH), np.float32)
        rr[part.ravel(), np.repeat(a_hi, M)] = Rv.ravel()
        in_maps.append({
            "rr": rr, "zt": zt.astype(bfloat16), "tcode": tcode,
            "sel8": sel8, "selbc": selbc, "biases": biases,
        })

    res = run_bass_kernel_spmd(nc, in_maps, core_ids=list(range(NC_CORES)),
                               trace=_TRACE[0])
    if _TRACE[0]:
        kernel.last_exec_ns = res.exec_time_ns
        kernel.last_profile = res

    out = np.zeros((B, N, C_OUT), np.float32)
    for core in range(NC_CORES):
        o = np.asarray(res.results[core]["out"]).astype(np.float32)
        out[core * B_LOC:(core + 1) * B_LOC] = o[orows, ocols].reshape(
            B_LOC, N, C_OUT)
    return out


# revision 36
# speedup vs baseline: 3.3854x; 1.0165x over previous
"""AtomicConvolution Trainium2 kernel (8 NeuronCores, data-parallel over B).

Pipeline per core (2 complexes, 4096 atoms, layout [par=(a_lo*32+m), free=a_hi]):
  host computes R (gather + norm) -> radial fn on device in two table-set
  batched ACT phases (Square+Exp phase, then Sin phase; half-angle form
  rsf = (sin(pi/2 - theta/2))^2 * 2*exp(..) avoids the slow 3-operand DVE
  op) writing contiguous-per-p rsf (bf16) -> per-atom-group masked type
  reduction on TensorE (block-diagonal 0/1 weights from is_equal) -> PSUM
  -> sym parked in SBUF (bf16, PSUM-native layout; host unscrambles) ->
  per-atom BN stats via selector matmuls + split AllReduce (first half
  overlaps the remaining tile loop) -> normalize in place -> one out DMA.
  Stored rsf = +2*f; BN epilogue uses eps*4 to compensate.
"""
import sys
import types
import numpy as np

ATOM_TYPES = (1, 6, 7, 8, 16)
BN_EPS = 1e-5
B, N, M, P = 16, 2048, 32, 48
T = len(ATOM_TYPES)
NC_CORES = 8
B_LOC = B // NC_CORES            # 2 complexes per core
A = B_LOC * N                    # 4096 atoms per core
AH = A // 4                      # 1024 free columns
C_OUT = P * T                    # 240 channels
NTILE = 26                       # ceil(1024 / 40) psum tiles
OUTW = NTILE * 480               # 12480 staged output columns
DVE_PREP = 34                    # how many p's compute the exp arg on DVE
ACT_SQ = 26                      # how many p's square the sin on ACT
HALF_T = 10                      # collective split point (tiles 0..9 | 10..25)
NMC = 20                         # mask columns per group (al*5+t5)

_TRACE = [False]

# ---------------------------------------------------------------- env patches
import concourse.bass as bass
import concourse.mybir as mybir
import concourse.tile as tile
import concourse.bass_utils as bu
from concourse.bass_utils import run_bass_kernel_spmd
from concourse.tile import TileContext, add_dep_helper


def _patch_tile_tail_drain():
    tile_mod = tile
    ScopedClock = None
    for _n in dir(tile_mod):
        if "ScopedClock" in _n:
            ScopedClock = getattr(tile_mod, _n)

    def _drain(self, tick_clock, wait_clock):
        nc = self.nc
        nops = [nc.sync.nop(nofuse=True) for _ in range(30)]
        drain_inst = nc.sync.drain()
        wait_clock.add_sem_waits(
            drain_inst.ins, ScopedClock({None: tick_clock.global_clock})
        )
        si = drain_inst.ins.sync_info
        if si is not None and si.on_wait and len(si.on_wait) > 1:
            waits = list(si.on_wait)
            si.on_wait = waits[:1]
            rest = waits[1:]
            assert len(rest) <= len(nops)
            for i, nop in enumerate(nops):
                chunk = rest[i:i + 1]
                if not chunk:
                    break
                nsi = nop.ins.sync_info
                if nsi is None:
                    nop.ins.sync_info = mybir.SyncInfo(on_wait=chunk, on_update=[])
                else:
                    nsi.on_wait = chunk
        nc.all_engine_barrier()
        popped = nc._tile_sem_poison_stack.pop()
        assert popped is self._sem_poison
        nc.clear_and_free_semaphores(list(self.sems.allocated().values()))
        nc.all_engine_barrier()

    TileContext._drain_and_barrier = _drain


WAIT_CAP = 1


def _make_spare_nops(nc, counts):
    # SP-engine carrier nops: the only engine whose sequencer NoOp reliably
    # encodes with sem waits in this walrus build.
    return {"carriers": [nc.sync.nop(nofuse=True) for _ in range(4000)]}


def _fix_sync_waits(nc, spares, relay):
    clr = nc.sync.sem_clear(relay)
    relay_count = [0]
    carriers = spares["carriers"]
    spare_names = {c.ins.name for c in carriers}
    # move the freshly-appended clear to the very beginning of the first block
    fn0 = nc.m.functions[0]
    for bb in fn0.blocks:
        if clr.ins in bb.instructions:
            bb.instructions.remove(clr.ins)
    fn0.blocks[0].instructions.insert(0, clr.ins)
    for fn in nc.m.functions:
        for bb in fn.blocks:
            bb.instructions[:] = [
                i for i in bb.instructions if i.name not in spare_names
            ]
    for fn in nc.m.functions:
        for bb in fn.blocks:
            new = []
            for inst in bb.instructions:
                si = inst.sync_info
                waits = list(si.on_wait) if si is not None and si.on_wait else []
                if len(waits) > WAIT_CAP:
                    for w in waits:
                        assert carriers, "out of relay carriers"
                        car = carriers.pop()
                        car.then_inc(relay, 1)
                        car.ins.sync_info.on_wait = [w]
                        relay_count[0] += 1
                        new.append(car.ins)
                    si.on_wait = [mybir.SyncWait(
                        sync_type="semaphore", id=relay.num,
                        ant_name=relay.name, wait_mode="sem-ge-imm",
                        wait_value=relay_count[0], wait_reg=None)]
                new.append(inst)
            bb.instructions[:] = new


def _patch_walrus_dyndma(size=16384):
    if getattr(bu.run_command, "_walrus_patched", False):
        return
    _orig = bu.run_command

    def run2(cmd, cwd=None, **kw):
        try:
            if cmd and "walrus_driver" in str(cmd[0]) and any(
                "codegen" in str(c) for c in cmd
            ):
                cmd = list(cmd) + [
                    f"--dynamic-dma-scratch-size-per-partition={size}"
                ]
        except Exception:
            pass
        return _orig(cmd, cwd=cwd, **kw)

    run2._walrus_patched = True
    bu.run_command = run2


def _install_ntff_hook():
    if "antenv.axon_hooks" in sys.modules:
        return
    try:
        from trn_agent_boot.trn_boot import _ntff_profile_via_ctypes
        hook = _ntff_profile_via_ctypes("/opt/axon/libaxon_pjrt.so")
    except Exception:
        hook = None
    m = types.ModuleType("antenv.axon_hooks")
    m._hook = hook
    m.get_axon_ntff_profile_hook = lambda: m._hook
    m.set_axon_ntff_profile_hook = lambda h: setattr(m, "_hook", h)
    sys.modules["antenv.axon_hooks"] = m
    try:
        import antenv
        antenv.axon_hooks = m
    except Exception:
        pass


_patch_tile_tail_drain()
_patch_walrus_dyndma()
_install_ntff_hook()

DT = mybir.dt


def _mk_ap(base_ap, off_elems, free_dims):
    return bass.AP(base_ap.tensor, base_ap.offset + off_elems,
                   [base_ap.ap[0]] + free_dims)


# ---------------------------------------------------------------- bass build
def build_nc(rcv, rsv, rev):
    nc = bass.Bass(dynamic_dma_scratch_size=8192)
    f32, bf16 = DT.float32, DT.bfloat16

    PIH = float(np.pi / 2.0)
    AL = mybir.AluOpType
    AF = mybir.ActivationFunctionType

    rr_ext = nc.declare_dram_parameter("rr", [128, AH], f32, isOutput=False)
    zt_ext = nc.declare_dram_parameter("zt", [128, AH], bf16, isOutput=False)
    tc_ext = nc.declare_dram_parameter("tcode", [128, NMC], bf16,
                                       isOutput=False)
    s8_ext = nc.declare_dram_parameter("sel8", [128, 8], bf16, isOutput=False)
    sb_ext = nc.declare_dram_parameter("selbc", [8, 128], f32, isOutput=False)
    # bias table: col p -> -rs_p (Square bias), col 48+p -> exp-path bias,
    # col 96 -> +pi/2 (Sin), col 97 -> ln2
    bi_ext = nc.declare_dram_parameter("biases", [128, 98], f32, isOutput=False)
    out_ext = nc.declare_dram_parameter("out", [128, OUTW], bf16, isOutput=True)

    st_w = [2 * HALF_T * 10, 2 * (260 - HALF_T * 10)]
    st_in = [nc.dram_tensor(f"st_in{h}", [8, st_w[h]], f32) for h in range(2)]
    st_out = [nc.dram_tensor(f"st_out{h}", [8, st_w[h]], f32,
                             addr_space="Shared") for h in range(2)]

    relay_sem = nc.semaphore("wait_relay").__enter__()
    with TileContext(nc) as tc:
        spares = _make_spare_nops(nc, {})
        with tc.tile_pool(name="main", bufs=1) as pool, \
             tc.tile_pool(name="work", bufs=2) as wpool, \
             tc.tile_pool(name="uprep", bufs=4) as upool, \
             tc.tile_pool(name="rcap", bufs=6) as rpool, \
             tc.tile_pool(name="wm", bufs=3) as mpool, \
             tc.tile_pool(name="psum", bufs=3, space="PSUM") as ppool, \
             tc.tile_pool(name="py", bufs=2, space="PSUM") as ypool, \
             tc.tile_pool(name="pstat", bufs=1, space="PSUM") as spool:

            # ---- loads (rr + biases first: they gate the radial phase)
            rr = pool.tile([128, AH], f32)
            nc.sync.dma_start(out=rr[:], in_=rr_ext[:])
            bia = pool.tile([128, 98], f32)
            nc.sync.dma_start(out=bia[:], in_=bi_ext[:])
            r2 = pool.tile([128, AH], f32)
            nc.vector.tensor_tensor(out=r2[:], in0=rr[:], in1=rr[:],
                                    op=AL.mult)
            zt = pool.tile([128, AH], bf16)
            nc.sync.dma_start(out=zt[:], in_=zt_ext[:])
            tcode = pool.tile([128, NMC], bf16)
            nc.sync.dma_start(out=tcode[:], in_=tc_ext[:])
            sel8 = pool.tile([128, 8], bf16)
            nc.sync.dma_start(out=sel8[:], in_=s8_ext[:])
            selbc = pool.tile([128, 128], f32)
            nc.sync.dma_start(out=selbc[0:8, :], in_=sb_ext[:])

            # rsf layout: contiguous per p -> col = p*AH + a_hi
            rsf = pool.tile([128, P * AH], bf16)

            # ---- phase A: kk'_p = 2*exp(-re*(R-rs)^2), bf16.
            # ACT-path p's (Square+Exp, one table set) run first so ACT
            # starts immediately; DVE_PREP p's build the exp arg on DVE
            # meanwhile (deeper uprep pool so DVE runs ahead).
            last_exp = None
            keys = []
            for p in range(P):
                if p < DVE_PREP:
                    keys.append(((p + 0.5) / DVE_PREP, 1, p))
                else:
                    keys.append(((p - DVE_PREP + 0.5) / (P - DVE_PREP), 0, p))
            p_order = [p for _, _, p in sorted(keys)]
            for p in p_order:
                re_p, rs_p = float(rev[p]), float(rsv[p])
                if p < DVE_PREP:
                    t1 = wpool.tile([128, AH], f32, tag="t1")
                    nc.vector.tensor_scalar(
                        out=t1[:], in0=rr[:], scalar1=-2.0 * rs_p,
                        scalar2=None, op0=AL.mult)
                    u = upool.tile([128, AH], f32, tag="u")
                    nc.vector.tensor_tensor(out=u[:], in0=t1[:], in1=r2[:],
                                            op=AL.add)
                    # exp(-re*u + (ln2 - re*rs^2)) = 2*exp(-re*(R-rs)^2)
                    ei = nc.scalar.activation(
                        out=rsf[:, p * AH:(p + 1) * AH], in_=u[:],
                        func=AF.Exp, scale=-re_p,
                        bias=bia[:, 48 + p:49 + p])
                else:
                    ua = wpool.tile([128, AH], f32, tag="ua")
                    nc.scalar.activation(out=ua[:], in_=rr[:], func=AF.Square,
                                         bias=bia[:, p:p + 1])
                    ei = nc.scalar.activation(
                        out=rsf[:, p * AH:(p + 1) * AH], in_=ua[:],
                        func=AF.Exp, scale=-re_p,
                        bias=bia[:, 97:98])
                last_exp = ei

            # ---- phase B: s = sin(pi/2 - pi*min(R,rc)/(2rc)) (>=0, LUT-safe)
            # rsf *= s*s  ->  rsf = 2*f_p.  Sins forced after all Exps so the
            # ACT table set switches exactly once; rcap pool is deep so DVE
            # computes sin args well ahead.
            for p in range(P):
                rc_p = float(rcv[p])
                rt = rpool.tile([128, AH], bf16, tag="rt")
                nc.vector.tensor_scalar(
                    out=rt[:], in0=rr[:], scalar1=rc_p,
                    scalar2=float(np.pi / (2.0 * rc_p)),
                    op0=AL.min, op1=AL.mult)
                cs = wpool.tile([128, AH], bf16, tag="cs")
                si = nc.scalar.activation(out=cs[:], in_=rt[:], func=AF.Sin,
                                          scale=-1.0, bias=bia[:, 96:97])
                add_dep_helper(si.ins, last_exp.ins,
                               reason="keep Sin phase after Exp phase")
                s2t = wpool.tile([128, AH], bf16, tag="s2t")
                if p < ACT_SQ:
                    # Square is in every table set -> no reload
                    nc.scalar.activation(out=s2t[:], in_=cs[:],
                                         func=AF.Square)
                else:
                    nc.vector.tensor_tensor(out=s2t[:], in0=cs[:], in1=cs[:],
                                            op=AL.mult)
                psl = rsf[:, p * AH:(p + 1) * AH]
                nc.vector.tensor_tensor(out=psl, in0=s2t[:], in1=psl,
                                        op=AL.mult)

            # ---- TensorE masked reduction; sym parked in SBUF (bf16)
            # group g = one a_hi; psum tile: rows 32*gp + (al*5+t5),
            # cols gf*48 + p, for g = ti*40 + gf*4 + gp
            sym = pool.tile([128, OUTW], bf16)
            rst = pool.tile([128, 260], f32)   # rows 0:8 = s1, 32:40 = s2
            mbc = pool.tile([128, 260], f32)
            ibc = pool.tile([128, 260], f32)
            sall = [None, None]
            GRP_T = 40

            def stats_half(h):
                # per-n stats already folded per tile; just ship + AllReduce
                c0, c1 = (0, HALF_T * 10) if h == 0 else (HALF_T * 10, 260)
                w = c1 - c0
                nc.sync.dma_start(out=st_in[h][:, 0:w], in_=rst[0:8, c0:c1])
                nc.sync.dma_start(out=st_in[h][:, w:2 * w],
                                  in_=rst[32:40, c0:c1])
                nc.gpsimd.collective_compute(
                    "AllReduce", AL.add,
                    ins=[st_in[h][:]], outs=[st_out[h][:]],
                    replica_groups=[list(range(NC_CORES))])
                sa = pool.tile([128, 2 * w], f32)
                nc.sync.dma_start(out=sa[0:8, :], in_=st_out[h][:])
                sall[h] = sa

            def epilogue_half(h):
                # mean/inv on [8,w], broadcast to 128 rows via matmul
                c0, c1 = (0, HALF_T * 10) if h == 0 else (HALF_T * 10, 260)
                w = c1 - c0
                sa = sall[h]
                inv_n = 1.0 / (B * C_OUT)
                mean = wpool.tile([128, 200], f32, tag="mean")
                nc.vector.tensor_scalar(out=mean[0:8, :w], in0=sa[0:8, 0:w],
                                        scalar1=inv_n, scalar2=None,
                                        op0=AL.mult)
                vpe = wpool.tile([128, 200], f32, tag="vpe")
                nc.vector.tensor_scalar(out=vpe[0:8, :w], in0=sa[0:8, w:2 * w],
                                        scalar1=inv_n, scalar2=None,
                                        op0=AL.mult)
                msq = wpool.tile([128, 200], f32, tag="msq")
                nc.vector.tensor_tensor(out=msq[0:8, :w], in0=mean[0:8, :w],
                                        in1=mean[0:8, :w], op=AL.mult)
                nc.vector.tensor_tensor(out=vpe[0:8, :w], in0=vpe[0:8, :w],
                                        in1=msq[0:8, :w], op=AL.subtract)
                nc.vector.tensor_scalar(out=vpe[0:8, :w], in0=vpe[0:8, :w],
                                        scalar1=float(4.0 * BN_EPS),
                                        scalar2=None, op0=AL.add)
                sdev = wpool.tile([128, 200], f32, tag="sdev")
                nc.scalar.activation(out=sdev[0:8, :w], in_=vpe[0:8, :w],
                                     func=AF.Sqrt)
                inv = wpool.tile([128, 200], f32, tag="inv")
                nc.vector.reciprocal(out=inv[0:8, :w], in_=sdev[0:8, :w])
                mbp = spool.tile([128, 200], f32, tag="mbp")
                ibp = spool.tile([128, 200], f32, tag="ibp")
                nc.tensor.matmul(out=mbp[:, :w], lhsT=selbc[0:8, :],
                                 rhs=mean[0:8, :w], start=True, stop=True)
                nc.tensor.matmul(out=ibp[:, :w], lhsT=selbc[0:8, :],
                                 rhs=inv[0:8, :w], start=True, stop=True)
                nc.vector.tensor_copy(out=mbc[:, c0:c1], in_=mbp[:, :w])
                nc.vector.tensor_copy(out=ibc[:, c0:c1], in_=ibp[:, :w])

            def normalize_tile(ti):
                nfgf = 10 if ti < 25 else 6
                ncol = nfgf * 48
                ssl = sym[:, ti * 480:ti * 480 + ncol]
                t1 = wpool.tile([128, 480], f32, tag="nt")
                nc.vector.tensor_tensor(
                    out=t1[:, :ncol], in0=ssl,
                    in1=_mk_ap(mbc[:], ti * 10, [[1, nfgf], [0, 48]]),
                    op=AL.subtract)
                nc.vector.tensor_tensor(
                    out=ssl, in0=t1[:, :ncol],
                    in1=_mk_ap(ibc[:], ti * 10, [[1, nfgf], [0, 48]]),
                    op=AL.mult)

            for ti in range(NTILE):
                ngrp = GRP_T if ti < 25 else 24
                nfgf = 10 if ti < 25 else 6
                ncol = nfgf * 48
                wmask = mpool.tile([128, GRP_T * NMC], bf16, tag="wmask")
                in0 = _mk_ap(zt[:], ti * GRP_T, [[1, ngrp], [0, NMC]])
                t0 = _mk_ap(tcode[:], 0, [[0, ngrp], [1, NMC]])
                nc.vector.tensor_tensor(
                    out=wmask[:, :ngrp * NMC], in0=in0, in1=t0, op=AL.is_equal)
                stp = ppool.tile([128, 480], f32, tag="stp")
                if ti < 3:
                    # 20-col masks leave psum rows 20..31 of each strip
                    # unwritten; clear each rotating buffer once so parked
                    # garbage is finite
                    nc.vector.memset(stp[:], 0.0)
                for gi in range(ngrp):
                    g = ti * GRP_T + gi
                    gp, gf = gi % 4, gi // 4
                    rhs = _mk_ap(rsf[:], g, [[AH, P]])
                    nc.tensor.matmul(
                        out=stp[32 * gp:32 * gp + 20, gf * 48:(gf + 1) * 48],
                        lhsT=wmask[:, gi * NMC:(gi + 1) * NMC],
                        rhs=rhs, start=True, stop=True,
                        tile_position=(0, 32 * gp))
                # park + square on idle ACT; fold rows via sel8 matmul, then
                # tiny per-gf reduces on the [8, ncol] results
                ssl = sym[:, ti * 480:ti * 480 + ncol]
                nc.scalar.activation(out=ssl, in_=stp[:, :ncol], func=AF.Copy)
                sqt = wpool.tile([128, 480], bf16, tag="sqt")
                nc.scalar.activation(out=sqt[:, :ncol], in_=stp[:, :ncol],
                                     func=AF.Square)
                y1 = ypool.tile([128, 480], f32, tag="y1")
                if ti < 2:
                    nc.vector.memset(y1[:], 0.0)
                nc.tensor.matmul(out=y1[0:8, :ncol], lhsT=sel8[:, 0:8],
                                 rhs=ssl, start=True, stop=True)
                nc.tensor.matmul(out=y1[32:40, :ncol], lhsT=sel8[:, 0:8],
                                 rhs=sqt[:, :ncol], start=True, stop=True,
                                 tile_position=(0, 32))
                nc.vector.tensor_reduce(
                    out=rst[0:40, ti * 10:ti * 10 + nfgf],
                    in_=_mk_ap(y1[0:40, :], 0, [[48, nfgf], [1, 48]]),
                    axis=mybir.AxisListType.X, op=AL.add)
                if ti == HALF_T - 1:
                    stats_half(0)
                    epilogue_half(0)
            stats_half(1)
            for ti in range(HALF_T):
                normalize_tile(ti)
            ch = HALF_T * 480
            nc.sync.dma_start(out=out_ext[:, 0:ch], in_=sym[:, 0:ch])
            epilogue_half(1)
            for ti in range(HALF_T, NTILE):
                normalize_tile(ti)
            nc.sync.dma_start(out=out_ext[:, ch:OUTW], in_=sym[:, ch:OUTW])

    _fix_sync_waits(nc, spares, relay_sem)
    return nc


# ---------------------------------------------------------------- host driver
def _host_tables(rsv, rev):
    from ml_dtypes import bfloat16
    LN2 = float(np.log(2.0))
    tcode = np.full((128, NMC), -1.0, np.float32)
    for al in range(4):
        for t5 in range(T):
            tcode[al * 32:(al + 1) * 32, al * 5 + t5] = float(ATOM_TYPES[t5])
    sel8 = np.zeros((128, 8), np.float32)
    selbc = np.zeros((8, 128), np.float32)
    for gp in range(4):
        for al in range(4):
            for t5 in range(T):
                row = 32 * gp + 5 * al + t5
                col = 4 * (al % 2) + gp
                sel8[row, col] = 1.0
                selbc[col, row] = 1.0
    biases = np.zeros((128, 98), np.float32)
    for p in range(P):
        biases[:, p] = -float(rsv[p])
        biases[:, 48 + p] = LN2 - float(rev[p]) * float(rsv[p]) ** 2
    biases[:, 96] = float(np.pi / 2.0)
    biases[:, 97] = LN2
    # output unscramble: [128, OUTW] -> [4096, 240]
    a = np.arange(A)
    al = a // AH
    ah = a % AH
    ti = ah // 40
    rem = ah % 40
    gf = rem // 4
    gp = rem % 4
    c = np.arange(C_OUT)
    t5 = c // P
    p = c % P
    rows = (32 * gp[:, None] + 5 * al[:, None] + t5[None, :]).astype(np.int64)
    cols = ((ti * 480 + gf * 48)[:, None] + p[None, :]).astype(np.int64)
    return (tcode.astype(bfloat16), sel8.astype(bfloat16), selbc, biases,
            rows, cols)


def kernel(X, rc, rs, re, Nbrs, Nbrs_Z):
    from ml_dtypes import bfloat16
    X = np.asarray(X, np.float32)
    rc = np.asarray(rc, np.float32).ravel()
    rs = np.asarray(rs, np.float32).ravel()
    re = np.asarray(re, np.float32).ravel()
    Nbrs = np.asarray(Nbrs, np.int32)
    Nbrs_Z = np.asarray(Nbrs_Z, np.int32)

    nc = build_nc(rc, rs, re)
    tcode, sel8, selbc, biases, orows, ocols = _host_tables(rs, re)

    # per-(a,m)-tile layouts: partition p = (a//1024)*32 + m, free = a % 1024
    in_maps = []
    for core in range(NC_CORES):
        bsl = slice(core * B_LOC, (core + 1) * B_LOC)
        Xc = X[bsl].reshape(A, 3)                       # a = b_loc*2048 + n
        Nb = Nbrs[bsl].reshape(A, M)
        Zb = Nbrs_Z[bsl].reshape(A, M)
        gidx = Nb + (np.arange(A)[:, None] // N) * N    # [A, M] global rows
        a_hi = np.arange(A) % AH
        part = ((np.arange(A) // AH)[:, None] * 32
                + np.arange(M)[None]).astype(np.int32)
        zt = np.zeros((128, AH), np.float32)
        zt[part.ravel(), np.repeat(a_hi, M)] = Zb.ravel().astype(np.float32)
        D = Xc[gidx] - Xc[:, None, :]                   # [A, M, 3]
        Rv = np.sqrt(np.sum(D * D, axis=2, dtype=np.float32))
        rr = np.zeros((128, AH), np.float32)
        rr[part.ravel(), np.repeat(a_hi, M)] = Rv.ravel()
        in_maps.append({
            "rr": rr, "zt": zt.astype(bfloat16), "tcode": tcode,
            "sel8": sel8, "selbc": selbc, "biases": biases,
        })

    res = run_bass_kernel_spmd(nc, in_maps, core_ids=list(range(NC_CORES)),
                               trace=_TRACE[0])
    if _TRACE[0]:
        kernel.last_exec_ns = res.exec_time_ns
        kernel.last_profile = res

    out = np.zeros((B, N, C_OUT), np.float32)
    for core in range(NC_CORES):
        o = np.asarray(res.results[core]["out"]).astype(np.float32)
        out[core * B_LOC:(core + 1) * B_LOC] = o[orows, ocols].reshape(
            B_LOC, N, C_OUT)
    return out


# revision 38
# speedup vs baseline: 3.4274x; 1.0124x over previous
"""AtomicConvolution Trainium2 kernel (8 NeuronCores, data-parallel over B).

Pipeline per core (2 complexes, 4096 atoms, layout [par=(a_lo*32+m), free=a_hi]):
  host computes R (gather + norm) -> radial fn on device in two table-set
  batched ACT phases (Square+Exp phase, then Sin phase; half-angle form
  rsf = (sin(pi/2 - theta/2))^2 * 2*exp(..) avoids the slow 3-operand DVE
  op) writing contiguous-per-p rsf (bf16) -> per-atom-group masked type
  reduction on TensorE (block-diagonal 0/1 weights from is_equal) -> PSUM
  -> sym parked in SBUF (bf16, PSUM-native layout; host unscrambles) ->
  per-atom BN stats via selector matmuls + split AllReduce (first half
  overlaps the remaining tile loop) -> normalize in place -> one out DMA.
  Stored rsf = +2*f; BN epilogue uses eps*4 to compensate.
"""
import sys
import types
import numpy as np

ATOM_TYPES = (1, 6, 7, 8, 16)
BN_EPS = 1e-5
B, N, M, P = 16, 2048, 32, 48
T = len(ATOM_TYPES)
NC_CORES = 8
B_LOC = B // NC_CORES            # 2 complexes per core
A = B_LOC * N                    # 4096 atoms per core
AH = A // 4                      # 1024 free columns
C_OUT = P * T                    # 240 channels
NTILE = 26                       # ceil(1024 / 40) psum tiles
OUTW = NTILE * 480               # 12480 staged output columns
DVE_PREP = 34                    # how many p's compute the exp arg on DVE
ACT_SQ = 26                      # how many p's square the sin on ACT
HALF_T = 10                      # collective split point (tiles 0..9 | 10..25)
NMC = 20                         # mask columns per group (al*5+t5)

_TRACE = [False]

# ---------------------------------------------------------------- env patches
import concourse.bass as bass
import concourse.mybir as mybir
import concourse.tile as tile
import concourse.bass_utils as bu
from concourse.bass_utils import run_bass_kernel_spmd
from concourse.tile import TileContext, add_dep_helper


def _patch_tile_tail_drain():
    tile_mod = tile
    ScopedClock = None
    for _n in dir(tile_mod):
        if "ScopedClock" in _n:
            ScopedClock = getattr(tile_mod, _n)

    def _drain(self, tick_clock, wait_clock):
        nc = self.nc
        nops = [nc.sync.nop(nofuse=True) for _ in range(30)]
        drain_inst = nc.sync.drain()
        wait_clock.add_sem_waits(
            drain_inst.ins, ScopedClock({None: tick_clock.global_clock})
        )
        si = drain_inst.ins.sync_info
        if si is not None and si.on_wait and len(si.on_wait) > 1:
            waits = list(si.on_wait)
            si.on_wait = waits[:1]
            rest = waits[1:]
            assert len(rest) <= len(nops)
            for i, nop in enumerate(nops):
                chunk = rest[i:i + 1]
                if not chunk:
                    break
                nsi = nop.ins.sync_info
                if nsi is None:
                    nop.ins.sync_info = mybir.SyncInfo(on_wait=chunk, on_update=[])
                else:
                    nsi.on_wait = chunk
        nc.all_engine_barrier()
        popped = nc._tile_sem_poison_stack.pop()
        assert popped is self._sem_poison
        nc.clear_and_free_semaphores(list(self.sems.allocated().values()))
        nc.all_engine_barrier()

    TileContext._drain_and_barrier = _drain


WAIT_CAP = 1


def _make_spare_nops(nc, counts):
    # SP-engine carrier nops: the only engine whose sequencer NoOp reliably
    # encodes with sem waits in this walrus build.
    return {"carriers": [nc.sync.nop(nofuse=True) for _ in range(4000)]}


def _fix_sync_waits(nc, spares, relay):
    clr = nc.sync.sem_clear(relay)
    relay_count = [0]
    carriers = spares["carriers"]
    spare_names = {c.ins.name for c in carriers}
    # move the freshly-appended clear to the very beginning of the first block
    fn0 = nc.m.functions[0]
    for bb in fn0.blocks:
        if clr.ins in bb.instructions:
            bb.instructions.remove(clr.ins)
    fn0.blocks[0].instructions.insert(0, clr.ins)
    for fn in nc.m.functions:
        for bb in fn.blocks:
            bb.instructions[:] = [
                i for i in bb.instructions if i.name not in spare_names
            ]
    for fn in nc.m.functions:
        for bb in fn.blocks:
            new = []
            for inst in bb.instructions:
                si = inst.sync_info
                waits = list(si.on_wait) if si is not None and si.on_wait else []
                if len(waits) > WAIT_CAP:
                    for w in waits:
                        assert carriers, "out of relay carriers"
                        car = carriers.pop()
                        car.then_inc(relay, 1)
                        car.ins.sync_info.on_wait = [w]
                        relay_count[0] += 1
                        new.append(car.ins)
                    si.on_wait = [mybir.SyncWait(
                        sync_type="semaphore", id=relay.num,
                        ant_name=relay.name, wait_mode="sem-ge-imm",
                        wait_value=relay_count[0], wait_reg=None)]
                new.append(inst)
            bb.instructions[:] = new


def _patch_walrus_dyndma(size=16384):
    if getattr(bu.run_command, "_walrus_patched", False):
        return
    _orig = bu.run_command

    def run2(cmd, cwd=None, **kw):
        try:
            if cmd and "walrus_driver" in str(cmd[0]) and any(
                "codegen" in str(c) for c in cmd
            ):
                cmd = list(cmd) + [
                    f"--dynamic-dma-scratch-size-per-partition={size}"
                ]
        except Exception:
            pass
        return _orig(cmd, cwd=cwd, **kw)

    run2._walrus_patched = True
    bu.run_command = run2


def _install_ntff_hook():
    if "antenv.axon_hooks" in sys.modules:
        return
    try:
        from trn_agent_boot.trn_boot import _ntff_profile_via_ctypes
        hook = _ntff_profile_via_ctypes("/opt/axon/libaxon_pjrt.so")
    except Exception:
        hook = None
    m = types.ModuleType("antenv.axon_hooks")
    m._hook = hook
    m.get_axon_ntff_profile_hook = lambda: m._hook
    m.set_axon_ntff_profile_hook = lambda h: setattr(m, "_hook", h)
    sys.modules["antenv.axon_hooks"] = m
    try:
        import antenv
        antenv.axon_hooks = m
    except Exception:
        pass


_patch_tile_tail_drain()
_patch_walrus_dyndma()
_install_ntff_hook()

DT = mybir.dt


def _mk_ap(base_ap, off_elems, free_dims):
    return bass.AP(base_ap.tensor, base_ap.offset + off_elems,
                   [base_ap.ap[0]] + free_dims)


# ---------------------------------------------------------------- bass build
def build_nc(rcv, rsv, rev):
    nc = bass.Bass(dynamic_dma_scratch_size=8192)
    f32, bf16 = DT.float32, DT.bfloat16

    PIH = float(np.pi / 2.0)
    AL = mybir.AluOpType
    AF = mybir.ActivationFunctionType

    rr_ext = nc.declare_dram_parameter("rr", [128, AH], f32, isOutput=False)
    zt_ext = nc.declare_dram_parameter("zt", [128, AH], bf16, isOutput=False)
    tc_ext = nc.declare_dram_parameter("tcode", [128, NMC], bf16,
                                       isOutput=False)
    s8_ext = nc.declare_dram_parameter("sel8", [128, 8], bf16, isOutput=False)
    sb_ext = nc.declare_dram_parameter("selbc", [8, 128], f32, isOutput=False)
    # bias table: col p -> -rs_p (Square bias), col 48+p -> exp-path bias,
    # col 96 -> +pi/2 (Sin), col 97 -> ln2
    bi_ext = nc.declare_dram_parameter("biases", [128, 98], f32, isOutput=False)
    out_ext = nc.declare_dram_parameter("out", [128, OUTW], bf16, isOutput=True)

    st_w = [2 * HALF_T * 10, 2 * (260 - HALF_T * 10)]
    st_in = [nc.dram_tensor(f"st_in{h}", [8, st_w[h]], f32) for h in range(2)]
    st_out = [nc.dram_tensor(f"st_out{h}", [8, st_w[h]], f32,
                             addr_space="Shared") for h in range(2)]

    relay_sem = nc.semaphore("wait_relay").__enter__()
    with TileContext(nc) as tc:
        spares = _make_spare_nops(nc, {})
        with tc.tile_pool(name="main", bufs=1) as pool, \
             tc.tile_pool(name="work", bufs=2) as wpool, \
             tc.tile_pool(name="uprep", bufs=4) as upool, \
             tc.tile_pool(name="rcap", bufs=6) as rpool, \
             tc.tile_pool(name="wm", bufs=3) as mpool, \
             tc.tile_pool(name="psum", bufs=3, space="PSUM") as ppool, \
             tc.tile_pool(name="py", bufs=3, space="PSUM") as ypool, \
             tc.tile_pool(name="pstat", bufs=1, space="PSUM") as spool:

            # ---- loads (rr + biases first: they gate the radial phase)
            rr = pool.tile([128, AH], f32)
            nc.sync.dma_start(out=rr[:], in_=rr_ext[:])
            bia = pool.tile([128, 98], f32)
            nc.sync.dma_start(out=bia[:], in_=bi_ext[:])
            r2 = pool.tile([128, AH], f32)
            nc.vector.tensor_tensor(out=r2[:], in0=rr[:], in1=rr[:],
                                    op=AL.mult)
            zt = pool.tile([128, AH], bf16)
            nc.sync.dma_start(out=zt[:], in_=zt_ext[:])
            tcode = pool.tile([128, NMC], bf16)
            nc.sync.dma_start(out=tcode[:], in_=tc_ext[:])
            sel8 = pool.tile([128, 8], bf16)
            nc.sync.dma_start(out=sel8[:], in_=s8_ext[:])
            selbc = pool.tile([128, 128], f32)
            nc.sync.dma_start(out=selbc[0:8, :], in_=sb_ext[:])

            # rsf layout: contiguous per p -> col = p*AH + a_hi
            rsf = pool.tile([128, P * AH], bf16)

            # ---- phase A: kk'_p = 2*exp(-re*(R-rs)^2), bf16.
            # ACT-path p's (Square+Exp, one table set) run first so ACT
            # starts immediately; DVE_PREP p's build the exp arg on DVE
            # meanwhile (deeper uprep pool so DVE runs ahead).
            last_exp = None
            keys = []
            for p in range(P):
                if p < DVE_PREP:
                    keys.append(((p + 0.5) / DVE_PREP, 1, p))
                else:
                    keys.append(((p - DVE_PREP + 0.5) / (P - DVE_PREP), 0, p))
            p_order = [p for _, _, p in sorted(keys)]
            for p in p_order:
                re_p, rs_p = float(rev[p]), float(rsv[p])
                if p < DVE_PREP:
                    t1 = wpool.tile([128, AH], f32, tag="t1")
                    nc.vector.tensor_scalar(
                        out=t1[:], in0=rr[:], scalar1=-2.0 * rs_p,
                        scalar2=None, op0=AL.mult)
                    u = upool.tile([128, AH], f32, tag="u")
                    nc.vector.tensor_tensor(out=u[:], in0=t1[:], in1=r2[:],
                                            op=AL.add)
                    # exp(-re*u + (ln2 - re*rs^2)) = 2*exp(-re*(R-rs)^2)
                    ei = nc.scalar.activation(
                        out=rsf[:, p * AH:(p + 1) * AH], in_=u[:],
                        func=AF.Exp, scale=-re_p,
                        bias=bia[:, 48 + p:49 + p])
                else:
                    ua = wpool.tile([128, AH], f32, tag="ua")
                    nc.scalar.activation(out=ua[:], in_=rr[:], func=AF.Square,
                                         bias=bia[:, p:p + 1])
                    ei = nc.scalar.activation(
                        out=rsf[:, p * AH:(p + 1) * AH], in_=ua[:],
                        func=AF.Exp, scale=-re_p,
                        bias=bia[:, 97:98])
                last_exp = ei

            # ---- phase B: s = sin(pi/2 - pi*min(R,rc)/(2rc)) (>=0, LUT-safe)
            # rsf *= s*s  ->  rsf = 2*f_p.  Sins forced after all Exps so the
            # ACT table set switches exactly once; rcap pool is deep so DVE
            # computes sin args well ahead.
            for p in range(P):
                rc_p = float(rcv[p])
                rt = rpool.tile([128, AH], bf16, tag="rt")
                nc.vector.tensor_scalar(
                    out=rt[:], in0=rr[:], scalar1=rc_p,
                    scalar2=float(np.pi / (2.0 * rc_p)),
                    op0=AL.min, op1=AL.mult)
                cs = wpool.tile([128, AH], bf16, tag="cs")
                si = nc.scalar.activation(out=cs[:], in_=rt[:], func=AF.Sin,
                                          scale=-1.0, bias=bia[:, 96:97])
                add_dep_helper(si.ins, last_exp.ins,
                               reason="keep Sin phase after Exp phase")
                s2t = wpool.tile([128, AH], bf16, tag="s2t")
                if p < ACT_SQ:
                    # Square is in every table set -> no reload
                    nc.scalar.activation(out=s2t[:], in_=cs[:],
                                         func=AF.Square)
                else:
                    nc.vector.tensor_tensor(out=s2t[:], in0=cs[:], in1=cs[:],
                                            op=AL.mult)
                psl = rsf[:, p * AH:(p + 1) * AH]
                nc.vector.tensor_tensor(out=psl, in0=s2t[:], in1=psl,
                                        op=AL.mult)

            # ---- TensorE masked reduction; sym parked in SBUF (bf16)
            # group g = one a_hi; psum tile: rows 32*gp + (al*5+t5),
            # cols gf*48 + p, for g = ti*40 + gf*4 + gp
            sym = pool.tile([128, OUTW], bf16)
            rst = pool.tile([128, 260], f32)   # rows 0:8 = s1, 32:40 = s2
            mbc = pool.tile([128, 260], f32)
            ibc = pool.tile([128, 260], f32)
            sall = [None, None]
            GRP_T = 40

            def stats_half(h):
                # per-n stats already folded per tile; just ship + AllReduce
                c0, c1 = (0, HALF_T * 10) if h == 0 else (HALF_T * 10, 260)
                w = c1 - c0
                nc.sync.dma_start(out=st_in[h][:, 0:w], in_=rst[0:8, c0:c1])
                nc.sync.dma_start(out=st_in[h][:, w:2 * w],
                                  in_=rst[32:40, c0:c1])
                nc.gpsimd.collective_compute(
                    "AllReduce", AL.add,
                    ins=[st_in[h][:]], outs=[st_out[h][:]],
                    replica_groups=[list(range(NC_CORES))])
                sa = pool.tile([128, 2 * w], f32)
                nc.sync.dma_start(out=sa[0:8, :], in_=st_out[h][:])
                sall[h] = sa

            def epilogue_half(h):
                # mean/inv on [8,w], broadcast to 128 rows via matmul
                c0, c1 = (0, HALF_T * 10) if h == 0 else (HALF_T * 10, 260)
                w = c1 - c0
                sa = sall[h]
                inv_n = 1.0 / (B * C_OUT)
                mean = wpool.tile([128, 200], f32, tag="mean")
                nc.vector.tensor_scalar(out=mean[0:8, :w], in0=sa[0:8, 0:w],
                                        scalar1=inv_n, scalar2=None,
                                        op0=AL.mult)
                vpe = wpool.tile([128, 200], f32, tag="vpe")
                nc.vector.tensor_scalar(out=vpe[0:8, :w], in0=sa[0:8, w:2 * w],
                                        scalar1=inv_n, scalar2=None,
                                        op0=AL.mult)
                msq = wpool.tile([128, 200], f32, tag="msq")
                nc.vector.tensor_tensor(out=msq[0:8, :w], in0=mean[0:8, :w],
                                        in1=mean[0:8, :w], op=AL.mult)
                nc.vector.tensor_tensor(out=vpe[0:8, :w], in0=vpe[0:8, :w],
                                        in1=msq[0:8, :w], op=AL.subtract)
                nc.vector.tensor_scalar(out=vpe[0:8, :w], in0=vpe[0:8, :w],
                                        scalar1=float(4.0 * BN_EPS),
                                        scalar2=None, op0=AL.add)
                sdev = wpool.tile([128, 200], f32, tag="sdev")
                nc.scalar.activation(out=sdev[0:8, :w], in_=vpe[0:8, :w],
                                     func=AF.Sqrt)
                inv = wpool.tile([128, 200], f32, tag="inv")
                nc.vector.reciprocal(out=inv[0:8, :w], in_=sdev[0:8, :w])
                mbp = spool.tile([128, 200], f32, tag="mbp")
                ibp = spool.tile([128, 200], f32, tag="ibp")
                nc.tensor.matmul(out=mbp[:, :w], lhsT=selbc[0:8, :],
                                 rhs=mean[0:8, :w], start=True, stop=True)
                nc.tensor.matmul(out=ibp[:, :w], lhsT=selbc[0:8, :],
                                 rhs=inv[0:8, :w], start=True, stop=True)
                nc.vector.tensor_copy(out=mbc[:, c0:c1], in_=mbp[:, :w])
                nc.vector.tensor_copy(out=ibc[:, c0:c1], in_=ibp[:, :w])

            def normalize_tile(ti):
                nfgf = 10 if ti < 25 else 6
                ncol = nfgf * 48
                ssl = sym[:, ti * 480:ti * 480 + ncol]
                t1 = wpool.tile([128, 480], f32, tag="nt")
                nc.vector.tensor_tensor(
                    out=t1[:, :ncol], in0=ssl,
                    in1=_mk_ap(mbc[:], ti * 10, [[1, nfgf], [0, 48]]),
                    op=AL.subtract)
                nc.vector.tensor_tensor(
                    out=ssl, in0=t1[:, :ncol],
                    in1=_mk_ap(ibc[:], ti * 10, [[1, nfgf], [0, 48]]),
                    op=AL.mult)

            for ti in range(NTILE):
                ngrp = GRP_T if ti < 25 else 24
                nfgf = 10 if ti < 25 else 6
                ncol = nfgf * 48
                wmask = mpool.tile([128, GRP_T * NMC], bf16, tag="wmask")
                in0 = _mk_ap(zt[:], ti * GRP_T, [[1, ngrp], [0, NMC]])
                t0 = _mk_ap(tcode[:], 0, [[0, ngrp], [1, NMC]])
                nc.vector.tensor_tensor(
                    out=wmask[:, :ngrp * NMC], in0=in0, in1=t0, op=AL.is_equal)
                stp = ppool.tile([128, 480], f32, tag="stp")
                if ti < 3:
                    # 20-col masks leave psum rows 20..31 of each strip
                    # unwritten; clear each rotating buffer once so parked
                    # garbage is finite
                    nc.vector.memset(stp[:], 0.0)
                for gi in range(ngrp):
                    g = ti * GRP_T + gi
                    gp, gf = gi % 4, gi // 4
                    rhs = _mk_ap(rsf[:], g, [[AH, P]])
                    nc.tensor.matmul(
                        out=stp[32 * gp:32 * gp + 20, gf * 48:(gf + 1) * 48],
                        lhsT=wmask[:, gi * NMC:(gi + 1) * NMC],
                        rhs=rhs, start=True, stop=True,
                        tile_position=(0, 32 * gp))
                # park + square on idle ACT; fold rows via sel8 matmul, then
                # tiny per-gf reduces on the [8, ncol] results
                ssl = sym[:, ti * 480:ti * 480 + ncol]
                nc.scalar.activation(out=ssl, in_=stp[:, :ncol], func=AF.Copy)
                sqt = wpool.tile([128, 480], bf16, tag="sqt")
                nc.scalar.activation(out=sqt[:, :ncol], in_=stp[:, :ncol],
                                     func=AF.Square)
                y1 = ypool.tile([128, 480], f32, tag="y1")
                if ti < 3:
                    nc.vector.memset(y1[:], 0.0)
                nc.tensor.matmul(out=y1[0:8, :ncol], lhsT=sel8[:, 0:8],
                                 rhs=ssl, start=True, stop=True)
                nc.tensor.matmul(out=y1[32:40, :ncol], lhsT=sel8[:, 0:8],
                                 rhs=sqt[:, :ncol], start=True, stop=True,
                                 tile_position=(0, 32))
                nc.vector.tensor_reduce(
                    out=rst[0:40, ti * 10:ti * 10 + nfgf],
                    in_=_mk_ap(y1[0:40, :], 0, [[48, nfgf], [1, 48]]),
                    axis=mybir.AxisListType.X, op=AL.add)
                if ti == HALF_T - 1:
                    stats_half(0)
                    epilogue_half(0)
            stats_half(1)
            for ti in range(HALF_T):
                normalize_tile(ti)
            ch = HALF_T * 480
            nc.sync.dma_start(out=out_ext[:, 0:ch], in_=sym[:, 0:ch])
            epilogue_half(1)
            for ti in range(HALF_T, NTILE):
                normalize_tile(ti)
            nc.sync.dma_start(out=out_ext[:, ch:OUTW], in_=sym[:, ch:OUTW])

    _fix_sync_waits(nc, spares, relay_sem)
    return nc


# ---------------------------------------------------------------- host driver
def _host_tables(rsv, rev):
    from ml_dtypes import bfloat16
    LN2 = float(np.log(2.0))
    tcode = np.full((128, NMC), -1.0, np.float32)
    for al in range(4):
        for t5 in range(T):
            tcode[al * 32:(al + 1) * 32, al * 5 + t5] = float(ATOM_TYPES[t5])
    sel8 = np.zeros((128, 8), np.float32)
    selbc = np.zeros((8, 128), np.float32)
    for gp in range(4):
        for al in range(4):
            for t5 in range(T):
                row = 32 * gp + 5 * al + t5
                col = 4 * (al % 2) + gp
                sel8[row, col] = 1.0
                selbc[col, row] = 1.0
    biases = np.zeros((128, 98), np.float32)
    for p in range(P):
        biases[:, p] = -float(rsv[p])
        biases[:, 48 + p] = LN2 - float(rev[p]) * float(rsv[p]) ** 2
    biases[:, 96] = float(np.pi / 2.0)
    biases[:, 97] = LN2
    # output unscramble: [128, OUTW] -> [4096, 240]
    a = np.arange(A)
    al = a // AH
    ah = a % AH
    ti = ah // 40
    rem = ah % 40
    gf = rem // 4
    gp = rem % 4
    c = np.arange(C_OUT)
    t5 = c // P
    p = c % P
    rows = (32 * gp[:, None] + 5 * al[:, None] + t5[None, :]).astype(np.int64)
    cols = ((ti * 480 + gf * 48)[:, None] + p[None, :]).astype(np.int64)
    return (tcode.astype(bfloat16), sel8.astype(bfloat16), selbc, biases,
            rows, cols)


def kernel(X, rc, rs, re, Nbrs, Nbrs_Z):
    from ml_dtypes import bfloat16
    X = np.asarray(X, np.float32)
    rc = np.asarray(rc, np.float32).ravel()
    rs = np.asarray(rs, np.float32).ravel()
    re = np.asarray(re, np.float32).ravel()
    Nbrs = np.asarray(Nbrs, np.int32)
    Nbrs_Z = np.asarray(Nbrs_Z, np.int32)

    nc = build_nc(rc, rs, re)
    tcode, sel8, selbc, biases, orows, ocols = _host_tables(rs, re)

    # per-(a,m)-tile layouts: partition p = (a//1024)*32 + m, free = a % 1024
    in_maps = []
    for core in range(NC_CORES):
        bsl = slice(core * B_LOC, (core + 1) * B_LOC)
        Xc = X[bsl].reshape(A, 3)                       # a = b_loc*2048 + n
        Nb = Nbrs[bsl].reshape(A, M)
        Zb = Nbrs_Z[bsl].reshape(A, M)
        gidx = Nb + (np.arange(A)[:, None] // N) * N    # [A, M] global rows
        a_hi = np.arange(A) % AH
        part = ((np.arange(A) // AH)[:, None] * 32
                + np.arange(M)[None]).astype(np.int32)
        zt = np.zeros((128, AH), np.float32)
        zt[part.ravel(), np.repeat(a_hi, M)] = Zb.ravel().astype(np.float32)
        D = Xc[gidx] - Xc[:, None, :]                   # [A, M, 3]
        Rv = np.sqrt(np.sum(D * D, axis=2, dtype=np.float32))
        rr = np.zeros((128, AH), np.float32)
        rr[part.ravel(), np.repeat(a_hi, M)] = Rv.ravel()
        in_maps.append({
            "rr": rr, "zt": zt.astype(bfloat16), "tcode": tcode,
            "sel8": sel8, "selbc": selbc, "biases": biases,
        })

    res = run_bass_kernel_spmd(nc, in_maps, core_ids=list(range(NC_CORES)),
                               trace=_TRACE[0])
    if _TRACE[0]:
        kernel.last_exec_ns = res.exec_time_ns
        kernel.last_profile = res

    out = np.zeros((B, N, C_OUT), np.float32)
    for core in range(NC_CORES):
        o = np.asarray(res.results[core]["out"]).astype(np.float32)
        out[core * B_LOC:(core + 1) * B_LOC] = o[orows, ocols].reshape(
            B_LOC, N, C_OUT)
    return out
